# revision 56
# baseline (speedup 1.0000x reference)
"""Trainium2 Bass kernel for nn_Attention_60567628808865.

Dense transformer attention block (B=4, S=1024, H=4096, NH=32, D=128):
  qkv = x @ W_pack; RoPE(q, k); causal-masked softmax attention; out @ W_o.

Sharding: tensor-parallel over heads across 8 NeuronCores. Each core computes
4 heads end-to-end; the host sums the 8 partial W_o outputs (row-sharded W_o).

Precision/performance scheme (validated on host to rel_err ~2.7e-3 vs the
2e-2 gate):
  - QKV and W_o projections run in fp8(e4m3) with the DoubleRow perf mode
    (K=256 per instruction, 0.5 cycles/row) using an exact-style two-term
    decomposition: x@W ~= x_hi@W_hi + [x_hi@W_lo + x_lo@W_hi], where
    t_hi = fp8(t*s), t_lo = fp8(t*s - t_hi). Both terms accumulate into ONE
    PSUM chain (identical scale), so the epilogue is unchanged. 48 DoubleRow
    instructions replace 32 f32r instructions per [128col x 256tok] unit:
    0.75x PE cycles.
  - hi/lo operands are slot-interleaved in a single packed tensor
    ([part, chunk, 2, free]) so the correction chain reads (hi,lo) slot pairs
    and the main chain reads (hi,hi) chunk pairs from the same SBUF bytes.
  - Attention is causal-aware: score/PV/denominator work is emitted only for
    the 20/32 key-tile x query-block units on or below the diagonal; the two
    diagonal tiles per query block are masked multiplicatively with a
    host-built exp(mask) pattern (asserted causal). q/k/v round-trip DRAM in
    bf16; scores/PV matmuls run in bf16 (same PE rate as f32r, half the DMA).
  - Softmax is unnormalized; denominators come from a ones-vector matmul
    accumulated in PSUM, broadcast back via a K=1 matmul; attention output is
    quantized to fp8 hi/lo pairs on the fly for the W_o DoubleRow chain.
  - Output partials are stored bf16; the host sum applies the global descale.
"""
import numpy as np

import concourse.bass as bass  # noqa: F401
import concourse.tile as tile
from contextlib import ExitStack
from concourse import bacc, mybir
from concourse import bass_utils

F32 = mybir.dt.float32
F32R = mybir.dt.float32r
BF16 = mybir.dt.bfloat16
F8 = mybir.dt.float8e4
AF = mybir.ActivationFunctionType
ALU = mybir.AluOpType
DR = mybir.MatmulPerfMode.DoubleRow

B, S, H, NH = 4, 1024, 4096, 32
D = H // NH          # 128
T = B * S            # 4096 tokens
N_CORES = 8
HC = NH // N_CORES   # 4 heads per core
SCALE = float(1.0 / np.sqrt(D))
ROPE_BASE = 10000.0

TB = 256             # phase-1 token block
NTB = T // TB        # 16
KC = H // 128        # 32 fp8 k-chunks of 128 features
S_X = 32.0           # x quant scale
S_W = 2048.0         # W_pack / W_o quant scale
S_A = 32.0           # attention-output quant scale
DESCALE = 1.0 / (S_X * S_W)

_CACHE = {}


def _build_module(phases=("p1", "p2")):
    nc = bacc.Bacc("TRN2", target_bir_lowering=False, debug=False,
                   num_devices=N_CORES)

    # packed fp8 inputs (see _host_prep for layouts)
    xq = nc.dram_tensor("xq", [128, NTB * KC * 2 * TB], F8, kind="ExternalInput").ap()
    wqk = nc.dram_tensor("wqk", [128, 8 * KC * 2 * 128], F8, kind="ExternalInput").ap()
    wv = nc.dram_tensor("wv", [128, 2 * KC * 2 * 256], F8, kind="ExternalInput").ap()
    wo = nc.dram_tensor("wo", [128, HC * 2 * H], F8, kind="ExternalInput").ap()
    cosT = nc.dram_tensor("cosT", [128, T], F32, kind="ExternalInput").ap()
    sinS = nc.dram_tensor("sinS", [128, T], F32, kind="ExternalInput").ap()
    maskD = nc.dram_tensor("maskD", [128, 512], BF16, kind="ExternalInput").ap()
    out_p = nc.dram_tensor("out_p", [T, H], BF16, kind="ExternalOutput").ap()

    import ml_dtypes
    ones128 = nc.inline_tensor(
        np.ones((128, 1), ml_dtypes.bfloat16), "ones128").ap()
    onesS = nc.inline_tensor(
        np.full((1, 128), S_A, np.float32), "onesS").ap().bitcast(F32R)

    with tile.TileContext(nc) as tc, \
         nc.allow_low_precision(reason="fp8/bf16 matmuls; verified vs reference"):
        with ExitStack() as octx:
            dram = octx.enter_context(tc.tile_pool(name="dram", bufs=1, space="DRAM"))
            cpool = octx.enter_context(tc.tile_pool(name="consts", bufs=1))
            # DRAM scratch: qkT rows ordered [q0,k0,q1,k1,q2,k2,q3,k3] x d
            qkT_d = dram.tile([8 * 128, T], BF16)
            v_d = dram.tile([T, HC * 128], BF16)

            o128 = cpool.tile([128, 1], BF16)
            nc.sync.dma_start(o128[:], ones128[:])
            oS = cpool.tile([1, 128], F32R)
            nc.sync.dma_start(oS[:], onesS[:])
            mask_t = cpool.tile([128, 512], BF16)
            nc.sync.dma_start(mask_t[:], maskD[:])

            # phase-2 tiles prefetched during phase 1 (wo_a has no deps; the
            # first head's kq/vt depend on the tb0-3 scratch stores)
            wopool = octx.enter_context(tc.tile_pool(name="p2wo", bufs=1))
            kqpool = octx.enter_context(tc.tile_pool(name="p2kq", bufs=2))
            vtpool = octx.enter_context(tc.tile_pool(name="p2vt", bufs=2))
            _wo_a = [None]
            _first_kv = [None]

            def load_kv(b, l):
                bs = b * S
                kq = kqpool.tile([128, 2, S], BF16, tag="kq")
                nc.sync.dma_start(
                    kq[:],
                    qkT_d[l * 256:(l + 1) * 256, bs:bs + S]
                        .rearrange("(j p) t -> p j t", p=128))
                vt = vtpool.tile([128, 8, 128], BF16, tag="vt")
                nc.sync.dma_start(
                    vt[:],
                    v_d[bs:bs + S, l * 128:(l + 1) * 128]
                        .rearrange("(kt p) d -> p kt d", p=128))
                return kq, vt

            def prefetch_wo():
                # W_o resident: [128, h(4), j(2), c(4096)]; j=0 -> hi, 1 -> lo
                wo_a = wopool.tile([128, HC, 2, H], F8, tag="wo")
                for h in range(HC):
                    nc.sync.dma_start(
                        wo_a[:, h],
                        wo[:, h * 2 * H:(h + 1) * 2 * H]
                            .rearrange("p (j c) -> p j c", j=2))
                _wo_a[0] = wo_a

            def prefetch_kv():
                _first_kv[0] = load_kv(0, 0)

            # ---------------- Phase 1: QKV projection (fp8 DoubleRow) -------
            if "p1" in phases:
              with ExitStack() as ctx:
                wpool = ctx.enter_context(tc.tile_pool(name="p1w", bufs=1))
                xpool = ctx.enter_context(tc.tile_pool(name="p1x", bufs=2))
                opool = ctx.enter_context(tc.tile_pool(name="p1o", bufs=2))
                cspool = ctx.enter_context(tc.tile_pool(name="p1cs", bufs=2))
                rpool = ctx.enter_context(tc.tile_pool(name="p1rope", bufs=3))
                pqk = ctx.enter_context(tc.tile_pool(name="p1pqk", bufs=4, space="PSUM"))
                pv = ctx.enter_context(tc.tile_pool(name="p1pv", bufs=2, space="PSUM"))

                def load_tb(tb):
                    t0 = tb * TB
                    # x pack [128, kk(32), j(2), t(256)]; j=0 -> x_hi, j=1 -> x_lo
                    xall = xpool.tile([128, KC, 2, TB], F8, tag="x")
                    nc.sync.dma_start(
                        xall[:],
                        xq[:, tb * 16384:(tb + 1) * 16384]
                            .rearrange("p (kk j t) -> p kk j t", kk=KC, j=2))
                    cos_tb = cspool.tile([128, TB], F32, tag="cos")
                    nc.sync.dma_start(cos_tb[:], cosT[:, t0:t0 + TB])
                    sin_tb = cspool.tile([128, TB], F32, tag="sin")
                    nc.sync.dma_start(sin_tb[:], sinS[:, t0:t0 + TB])
                    return xall, cos_tb, sin_tb

                # tb0 inputs first (first chain needs x + wqk ct0 only), then
                # resident weights: wqk [128, ct(8), kk(32), j(2), c(128)],
                # wv [128, ct(2), kk(32), j(2), c(256)]; j=0 -> W_lo, j=1 -> W_hi
                tb0_inputs = load_tb(0)
                wqk_a = wpool.tile([128, 8, KC, 2, 128], F8, tag="wqk")
                wv_a = wpool.tile([128, 2, KC, 2, 256], F8, tag="wv")
                for ct in range(8):
                    nc.sync.dma_start(
                        wqk_a[:, ct],
                        wqk[:, ct * 8192:(ct + 1) * 8192]
                            .rearrange("p (kk j c) -> p kk j c", kk=KC, j=2))
                for cv in range(2):
                    nc.sync.dma_start(
                        wv_a[:, cv],
                        wv[:, cv * 16384:(cv + 1) * 16384]
                            .rearrange("p (kk j c) -> p kk j c", kk=KC, j=2))

                def emit_qk(xall, cos_tb, sin_tb, t0):
                    qs_all = opool.tile([128, 8, TB], BF16, tag="qs")
                    for i in range(8):
                        ps = pqk.tile([128, TB], F32, tag="qk")
                        for c in range(16):
                            nc.tensor.matmul(
                                ps[:], wqk_a[:, i, 2 * c:2 * c + 2, 1, :],
                                xall[:, 2 * c:2 * c + 2, 0, :],
                                start=(c == 0), stop=False, perf_mode=DR)
                        for kk in range(KC):
                            nc.tensor.matmul(
                                ps[:], wqk_a[:, i, kk, :, :],
                                xall[:, kk, :, :],
                                start=False, stop=(kk == KC - 1), perf_mode=DR)
                        # RoPE epilogue (psum scale folded into cos/sin tables)
                        rot = rpool.tile([128, TB], F32, tag="rot")
                        nc.scalar.copy(rot[0:64, :], ps[64:128, :])
                        nc.vector.tensor_copy(rot[64:128, :], ps[0:64, :])
                        m1_ = rpool.tile([128, TB], F32, tag="m1")
                        nc.vector.tensor_tensor(m1_[:], ps[:], cos_tb[:], op=ALU.mult)
                        m2_ = rpool.tile([128, TB], F32, tag="m2")
                        nc.vector.tensor_tensor(m2_[:], rot[:], sin_tb[:], op=ALU.mult)
                        nc.vector.tensor_tensor(qs_all[:, i, :], m1_[:], m2_[:],
                                                op=ALU.add)
                    nc.sync.dma_start(
                        qkT_d[:, t0:t0 + TB].rearrange("(i p) t -> p i t", p=128),
                        qs_all[:])

                def emit_v(xall, t0):
                    vs_all = opool.tile([128, 2, 2, 256], BF16, tag="vs")
                    for th in range(2):
                        for ch in range(2):
                            ps = pv.tile([128, 256], F32, tag="v")
                            for c in range(16):
                                nc.tensor.matmul(
                                    ps[:],
                                    xall[:, 2 * c:2 * c + 2, 0,
                                         th * 128:(th + 1) * 128],
                                    wv_a[:, ch, 2 * c:2 * c + 2, 1, :],
                                    start=(c == 0), stop=False, perf_mode=DR)
                            for kk in range(KC):
                                nc.tensor.matmul(
                                    ps[:],
                                    xall[:, kk, :, th * 128:(th + 1) * 128],
                                    wv_a[:, ch, kk, :, :],
                                    start=False, stop=(kk == KC - 1), perf_mode=DR)
                            nc.scalar.activation(vs_all[:, th, ch, :], ps[:],
                                                 AF.Copy, scale=DESCALE)
                    nc.sync.dma_start(
                        v_d[t0:t0 + TB, :]
                            .rearrange("(th p) (ch c) -> p th ch c", p=128, ch=2),
                        vs_all[:])

                # v(0) is deferred until after qk(1): tb0's PE work then
                # needs only x+wqk, hiding the wv weight-load latency
                deferred_v0 = [None]
                for tb in range(NTB):
                    t0 = tb * TB
                    if tb == 0:
                        xall, cos_tb, sin_tb = tb0_inputs
                    else:
                        xall, cos_tb, sin_tb = load_tb(tb)
                    if tb == 1:
                        prefetch_wo()
                    elif tb == 4:
                        prefetch_kv()
                    emit_qk(xall, cos_tb, sin_tb, t0)
                    if tb == 0:
                        deferred_v0[0] = (xall, t0)
                    else:
                        if deferred_v0[0] is not None:
                            emit_v(*deferred_v0[0])
                            deferred_v0[0] = None
                        emit_v(xall, t0)

            # ---------------- Phase 2: attention + W_o ----------------------
            if "p2" in phases:
              with ExitStack() as ctx:
                apool = ctx.enter_context(tc.tile_pool(name="p2a", bufs=2))
                efpool = ctx.enter_context(tc.tile_pool(name="p2ef", bufs=14))
                tpool = ctx.enter_context(tc.tile_pool(name="p2t", bufs=3))
                rpool2 = ctx.enter_context(tc.tile_pool(name="p2rd", bufs=3))
                opool = ctx.enter_context(tc.tile_pool(name="p2o", bufs=2))
                ps_s = ctx.enter_context(tc.tile_pool(name="p2ps", bufs=2, space="PSUM"))
                ps_av = ctx.enter_context(tc.tile_pool(name="p2pav", bufs=1, space="PSUM"))
                ps_d = ctx.enter_context(tc.tile_pool(name="p2pd", bufs=1, space="PSUM"))
                ps_o = ctx.enter_context(tc.tile_pool(name="p2po", bufs=2, space="PSUM"))

                if _wo_a[0] is None:     # p2-only debug build
                    prefetch_wo()
                    prefetch_kv()
                wo_a = _wo_a[0]
                for b in range(B):
                    bs = b * S
                    # attn pack [128, lh(2), l(4), t(1024)]; lh=0 -> lo, 1 -> hi
                    apack = apool.tile([128, 2, HC, S], F8, tag="apack")
                    pending = [None]
                    for l in range(HC):
                        if b == 0 and l == 0:
                            kq, vt = _first_kv[0]
                        else:
                            kq, vt = load_kv(b, l)

                        psd_l = ps_d.tile([1, 512], F32, tag="dbc")
                        psav_l = ps_av.tile([128, 512], F32, tag="av")
                        all_efs = {}

                        def emit_scores(qb):
                            u = 2 * qb + 2
                            q_sl = kq[:, 0, qb * 256:(qb + 1) * 256]
                            efs = [None] * u
                            # diagonal pair first: its exp+mask latency hides
                            # behind the remaining pairs' matmuls
                            for g in [qb] + list(range(qb)):
                                pss = ps_s.tile([128, 512], F32, tag="s")
                                for sHalf in range(2):
                                    mt = 2 * g + sHalf
                                    nc.tensor.matmul(
                                        pss[:, sHalf * 256:(sHalf + 1) * 256],
                                        kq[:, 1, mt * 128:(mt + 1) * 128],
                                        q_sl, start=True, stop=True)
                                ef = efpool.tile([128, 512], BF16, tag="ef")
                                nc.scalar.activation(ef[:], pss[:], AF.Exp,
                                                     scale=SCALE)
                                if g == qb:  # diagonal: multiplicative mask.
                                    # qb0 is latency-critical (its PV block is
                                    # all-diagonal and starts soon after): use
                                    # DVE; the rest have slack, use Pool
                                    efm = efpool.tile([128, 512], BF16, tag="efm")
                                    eng = nc.vector if qb == 0 else nc.gpsimd
                                    eng.tensor_tensor(efm[:], ef[:],
                                                      mask_t[:], op=ALU.mult)
                                    ef = efm
                                efs[2 * g] = ef[:, 0:256]
                                efs[2 * g + 1] = ef[:, 256:512]
                            all_efs[qb] = efs

                        def emit_pv(qb):
                            u = 2 * qb + 2
                            efs = all_efs.pop(qb)
                            # masked diagonal units last in the chains
                            order = list(range(2 * qb)) + [2 * qb, 2 * qb + 1]
                            if pending[0] is not None:
                                pending[0]()
                                pending[0] = None
                            psav = psav_l[:, (qb % 2) * 256:(qb % 2 + 1) * 256]
                            for n, mt in enumerate(order):
                                nc.tensor.matmul(
                                    psav, vt[:, mt, :], efs[mt],
                                    start=(n == 0), stop=(n == u - 1))
                            psd = psd_l[:, (qb % 2) * 256:(qb % 2 + 1) * 256]
                            for n, mt in enumerate(order):
                                nc.tensor.matmul(
                                    psd, o128[:], efs[mt],
                                    start=(n == 0), stop=(n == u - 1))
                            if qb % 2 == 0:
                                return
                            # pair epilogue (qb-1, qb): unnormalized attn to
                            # SBUF (frees the psum bank), denominators to
                            # reciprocal; the normalization + fp8 hi/lo split
                            # is deferred into the next PV block
                            rd = rpool2.tile([1, 512], F32R, tag="rd")
                            nc.vector.reciprocal(rd[:], psd_l[:])
                            av_s = tpool.tile([128, 512], F32, tag="avs")
                            nc.vector.tensor_copy(av_s[:], psav_l[:])

                            def make_epilogue(qb=qb, av_s=av_s, rd=rd, l=l,
                                              apack=apack):
                                def emit():
                                    psbc = ps_o.tile([128, 512], F32, tag="o")
                                    nc.tensor.matmul(psbc[:], oS[:], rd[:],
                                                     start=True, stop=True)
                                    t_ = tpool.tile([128, 512], F32, tag="t")
                                    nc.vector.tensor_tensor(t_[:], av_s[:],
                                                            psbc[:], op=ALU.mult)
                                    q0 = (qb - 1) * 256
                                    hi = apack[:, 1, l, q0:q0 + 512]
                                    nc.vector.tensor_copy(hi, t_[:])
                                    nc.gpsimd.tensor_tensor(
                                        apack[:, 0, l, q0:q0 + 512], t_[:], hi,
                                        op=ALU.subtract)
                                return emit
                            pending[0] = make_epilogue()

                        # scores run two query-blocks ahead of PV/denominator
                        emit_scores(0)
                        emit_scores(1)
                        emit_scores(2)
                        emit_pv(0)
                        emit_scores(3)
                        emit_pv(1)
                        emit_pv(2)
                        emit_pv(3)
                    # W_o projection for batch b (fp8 DoubleRow main+corr);
                    # two 256-col chains per PSUM bank, one wide copy each.
                    # The last pair epilogue (l=3, qb 2-3) flushes after the
                    # first m-block, which only reads early tokens.
                    for m in range(8):
                        osb = opool.tile([128, 4, 1024], BF16, tag="osb")
                        msl = slice(m * 128, (m + 1) * 128)
                        for quad in range(4):
                            pso = ps_o.tile([128, 1024], F32, tag="o")
                            for part in range(4):
                                csl = slice((4 * quad + part) * 256,
                                            (4 * quad + part + 1) * 256)
                                po = pso[:, part * 256:(part + 1) * 256]
                                for c in range(2):
                                    nc.tensor.matmul(
                                        po, apack[:, 1, 2 * c:2 * c + 2, msl],
                                        wo_a[:, 2 * c:2 * c + 2, 0, csl],
                                        start=(c == 0), stop=False, perf_mode=DR)
                                for h in range(HC):
                                    nc.tensor.matmul(
                                        po, apack[:, :, h, msl],
                                        wo_a[:, h, :, csl],
                                        start=False, stop=(h == HC - 1),
                                        perf_mode=DR)
                            if quad % 2 == 0:
                                nc.vector.tensor_copy(osb[:, quad, :], pso[:])
                            else:
                                nc.scalar.copy(osb[:, quad, :], pso[:])
                        nc.sync.dma_start(
                            out_p[bs + m * 128:bs + (m + 1) * 128, :],
                            osb[:].rearrange("p nc c -> p (nc c)"))
                        if m == 0 and pending[0] is not None:
                            pending[0]()
                            pending[0] = None
    nc.compile()
    return nc


def _q8hl(a, scale):
    """Quantize to fp8 e4m3 hi/lo pair at a shared scale."""
    import ml_dtypes
    hi = (a * scale).astype(ml_dtypes.float8_e4m3)
    lo = ((a * scale) - hi.astype(np.float32)).astype(ml_dtypes.float8_e4m3)
    return hi, lo


def _host_prep(hidden_states, W_pack, W_o, attention_mask, position_ids):
    import ml_dtypes
    x = np.asarray(hidden_states, dtype=np.float32).reshape(T, H)
    W_pack = np.asarray(W_pack, dtype=np.float32)
    W_o = np.asarray(W_o, dtype=np.float32)
    mask = np.asarray(attention_mask, dtype=np.float32)
    pos = np.asarray(position_ids)

    # causal structure is hardcoded in the kernel; verify it holds
    m0 = mask[0, 0]
    iu = np.triu_indices(S, 1)
    assert (m0[iu] < -1e8).all() and (np.tril(m0) == 0).all(), \
        "kernel requires the standard causal mask"

    # x pack: [128p, tb, kk, j(hi,lo), t] -> flat [128, NTB*KC*2*TB]
    xh, xl = _q8hl(x, S_X)
    xv_h = xh.reshape(NTB, TB, KC, 128).transpose(3, 0, 2, 1)
    xv_l = xl.reshape(NTB, TB, KC, 128).transpose(3, 0, 2, 1)
    xq_np = np.empty((128, NTB, KC, 2, TB), ml_dtypes.float8_e4m3)
    xq_np[:, :, :, 0, :] = xv_h
    xq_np[:, :, :, 1, :] = xv_l
    xq_np = np.ascontiguousarray(xq_np.reshape(128, -1))

    # rope tables with the fp8 descale folded in; rotate-half sign in sinS
    inv = 1.0 / (ROPE_BASE ** (np.arange(0, D, 2, dtype=np.float64) / D))
    inv = np.concatenate([inv, inv])
    ang = pos.astype(np.float64).reshape(T)[None, :] * inv[:, None]   # [D, T]
    cosT_np = np.ascontiguousarray((np.cos(ang) * DESCALE).astype(np.float32))
    sinT = (np.sin(ang) * DESCALE).astype(np.float32)
    sinS_np = sinT.copy()
    sinS_np[:64] = -sinT[:64]
    sinS_np = np.ascontiguousarray(sinS_np)

    # diagonal exp-mask patterns [128p(key), s(2)*256(query)] bf16
    em = np.exp(m0)
    maskD_np = np.empty((128, 2, 256), ml_dtypes.bfloat16)
    maskD_np[:, 0, :] = em[0:256, 0:128].T       # offset 0 pattern
    maskD_np[:, 1, :] = em[0:256, 128:256].T     # offset 128 pattern
    maskD_np = np.ascontiguousarray(maskD_np.reshape(128, 512))

    in_maps = []
    for core in range(N_CORES):
        h0 = core * HC
        # wqk cols ordered [q0,k0,q1,k1,q2,k2,q3,k3] per head slice
        cols = []
        for l in range(HC):
            cols.append(W_pack[:, (h0 + l) * D:(h0 + l + 1) * D])
            cols.append(W_pack[:, H + (h0 + l) * D:H + (h0 + l + 1) * D])
        wqk_f = np.concatenate(cols, axis=1)              # [H, 1024]
        wh, wl = _q8hl(wqk_f, S_W)
        wv_h = wh.reshape(KC, 128, 8, 128).transpose(1, 2, 0, 3)
        wv_l = wl.reshape(KC, 128, 8, 128).transpose(1, 2, 0, 3)
        wqk_np = np.empty((128, 8, KC, 2, 128), ml_dtypes.float8_e4m3)
        wqk_np[:, :, :, 0, :] = wv_l
        wqk_np[:, :, :, 1, :] = wv_h
        wqk_np = np.ascontiguousarray(wqk_np.reshape(128, -1))

        wv_f = np.concatenate(
            [W_pack[:, 2 * H + (h0 + l) * D:2 * H + (h0 + l + 1) * D]
             for l in range(HC)], axis=1)                 # [H, 512]
        wh, wl = _q8hl(wv_f, S_W)
        wvv_h = wh.reshape(KC, 128, 2, 256).transpose(1, 2, 0, 3)
        wvv_l = wl.reshape(KC, 128, 2, 256).transpose(1, 2, 0, 3)
        wv_np = np.empty((128, 2, KC, 2, 256), ml_dtypes.float8_e4m3)
        wv_np[:, :, :, 0, :] = wvv_l
        wv_np[:, :, :, 1, :] = wvv_h
        wv_np = np.ascontiguousarray(wv_np.reshape(128, -1))

        wo_f = W_o[h0 * D:(h0 + HC) * D, :]               # [512, H]
        wh, wl = _q8hl(wo_f, S_W)
        wov_h = wh.reshape(HC, 128, H).transpose(1, 0, 2)
        wov_l = wl.reshape(HC, 128, H).transpose(1, 0, 2)
        wo_np = np.empty((128, HC, 2, H), ml_dtypes.float8_e4m3)
        wo_np[:, :, 0, :] = wov_h
        wo_np[:, :, 1, :] = wov_l
        wo_np = np.ascontiguousarray(wo_np.reshape(128, -1))

        in_maps.append({
            "xq": xq_np, "wqk": wqk_np, "wv": wv_np, "wo": wo_np,
            "cosT": cosT_np, "sinS": sinS_np, "maskD": maskD_np,
        })
    return in_maps


def kernel(hidden_states, W_pack, W_o, attention_mask, position_ids):
    if "nc" not in _CACHE:
        _CACHE["nc"] = _build_module()
    nc = _CACHE["nc"]
    in_maps = _host_prep(hidden_states, W_pack, W_o, attention_mask, position_ids)
    res = bass_utils.run_bass_kernel_spmd(nc, in_maps, core_ids=list(range(N_CORES)))
    out = res.results[0]["out_p"].astype(np.float32)
    for c in range(1, N_CORES):
        out += res.results[c]["out_p"]
    out *= 1.0 / (S_A * S_W)
    return out.reshape(B, S, H).astype(np.float32)


# revision 66
# speedup vs baseline: 1.0066x; 1.0066x over previous
"""Trainium2 Bass kernel for nn_Attention_60567628808865.

Dense transformer attention block (B=4, S=1024, H=4096, NH=32, D=128):
  qkv = x @ W_pack; RoPE(q, k); causal-masked softmax attention; out @ W_o.

Sharding: tensor-parallel over heads across 8 NeuronCores. Each core computes
4 heads end-to-end; the host sums the 8 partial W_o outputs (row-sharded W_o).

Precision/performance scheme (validated on host to rel_err ~2.7e-3 vs the
2e-2 gate):
  - QKV and W_o projections run in fp8(e4m3) with the DoubleRow perf mode
    (K=256 per instruction, 0.5 cycles/row) using an exact-style two-term
    decomposition: x@W ~= x_hi@W_hi + [x_hi@W_lo + x_lo@W_hi], where
    t_hi = fp8(t*s), t_lo = fp8(t*s - t_hi). Both terms accumulate into ONE
    PSUM chain (identical scale), so the epilogue is unchanged. 48 DoubleRow
    instructions replace 32 f32r instructions per [128col x 256tok] unit:
    0.75x PE cycles.
  - hi/lo operands are slot-interleaved in a single packed tensor
    ([part, chunk, 2, free]) so the correction chain reads (hi,lo) slot pairs
    and the main chain reads (hi,hi) chunk pairs from the same SBUF bytes.
  - Attention is causal-aware: score/PV/denominator work is emitted only for
    the 20/32 key-tile x query-block units on or below the diagonal; the two
    diagonal tiles per query block are masked multiplicatively with a
    host-built exp(mask) pattern (asserted causal). q/k/v round-trip DRAM in
    bf16; scores/PV matmuls run in bf16 (same PE rate as f32r, half the DMA).
  - Softmax is unnormalized; denominators come from a ones-vector matmul
    accumulated in PSUM, broadcast back via a K=1 matmul; attention output is
    quantized to fp8 hi/lo pairs on the fly for the W_o DoubleRow chain.
  - Output partials are stored bf16; the host sum applies the global descale.
"""
import numpy as np

import concourse.bass as bass  # noqa: F401
import concourse.tile as tile
from contextlib import ExitStack
from concourse import bacc, mybir
from concourse import bass_utils

F32 = mybir.dt.float32
F32R = mybir.dt.float32r
BF16 = mybir.dt.bfloat16
F8 = mybir.dt.float8e4
AF = mybir.ActivationFunctionType
ALU = mybir.AluOpType
DR = mybir.MatmulPerfMode.DoubleRow

B, S, H, NH = 4, 1024, 4096, 32
D = H // NH          # 128
T = B * S            # 4096 tokens
N_CORES = 8
HC = NH // N_CORES   # 4 heads per core
SCALE = float(1.0 / np.sqrt(D))
ROPE_BASE = 10000.0

TB = 256             # phase-1 token block
NTB = T // TB        # 16
KC = H // 128        # 32 fp8 k-chunks of 128 features
S_X = 32.0           # x quant scale
S_W = 2048.0         # W_pack / W_o quant scale
S_A = 32.0           # attention-output quant scale
DESCALE = 1.0 / (S_X * S_W)

_CACHE = {}


def _build_module(phases=("p1", "p2")):
    nc = bacc.Bacc("TRN2", target_bir_lowering=False, debug=False,
                   num_devices=N_CORES)

    # packed fp8 inputs (see _host_prep for layouts)
    xq = nc.dram_tensor("xq", [128, NTB * KC * 2 * TB], F8, kind="ExternalInput").ap()
    wqk = nc.dram_tensor("wqk", [128, 8 * KC * 2 * 128], F8, kind="ExternalInput").ap()
    wv = nc.dram_tensor("wv", [128, 2 * KC * 2 * 256], F8, kind="ExternalInput").ap()
    wo = nc.dram_tensor("wo", [128, HC * 2 * H], F8, kind="ExternalInput").ap()
    cosT = nc.dram_tensor("cosT", [128, T], F32, kind="ExternalInput").ap()
    sinS = nc.dram_tensor("sinS", [128, T], F32, kind="ExternalInput").ap()
    maskD = nc.dram_tensor("maskD", [128, 256], BF16, kind="ExternalInput").ap()
    out_p = nc.dram_tensor("out_p", [T, H], BF16, kind="ExternalOutput").ap()

    import ml_dtypes
    # denominator ones-vector carries 1/S_A so the reciprocal yields
    # S_A/denom directly (the fp8 attn quant scale)
    ones128 = nc.inline_tensor(
        np.full((128, 1), 1.0 / S_A, ml_dtypes.bfloat16), "ones128").ap()

    with tile.TileContext(nc) as tc, \
         nc.allow_low_precision(reason="fp8/bf16 matmuls; verified vs reference"):
        with ExitStack() as octx:
            dram = octx.enter_context(tc.tile_pool(name="dram", bufs=1, space="DRAM"))
            cpool = octx.enter_context(tc.tile_pool(name="consts", bufs=1))
            # DRAM scratch: qkT rows ordered [q0,k0,q1,k1,q2,k2,q3,k3] x d
            qkT_d = dram.tile([8 * 128, T], BF16)
            v_d = dram.tile([T, HC * 128], BF16)

            o128 = cpool.tile([128, 1], BF16)
            nc.sync.dma_start(o128[:], ones128[:])
            # [tri | tri]: the same lower-triangle pattern serves both
            # diagonal tiles (B's triangle is A's shifted by 128 both ways)
            mask_t = cpool.tile([128, 256], BF16)
            nc.sync.dma_start(mask_t[:], maskD[:])

            # phase-2 tiles prefetched during phase 1 (wo_a has no deps; the
            # first head's kq/vt depend on the tb0-3 scratch stores)
            wopool = octx.enter_context(tc.tile_pool(name="p2wo", bufs=1))
            kqpool = octx.enter_context(tc.tile_pool(name="p2kq", bufs=2))
            vtpool = octx.enter_context(tc.tile_pool(name="p2vt", bufs=2))
            _wo_a = [None]
            _first_kv = [None]

            def load_kv(b, l):
                bs = b * S
                kq = kqpool.tile([128, 2, S], BF16, tag="kq")
                nc.sync.dma_start(
                    kq[:],
                    qkT_d[l * 256:(l + 1) * 256, bs:bs + S]
                        .rearrange("(j p) t -> p j t", p=128))
                vt = vtpool.tile([128, 8, 128], BF16, tag="vt")
                nc.sync.dma_start(
                    vt[:],
                    v_d[bs:bs + S, l * 128:(l + 1) * 128]
                        .rearrange("(kt p) d -> p kt d", p=128))
                return kq, vt

            def prefetch_wo():
                # W_o resident: [128, h(4), j(2), c(4096)]; j=0 -> hi, 1 -> lo
                wo_a = wopool.tile([128, HC, 2, H], F8, tag="wo")
                for h in range(HC):
                    nc.sync.dma_start(
                        wo_a[:, h],
                        wo[:, h * 2 * H:(h + 1) * 2 * H]
                            .rearrange("p (j c) -> p j c", j=2))
                _wo_a[0] = wo_a

            def prefetch_kv():
                _first_kv[0] = load_kv(0, 0)

            # ---------------- Phase 1: QKV projection (fp8 DoubleRow) -------
            if "p1" in phases:
              with ExitStack() as ctx:
                wpool = ctx.enter_context(tc.tile_pool(name="p1w", bufs=1))
                xpool = ctx.enter_context(tc.tile_pool(name="p1x", bufs=2))
                opool = ctx.enter_context(tc.tile_pool(name="p1o", bufs=2))
                cspool = ctx.enter_context(tc.tile_pool(name="p1cs", bufs=2))
                rpool = ctx.enter_context(tc.tile_pool(name="p1rope", bufs=3))
                pqk = ctx.enter_context(tc.tile_pool(name="p1pqk", bufs=4, space="PSUM"))
                pv = ctx.enter_context(tc.tile_pool(name="p1pv", bufs=2, space="PSUM"))

                def load_tb(tb):
                    t0 = tb * TB
                    # x pack [128, kk(32), j(2), t(256)]; j=0 -> x_hi, j=1 -> x_lo
                    xall = xpool.tile([128, KC, 2, TB], F8, tag="x")
                    nc.sync.dma_start(
                        xall[:],
                        xq[:, tb * 16384:(tb + 1) * 16384]
                            .rearrange("p (kk j t) -> p kk j t", kk=KC, j=2))
                    cos_tb = cspool.tile([128, TB], F32, tag="cos")
                    nc.sync.dma_start(cos_tb[:], cosT[:, t0:t0 + TB])
                    sin_tb = cspool.tile([128, TB], F32, tag="sin")
                    nc.sync.dma_start(sin_tb[:], sinS[:, t0:t0 + TB])
                    return xall, cos_tb, sin_tb

                # tb0 inputs first (first chain needs x + wqk ct0 only), then
                # resident weights: wqk [128, ct(8), kk(32), j(2), c(128)],
                # wv [128, ct(2), kk(32), j(2), c(256)]; j=0 -> W_lo, j=1 -> W_hi
                tb0_inputs = load_tb(0)
                wqk_a = wpool.tile([128, 8, KC, 2, 128], F8, tag="wqk")
                wv_a = wpool.tile([128, 2, KC, 2, 256], F8, tag="wv")
                for ct in range(8):
                    nc.sync.dma_start(
                        wqk_a[:, ct],
                        wqk[:, ct * 8192:(ct + 1) * 8192]
                            .rearrange("p (kk j c) -> p kk j c", kk=KC, j=2))
                for cv in range(2):
                    nc.sync.dma_start(
                        wv_a[:, cv],
                        wv[:, cv * 16384:(cv + 1) * 16384]
                            .rearrange("p (kk j c) -> p kk j c", kk=KC, j=2))

                def emit_qk(xall, cos_tb, sin_tb, t0):
                    qs_all = opool.tile([128, 8, TB], BF16, tag="qs")
                    for i in range(8):
                        ps = pqk.tile([128, TB], F32, tag="qk")
                        for c in range(16):
                            nc.tensor.matmul(
                                ps[:], wqk_a[:, i, 2 * c:2 * c + 2, 1, :],
                                xall[:, 2 * c:2 * c + 2, 0, :],
                                start=(c == 0), stop=False, perf_mode=DR)
                        for kk in range(KC):
                            nc.tensor.matmul(
                                ps[:], wqk_a[:, i, kk, :, :],
                                xall[:, kk, :, :],
                                start=False, stop=(kk == KC - 1), perf_mode=DR)
                        # RoPE epilogue (psum scale folded into cos/sin tables)
                        rot = rpool.tile([128, TB], F32, tag="rot")
                        nc.scalar.copy(rot[0:64, :], ps[64:128, :])
                        nc.vector.tensor_copy(rot[64:128, :], ps[0:64, :])
                        m1_ = rpool.tile([128, TB], F32, tag="m1")
                        nc.vector.tensor_tensor(m1_[:], ps[:], cos_tb[:], op=ALU.mult)
                        m2_ = rpool.tile([128, TB], F32, tag="m2")
                        nc.vector.tensor_tensor(m2_[:], rot[:], sin_tb[:], op=ALU.mult)
                        nc.vector.tensor_tensor(qs_all[:, i, :], m1_[:], m2_[:],
                                                op=ALU.add)
                    nc.sync.dma_start(
                        qkT_d[:, t0:t0 + TB].rearrange("(i p) t -> p i t", p=128),
                        qs_all[:])

                def emit_v(xall, t0):
                    vs_all = opool.tile([128, 2, 2, 256], BF16, tag="vs")
                    for th in range(2):
                        for ch in range(2):
                            ps = pv.tile([128, 256], F32, tag="v")
                            for c in range(16):
                                nc.tensor.matmul(
                                    ps[:],
                                    xall[:, 2 * c:2 * c + 2, 0,
                                         th * 128:(th + 1) * 128],
                                    wv_a[:, ch, 2 * c:2 * c + 2, 1, :],
                                    start=(c == 0), stop=False, perf_mode=DR)
                            for kk in range(KC):
                                nc.tensor.matmul(
                                    ps[:],
                                    xall[:, kk, :, th * 128:(th + 1) * 128],
                                    wv_a[:, ch, kk, :, :],
                                    start=False, stop=(kk == KC - 1), perf_mode=DR)
                            nc.scalar.activation(vs_all[:, th, ch, :], ps[:],
                                                 AF.Copy, scale=DESCALE)
                    nc.sync.dma_start(
                        v_d[t0:t0 + TB, :]
                            .rearrange("(th p) (ch c) -> p th ch c", p=128, ch=2),
                        vs_all[:])

                # v(0) is deferred until after qk(1): tb0's PE work then
                # needs only x+wqk, hiding the wv weight-load latency
                deferred_v0 = [None]
                for tb in range(NTB):
                    t0 = tb * TB
                    if tb == 0:
                        xall, cos_tb, sin_tb = tb0_inputs
                    else:
                        xall, cos_tb, sin_tb = load_tb(tb)
                    if tb == 1:
                        prefetch_wo()
                    elif tb == 4:
                        prefetch_kv()
                    emit_qk(xall, cos_tb, sin_tb, t0)
                    if tb == 0:
                        deferred_v0[0] = (xall, t0)
                    else:
                        if deferred_v0[0] is not None:
                            emit_v(*deferred_v0[0])
                            deferred_v0[0] = None
                        emit_v(xall, t0)

            # ---------------- Phase 2: attention + W_o ----------------------
            if "p2" in phases:
              with ExitStack() as ctx:
                apool = ctx.enter_context(tc.tile_pool(name="p2a", bufs=2))
                efpool = ctx.enter_context(tc.tile_pool(name="p2ef", bufs=14))
                tpool = ctx.enter_context(tc.tile_pool(name="p2t", bufs=3))
                rpool2 = ctx.enter_context(tc.tile_pool(name="p2rd", bufs=3))
                opool = ctx.enter_context(tc.tile_pool(name="p2o", bufs=2))
                ps_s = ctx.enter_context(tc.tile_pool(name="p2ps", bufs=2, space="PSUM"))
                ps_av = ctx.enter_context(tc.tile_pool(name="p2pav", bufs=1, space="PSUM"))
                ps_d = ctx.enter_context(tc.tile_pool(name="p2pd", bufs=1, space="PSUM"))
                ps_o = ctx.enter_context(tc.tile_pool(name="p2po", bufs=2, space="PSUM"))

                if _wo_a[0] is None:     # p2-only debug build
                    prefetch_wo()
                    prefetch_kv()
                wo_a = _wo_a[0]
                for b in range(B):
                    bs = b * S
                    # attn pack [128, lh(2), l(4), t(1024)]; lh=0 -> lo, 1 -> hi
                    apack = apool.tile([128, 2, HC, S], F8, tag="apack")
                    pending = [None]
                    for l in range(HC):
                        if b == 0 and l == 0:
                            kq, vt = _first_kv[0]
                        else:
                            kq, vt = load_kv(b, l)

                        psd_l = ps_d.tile([1, 512], F32, tag="dbc")
                        psav_l = ps_av.tile([128, 512], F32, tag="av")
                        all_efs = {}

                        def emit_scores(qb):
                            u = 2 * qb + 2
                            q_sl = kq[:, 0, qb * 256:(qb + 1) * 256]
                            efs = [None] * u
                            # diagonal pair first: its exp+mask latency hides
                            # behind the remaining pairs' matmuls
                            for g in [qb] + list(range(qb)):
                                pss = ps_s.tile([128, 512], F32, tag="s")
                                if g == qb:
                                    # diagonal pair: tile A is full; tile B
                                    # only sees the top query half. Scores for
                                    # B's live half go to pss cols 384:512;
                                    # cols 256:384 stay stale and are never
                                    # read downstream.
                                    nc.tensor.matmul(
                                        pss[:, 0:256],
                                        kq[:, 1, 2 * g * 128:(2 * g + 1) * 128],
                                        q_sl, start=True, stop=True)
                                    nc.tensor.matmul(
                                        pss[:, 384:512],
                                        kq[:, 1, (2 * g + 1) * 128:(2 * g + 2) * 128],
                                        q_sl[:, 128:256], start=True, stop=True)
                                else:
                                    for sHalf in range(2):
                                        mt = 2 * g + sHalf
                                        nc.tensor.matmul(
                                            pss[:, sHalf * 256:(sHalf + 1) * 256],
                                            kq[:, 1, mt * 128:(mt + 1) * 128],
                                            q_sl, start=True, stop=True)
                                ef = efpool.tile([128, 512], BF16, tag="ef")
                                nc.scalar.activation(ef[:], pss[:], AF.Exp,
                                                     scale=SCALE)
                                if g == qb:
                                    # in-place triangular mask on A's left
                                    # quarter and B's live quarter (same
                                    # pattern), one strided DVE op
                                    quarters = ef[:].rearrange(
                                        "p (g c) -> p g c", g=4)[:, ::3, :]
                                    nc.vector.tensor_tensor(
                                        quarters, quarters,
                                        mask_t[:].rearrange(
                                            "p (j c) -> p j c", j=2),
                                        op=ALU.mult)
                                efs[2 * g] = ef[:, 0:256]
                                efs[2 * g + 1] = ef[:, 384:512]
                            all_efs[qb] = efs

                        def emit_pv(qb):
                            u = 2 * qb + 2
                            efs = all_efs.pop(qb)
                            # masked diagonal units last in the chains
                            order = list(range(2 * qb)) + [2 * qb, 2 * qb + 1]
                            if pending[0] is not None:
                                pending[0]()
                                pending[0] = None
                            psav = psav_l[:, (qb % 2) * 256:(qb % 2 + 1) * 256]
                            for n, mt in enumerate(order):
                                nc.tensor.matmul(
                                    psav, vt[:, mt, :], efs[mt],
                                    start=(n == 0), stop=(n == u - 1))
                            psd = psd_l[:, (qb % 2) * 256:(qb % 2 + 1) * 256]
                            for n, mt in enumerate(order):
                                nc.tensor.matmul(
                                    psd, o128[:], efs[mt],
                                    start=(n == 0), stop=(n == u - 1))
                            if qb % 2 == 0:
                                return
                            # pair epilogue (qb-1, qb): unnormalized attn to
                            # SBUF (frees the psum bank), denominators to
                            # reciprocal; the normalization + fp8 hi/lo split
                            # is deferred into the next PV block
                            rd = rpool2.tile([1, 512], F32, tag="rd")
                            nc.vector.reciprocal(rd[:], psd_l[:])
                            av_s = tpool.tile([128, 512], F32, tag="avs")
                            nc.vector.tensor_copy(av_s[:], psav_l[:])

                            def make_epilogue(qb=qb, av_s=av_s, rd=rd, l=l,
                                              apack=apack):
                                def emit():
                                    # s_a/denom broadcast across partitions on
                                    # GpSimd: no PE matmul, no PSUM bank
                                    bc = tpool.tile([128, 512], F32, tag="bc")
                                    nc.gpsimd.partition_broadcast(bc[:], rd[:])
                                    t_ = tpool.tile([128, 512], F32, tag="t")
                                    nc.vector.tensor_tensor(t_[:], av_s[:],
                                                            bc[:], op=ALU.mult)
                                    q0 = (qb - 1) * 256
                                    hi = apack[:, 1, l, q0:q0 + 512]
                                    nc.gpsimd.tensor_copy(hi, t_[:])
                                    nc.gpsimd.tensor_tensor(
                                        apack[:, 0, l, q0:q0 + 512], t_[:], hi,
                                        op=ALU.subtract)
                                return emit
                            pending[0] = make_epilogue()

                        # all scores (and their masks) are emitted before any
                        # PV block: every engine queue sees the masks first
                        emit_scores(0)
                        emit_scores(1)
                        emit_scores(2)
                        emit_scores(3)
                        emit_pv(0)
                        emit_pv(1)
                        emit_pv(2)
                        emit_pv(3)
                    # W_o projection for batch b (fp8 DoubleRow main+corr);
                    # two 256-col chains per PSUM bank, one wide copy each.
                    # The last pair epilogue (l=3, qb 2-3) flushes after the
                    # first m-block, which only reads early tokens.
                    for m in range(8):
                        osb = opool.tile([128, 4, 1024], BF16, tag="osb")
                        msl = slice(m * 128, (m + 1) * 128)
                        for quad in range(4):
                            pso = ps_o.tile([128, 1024], F32, tag="o")
                            for part in range(4):
                                csl = slice((4 * quad + part) * 256,
                                            (4 * quad + part + 1) * 256)
                                po = pso[:, part * 256:(part + 1) * 256]
                                for c in range(2):
                                    nc.tensor.matmul(
                                        po, apack[:, 1, 2 * c:2 * c + 2, msl],
                                        wo_a[:, 2 * c:2 * c + 2, 0, csl],
                                        start=(c == 0), stop=False, perf_mode=DR)
                                for h in range(HC):
                                    nc.tensor.matmul(
                                        po, apack[:, :, h, msl],
                                        wo_a[:, h, :, csl],
                                        start=False, stop=(h == HC - 1),
                                        perf_mode=DR)
                            if quad % 2 == 0:
                                nc.vector.tensor_copy(osb[:, quad, :], pso[:])
                            else:
                                nc.scalar.copy(osb[:, quad, :], pso[:])
                        nc.sync.dma_start(
                            out_p[bs + m * 128:bs + (m + 1) * 128, :],
                            osb[:].rearrange("p nc c -> p (nc c)"))
                        if m == 0 and pending[0] is not None:
                            pending[0]()
                            pending[0] = None
    nc.compile()
    return nc


def _q8hl(a, scale):
    """Quantize to fp8 e4m3 hi/lo pair at a shared scale."""
    import ml_dtypes
    hi = (a * scale).astype(ml_dtypes.float8_e4m3)
    lo = ((a * scale) - hi.astype(np.float32)).astype(ml_dtypes.float8_e4m3)
    return hi, lo


def _host_prep(hidden_states, W_pack, W_o, attention_mask, position_ids):
    import ml_dtypes
    x = np.asarray(hidden_states, dtype=np.float32).reshape(T, H)
    W_pack = np.asarray(W_pack, dtype=np.float32)
    W_o = np.asarray(W_o, dtype=np.float32)
    mask = np.asarray(attention_mask, dtype=np.float32)
    pos = np.asarray(position_ids)

    # causal structure is hardcoded in the kernel; verify it holds
    m0 = mask[0, 0]
    iu = np.triu_indices(S, 1)
    assert (m0[iu] < -1e8).all() and (np.tril(m0) == 0).all(), \
        "kernel requires the standard causal mask"

    # x pack: [128p, tb, kk, j(hi,lo), t] -> flat [128, NTB*KC*2*TB]
    xh, xl = _q8hl(x, S_X)
    xv_h = xh.reshape(NTB, TB, KC, 128).transpose(3, 0, 2, 1)
    xv_l = xl.reshape(NTB, TB, KC, 128).transpose(3, 0, 2, 1)
    xq_np = np.empty((128, NTB, KC, 2, TB), ml_dtypes.float8_e4m3)
    xq_np[:, :, :, 0, :] = xv_h
    xq_np[:, :, :, 1, :] = xv_l
    xq_np = np.ascontiguousarray(xq_np.reshape(128, -1))

    # rope tables with the fp8 descale folded in; rotate-half sign in sinS
    inv = 1.0 / (ROPE_BASE ** (np.arange(0, D, 2, dtype=np.float64) / D))
    inv = np.concatenate([inv, inv])
    ang = pos.astype(np.float64).reshape(T)[None, :] * inv[:, None]   # [D, T]
    cosT_np = np.ascontiguousarray((np.cos(ang) * DESCALE).astype(np.float32))
    sinT = (np.sin(ang) * DESCALE).astype(np.float32)
    sinS_np = sinT.copy()
    sinS_np[:64] = -sinT[:64]
    sinS_np = np.ascontiguousarray(sinS_np)

    # diagonal exp-mask patterns [128p(key), s(2)*256(query)] bf16
    em = np.exp(m0)
    maskD_np = np.empty((128, 2, 256), ml_dtypes.bfloat16)
    maskD_np[:, 0, :] = em[0:256, 0:128].T       # offset 0 pattern
    maskD_np[:, 1, :] = em[0:256, 128:256].T     # offset 128 pattern
    maskD_np = np.ascontiguousarray(maskD_np.reshape(128, 512))

    in_maps = []
    for core in range(N_CORES):
        h0 = core * HC
        # wqk cols ordered [q0,k0,q1,k1,q2,k2,q3,k3] per head slice
        cols = []
        for l in range(HC):
            cols.append(W_pack[:, (h0 + l) * D:(h0 + l + 1) * D])
            cols.append(W_pack[:, H + (h0 + l) * D:H + (h0 + l + 1) * D])
        wqk_f = np.concatenate(cols, axis=1)              # [H, 1024]
        wh, wl = _q8hl(wqk_f, S_W)
        wv_h = wh.reshape(KC, 128, 8, 128).transpose(1, 2, 0, 3)
        wv_l = wl.reshape(KC, 128, 8, 128).transpose(1, 2, 0, 3)
        wqk_np = np.empty((128, 8, KC, 2, 128), ml_dtypes.float8_e4m3)
        wqk_np[:, :, :, 0, :] = wv_l
        wqk_np[:, :, :, 1, :] = wv_h
        wqk_np = np.ascontiguousarray(wqk_np.reshape(128, -1))

        wv_f = np.concatenate(
            [W_pack[:, 2 * H + (h0 + l) * D:2 * H + (h0 + l + 1) * D]
             for l in range(HC)], axis=1)                 # [H, 512]
        wh, wl = _q8hl(wv_f, S_W)
        wvv_h = wh.reshape(KC, 128, 2, 256).transpose(1, 2, 0, 3)
        wvv_l = wl.reshape(KC, 128, 2, 256).transpose(1, 2, 0, 3)
        wv_np = np.empty((128, 2, KC, 2, 256), ml_dtypes.float8_e4m3)
        wv_np[:, :, :, 0, :] = wvv_l
        wv_np[:, :, :, 1, :] = wvv_h
        wv_np = np.ascontiguousarray(wv_np.reshape(128, -1))

        wo_f = W_o[h0 * D:(h0 + HC) * D, :]               # [512, H]
        wh, wl = _q8hl(wo_f, S_W)
        wov_h = wh.reshape(HC, 128, H).transpose(1, 0, 2)
        wov_l = wl.reshape(HC, 128, H).transpose(1, 0, 2)
        wo_np = np.empty((128, HC, 2, H), ml_dtypes.float8_e4m3)
        wo_np[:, :, 0, :] = wov_h
        wo_np[:, :, 1, :] = wov_l
        wo_np = np.ascontiguousarray(wo_np.reshape(128, -1))

        in_maps.append({
            "xq": xq_np, "wqk": wqk_np, "wv": wv_np, "wo": wo_np,
            "cosT": cosT_np, "sinS": sinS_np, "maskD": maskD_np,
        })
    return in_maps


def kernel(hidden_states, W_pack, W_o, attention_mask, position_ids):
    if "nc" not in _CACHE:
        _CACHE["nc"] = _build_module()
    nc = _CACHE["nc"]
    in_maps = _host_prep(hidden_states, W_pack, W_o, attention_mask, position_ids)
    res = bass_utils.run_bass_kernel_spmd(nc, in_maps, core_ids=list(range(N_CORES)))
    out = res.results[0]["out_p"].astype(np.float32)
    for c in range(1, N_CORES):
        out += res.results[c]["out_p"]
    out *= 1.0 / (S_A * S_W)
    return out.reshape(B, S, H).astype(np.float32)


# revision 71
# speedup vs baseline: 1.0124x; 1.0058x over previous
"""Trainium2 Bass kernel for nn_Attention_60567628808865.

Dense transformer attention block (B=4, S=1024, H=4096, NH=32, D=128):
  qkv = x @ W_pack; RoPE(q, k); causal-masked softmax attention; out @ W_o.

Sharding: tensor-parallel over heads across 8 NeuronCores. Each core computes
4 heads end-to-end; the host sums the 8 partial W_o outputs (row-sharded W_o).

Precision/performance scheme (validated on host to rel_err ~2.7e-3 vs the
2e-2 gate):
  - QKV and W_o projections run in fp8(e4m3) with the DoubleRow perf mode
    (K=256 per instruction, 0.5 cycles/row) using an exact-style two-term
    decomposition: x@W ~= x_hi@W_hi + [x_hi@W_lo + x_lo@W_hi], where
    t_hi = fp8(t*s), t_lo = fp8(t*s - t_hi). Both terms accumulate into ONE
    PSUM chain (identical scale), so the epilogue is unchanged. 48 DoubleRow
    instructions replace 32 f32r instructions per [128col x 256tok] unit:
    0.75x PE cycles.
  - hi/lo operands are slot-interleaved in a single packed tensor
    ([part, chunk, 2, free]) so the correction chain reads (hi,lo) slot pairs
    and the main chain reads (hi,hi) chunk pairs from the same SBUF bytes.
  - Attention is causal-aware: score/PV/denominator work is emitted only for
    the 20/32 key-tile x query-block units on or below the diagonal; the two
    diagonal tiles per query block are masked multiplicatively with a
    host-built exp(mask) pattern (asserted causal). q/k/v round-trip DRAM in
    bf16; scores/PV matmuls run in bf16 (same PE rate as f32r, half the DMA).
  - Softmax is unnormalized; denominators come from a ones-vector matmul
    accumulated in PSUM, broadcast back via a K=1 matmul; attention output is
    quantized to fp8 hi/lo pairs on the fly for the W_o DoubleRow chain.
  - Output partials are stored bf16; the host sum applies the global descale.
"""
import numpy as np

import concourse.bass as bass  # noqa: F401
import concourse.tile as tile
from contextlib import ExitStack
from concourse import bacc, mybir
from concourse import bass_utils

F32 = mybir.dt.float32
F32R = mybir.dt.float32r
BF16 = mybir.dt.bfloat16
F8 = mybir.dt.float8e4
AF = mybir.ActivationFunctionType
ALU = mybir.AluOpType
DR = mybir.MatmulPerfMode.DoubleRow

B, S, H, NH = 4, 1024, 4096, 32
D = H // NH          # 128
T = B * S            # 4096 tokens
N_CORES = 8
HC = NH // N_CORES   # 4 heads per core
SCALE = float(1.0 / np.sqrt(D))
ROPE_BASE = 10000.0

TB = 256             # phase-1 token block
NTB = T // TB        # 16
KC = H // 128        # 32 fp8 k-chunks of 128 features
S_X = 32.0           # x quant scale
S_W = 2048.0         # W_pack / W_o quant scale
S_A = 32.0           # attention-output quant scale
DESCALE = 1.0 / (S_X * S_W)

_CACHE = {}


def _build_module(phases=("p1", "p2")):
    nc = bacc.Bacc("TRN2", target_bir_lowering=False, debug=False,
                   num_devices=N_CORES)

    # packed fp8 inputs (see _host_prep for layouts)
    xq = nc.dram_tensor("xq", [128, NTB * KC * 2 * TB], F8, kind="ExternalInput").ap()
    wqk = nc.dram_tensor("wqk", [128, 8 * KC * 2 * 128], F8, kind="ExternalInput").ap()
    wv = nc.dram_tensor("wv", [128, 2 * KC * 2 * 256], F8, kind="ExternalInput").ap()
    wo = nc.dram_tensor("wo", [128, HC * 2 * H], F8, kind="ExternalInput").ap()
    cosT = nc.dram_tensor("cosT", [128, T], F32, kind="ExternalInput").ap()
    sinS = nc.dram_tensor("sinS", [128, T], F32, kind="ExternalInput").ap()
    maskD = nc.dram_tensor("maskD", [128, 256], BF16, kind="ExternalInput").ap()
    out_p = nc.dram_tensor("out_p", [T, H], BF16, kind="ExternalOutput").ap()

    import ml_dtypes
    # denominator ones-vector carries 1/S_A so the reciprocal yields
    # S_A/denom directly (the fp8 attn quant scale)
    ones128 = nc.inline_tensor(
        np.full((128, 1), 1.0 / S_A, ml_dtypes.bfloat16), "ones128").ap()

    with tile.TileContext(nc) as tc, \
         nc.allow_low_precision(reason="fp8/bf16 matmuls; verified vs reference"):
        with ExitStack() as octx:
            dram = octx.enter_context(tc.tile_pool(name="dram", bufs=1, space="DRAM"))
            cpool = octx.enter_context(tc.tile_pool(name="consts", bufs=1))
            # DRAM scratch: qkT rows ordered [q0,k0,q1,k1,q2,k2,q3,k3] x d
            qkT_d = dram.tile([8 * 128, T], BF16)
            v_d = dram.tile([T, HC * 128], BF16)

            o128 = cpool.tile([128, 1], BF16)
            nc.sync.dma_start(o128[:], ones128[:])
            # [tri | tri]: the same lower-triangle pattern serves both
            # diagonal tiles (B's triangle is A's shifted by 128 both ways)
            mask_t = cpool.tile([128, 256], BF16)
            nc.sync.dma_start(mask_t[:], maskD[:])

            # phase-2 tiles prefetched during phase 1 (wo_a has no deps; the
            # first head's kq/vt depend on the tb0-3 scratch stores)
            wopool = octx.enter_context(tc.tile_pool(name="p2wo", bufs=1))
            kqpool = octx.enter_context(tc.tile_pool(name="p2kq", bufs=3))
            vtpool = octx.enter_context(tc.tile_pool(name="p2vt", bufs=2))
            _wo_a = [None]
            _first_kv = [None]

            def load_kv(b, l):
                bs = b * S
                kq = kqpool.tile([128, 2, S], BF16, tag="kq")
                nc.sync.dma_start(
                    kq[:],
                    qkT_d[l * 256:(l + 1) * 256, bs:bs + S]
                        .rearrange("(j p) t -> p j t", p=128))
                vt = vtpool.tile([128, 8, 128], BF16, tag="vt")
                nc.sync.dma_start(
                    vt[:],
                    v_d[bs:bs + S, l * 128:(l + 1) * 128]
                        .rearrange("(kt p) d -> p kt d", p=128))
                return kq, vt

            def prefetch_wo():
                # W_o resident: [128, h(4), j(2), c(4096)]; j=0 -> hi, 1 -> lo
                wo_a = wopool.tile([128, HC, 2, H], F8, tag="wo")
                for h in range(HC):
                    nc.sync.dma_start(
                        wo_a[:, h],
                        wo[:, h * 2 * H:(h + 1) * 2 * H]
                            .rearrange("p (j c) -> p j c", j=2))
                _wo_a[0] = wo_a

            def prefetch_kv():
                _first_kv[0] = load_kv(0, 0)

            # ---------------- Phase 1: QKV projection (fp8 DoubleRow) -------
            if "p1" in phases:
              with ExitStack() as ctx:
                wpool = ctx.enter_context(tc.tile_pool(name="p1w", bufs=1))
                xpool = ctx.enter_context(tc.tile_pool(name="p1x", bufs=2))
                opool = ctx.enter_context(tc.tile_pool(name="p1o", bufs=2))
                cspool = ctx.enter_context(tc.tile_pool(name="p1cs", bufs=2))
                rpool = ctx.enter_context(tc.tile_pool(name="p1rope", bufs=3))
                pqk = ctx.enter_context(tc.tile_pool(name="p1pqk", bufs=4, space="PSUM"))
                pv = ctx.enter_context(tc.tile_pool(name="p1pv", bufs=2, space="PSUM"))

                def load_tb(tb):
                    t0 = tb * TB
                    # x pack [128, kk(32), j(2), t(256)]; j=0 -> x_hi, j=1 -> x_lo
                    xall = xpool.tile([128, KC, 2, TB], F8, tag="x")
                    nc.sync.dma_start(
                        xall[:],
                        xq[:, tb * 16384:(tb + 1) * 16384]
                            .rearrange("p (kk j t) -> p kk j t", kk=KC, j=2))
                    cos_tb = cspool.tile([128, TB], F32, tag="cos")
                    nc.sync.dma_start(cos_tb[:], cosT[:, t0:t0 + TB])
                    sin_tb = cspool.tile([128, TB], F32, tag="sin")
                    nc.sync.dma_start(sin_tb[:], sinS[:, t0:t0 + TB])
                    return xall, cos_tb, sin_tb

                # tb0 inputs first (first chain needs x + wqk ct0 only), then
                # resident weights: wqk [128, ct(8), kk(32), j(2), c(128)],
                # wv [128, ct(2), kk(32), j(2), c(256)]; j=0 -> W_lo, j=1 -> W_hi
                tb0_inputs = load_tb(0)
                wqk_a = wpool.tile([128, 8, KC, 2, 128], F8, tag="wqk")
                wv_a = wpool.tile([128, 2, KC, 2, 256], F8, tag="wv")
                for ct in range(8):
                    nc.sync.dma_start(
                        wqk_a[:, ct],
                        wqk[:, ct * 8192:(ct + 1) * 8192]
                            .rearrange("p (kk j c) -> p kk j c", kk=KC, j=2))
                for cv in range(2):
                    nc.sync.dma_start(
                        wv_a[:, cv],
                        wv[:, cv * 16384:(cv + 1) * 16384]
                            .rearrange("p (kk j c) -> p kk j c", kk=KC, j=2))

                def emit_qk(xall, cos_tb, sin_tb, t0):
                    qs_all = opool.tile([128, 8, TB], BF16, tag="qs")
                    for i in range(8):
                        ps = pqk.tile([128, TB], F32, tag="qk")
                        for c in range(16):
                            nc.tensor.matmul(
                                ps[:], wqk_a[:, i, 2 * c:2 * c + 2, 1, :],
                                xall[:, 2 * c:2 * c + 2, 0, :],
                                start=(c == 0), stop=False, perf_mode=DR)
                        for kk in range(KC):
                            nc.tensor.matmul(
                                ps[:], wqk_a[:, i, kk, :, :],
                                xall[:, kk, :, :],
                                start=False, stop=(kk == KC - 1), perf_mode=DR)
                        # RoPE epilogue (psum scale folded into cos/sin tables)
                        rot = rpool.tile([128, TB], F32, tag="rot")
                        nc.scalar.copy(rot[0:64, :], ps[64:128, :])
                        nc.vector.tensor_copy(rot[64:128, :], ps[0:64, :])
                        m1_ = rpool.tile([128, TB], F32, tag="m1")
                        nc.vector.tensor_tensor(m1_[:], ps[:], cos_tb[:], op=ALU.mult)
                        m2_ = rpool.tile([128, TB], F32, tag="m2")
                        nc.vector.tensor_tensor(m2_[:], rot[:], sin_tb[:], op=ALU.mult)
                        nc.vector.tensor_tensor(qs_all[:, i, :], m1_[:], m2_[:],
                                                op=ALU.add)
                    nc.sync.dma_start(
                        qkT_d[:, t0:t0 + TB].rearrange("(i p) t -> p i t", p=128),
                        qs_all[:])

                def emit_v(xall, t0):
                    vs_all = opool.tile([128, 2, 2, 256], BF16, tag="vs")
                    for th in range(2):
                        for ch in range(2):
                            ps = pv.tile([128, 256], F32, tag="v")
                            for c in range(16):
                                nc.tensor.matmul(
                                    ps[:],
                                    xall[:, 2 * c:2 * c + 2, 0,
                                         th * 128:(th + 1) * 128],
                                    wv_a[:, ch, 2 * c:2 * c + 2, 1, :],
                                    start=(c == 0), stop=False, perf_mode=DR)
                            for kk in range(KC):
                                nc.tensor.matmul(
                                    ps[:],
                                    xall[:, kk, :, th * 128:(th + 1) * 128],
                                    wv_a[:, ch, kk, :, :],
                                    start=False, stop=(kk == KC - 1), perf_mode=DR)
                            nc.scalar.activation(vs_all[:, th, ch, :], ps[:],
                                                 AF.Copy, scale=DESCALE)
                    nc.sync.dma_start(
                        v_d[t0:t0 + TB, :]
                            .rearrange("(th p) (ch c) -> p th ch c", p=128, ch=2),
                        vs_all[:])

                # v(0) is deferred until after qk(1): tb0's PE work then
                # needs only x+wqk, hiding the wv weight-load latency
                deferred_v0 = [None]
                for tb in range(NTB):
                    t0 = tb * TB
                    if tb == 0:
                        xall, cos_tb, sin_tb = tb0_inputs
                    else:
                        xall, cos_tb, sin_tb = load_tb(tb)
                    if tb == 1:
                        prefetch_wo()
                    elif tb == 4:
                        prefetch_kv()
                    emit_qk(xall, cos_tb, sin_tb, t0)
                    if tb == 0:
                        deferred_v0[0] = (xall, t0)
                    else:
                        if deferred_v0[0] is not None:
                            emit_v(*deferred_v0[0])
                            deferred_v0[0] = None
                        emit_v(xall, t0)

            # ---------------- Phase 2: attention + W_o ----------------------
            if "p2" in phases:
              with ExitStack() as ctx:
                apool = ctx.enter_context(tc.tile_pool(name="p2a", bufs=2))
                efpool = ctx.enter_context(tc.tile_pool(name="p2ef", bufs=16))
                tpool = ctx.enter_context(tc.tile_pool(name="p2t", bufs=3))
                rpool2 = ctx.enter_context(tc.tile_pool(name="p2rd", bufs=3))
                opool = ctx.enter_context(tc.tile_pool(name="p2o", bufs=2))
                ps_s = ctx.enter_context(tc.tile_pool(name="p2ps", bufs=2, space="PSUM"))
                ps_av = ctx.enter_context(tc.tile_pool(name="p2pav", bufs=1, space="PSUM"))
                ps_d = ctx.enter_context(tc.tile_pool(name="p2pd", bufs=1, space="PSUM"))
                ps_o = ctx.enter_context(tc.tile_pool(name="p2po", bufs=2, space="PSUM"))

                if _wo_a[0] is None:     # p2-only debug build
                    prefetch_wo()
                    prefetch_kv()
                wo_a = _wo_a[0]
                for b in range(B):
                    bs = b * S
                    # attn pack [128, lh(2), l(4), t(1024)]; lh=0 -> lo, 1 -> hi
                    apack = apool.tile([128, 2, HC, S], F8, tag="apack")
                    pending = [None]
                    for l in range(HC):
                        if b == 0 and l == 0:
                            kq, vt = _first_kv[0]
                        else:
                            kq, vt = load_kv(b, l)

                        psd_l = ps_d.tile([1, 512], F32, tag="dbc")
                        psav_l = ps_av.tile([128, 512], F32, tag="av")
                        all_efs = {}

                        def emit_scores(qb):
                            u = 2 * qb + 2
                            q_sl = kq[:, 0, qb * 256:(qb + 1) * 256]
                            efs = [None] * u
                            # diagonal pair first: its exp+mask latency hides
                            # behind the remaining pairs' matmuls
                            for g in [qb] + list(range(qb)):
                                pss = ps_s.tile([128, 512], F32, tag="s")
                                if g == qb:
                                    # diagonal pair: tile A is full; tile B
                                    # only sees the top query half, packed
                                    # right after A so the exp is [128,384]
                                    nc.tensor.matmul(
                                        pss[:, 0:256],
                                        kq[:, 1, 2 * g * 128:(2 * g + 1) * 128],
                                        q_sl, start=True, stop=True)
                                    nc.tensor.matmul(
                                        pss[:, 256:384],
                                        kq[:, 1, (2 * g + 1) * 128:(2 * g + 2) * 128],
                                        q_sl[:, 128:256], start=True, stop=True)
                                    ef = efpool.tile([128, 384], BF16, tag="ef")
                                    nc.scalar.activation(ef[:], pss[:, 0:384],
                                                         AF.Exp, scale=SCALE)
                                    # in-place triangular mask on A's left
                                    # quarter and B's live quarter (same
                                    # pattern), one strided DVE op
                                    quarters = ef[:].rearrange(
                                        "p (g c) -> p g c", g=3)[:, ::2, :]
                                    nc.vector.tensor_tensor(
                                        quarters, quarters,
                                        mask_t[:].rearrange(
                                            "p (j c) -> p j c", j=2),
                                        op=ALU.mult)
                                    efs[2 * g] = ef[:, 0:256]
                                    efs[2 * g + 1] = ef[:, 256:384]
                                else:
                                    for sHalf in range(2):
                                        mt = 2 * g + sHalf
                                        nc.tensor.matmul(
                                            pss[:, sHalf * 256:(sHalf + 1) * 256],
                                            kq[:, 1, mt * 128:(mt + 1) * 128],
                                            q_sl, start=True, stop=True)
                                    ef = efpool.tile([128, 512], BF16, tag="ef")
                                    nc.scalar.activation(ef[:], pss[:], AF.Exp,
                                                         scale=SCALE)
                                    efs[2 * g] = ef[:, 0:256]
                                    efs[2 * g + 1] = ef[:, 256:512]
                            all_efs[qb] = efs

                        def emit_pv(qb):
                            u = 2 * qb + 2
                            efs = all_efs.pop(qb)
                            # masked diagonal units last in the chains
                            order = list(range(2 * qb)) + [2 * qb, 2 * qb + 1]
                            if pending[0] is not None:
                                pending[0]()
                                pending[0] = None
                            # the final (diagonal-B) unit only covers the top
                            # query half: half-width accumulation step
                            psav = psav_l[:, (qb % 2) * 256:(qb % 2 + 1) * 256]
                            for n, mt in enumerate(order):
                                half = mt == 2 * qb + 1
                                nc.tensor.matmul(
                                    psav[:, 128:256] if half else psav,
                                    vt[:, mt, :], efs[mt],
                                    start=(n == 0), stop=(n == u - 1),
                                    skip_group_check=True)
                            psd = psd_l[:, (qb % 2) * 256:(qb % 2 + 1) * 256]
                            for n, mt in enumerate(order):
                                half = mt == 2 * qb + 1
                                nc.tensor.matmul(
                                    psd[:, 128:256] if half else psd,
                                    o128[:], efs[mt],
                                    start=(n == 0), stop=(n == u - 1),
                                    skip_group_check=True)
                            if qb % 2 == 0:
                                return
                            # pair epilogue (qb-1, qb): unnormalized attn to
                            # SBUF (frees the psum bank), denominators to
                            # reciprocal; the normalization + fp8 hi/lo split
                            # is deferred into the next PV block
                            rd = rpool2.tile([1, 512], F32, tag="rd")
                            nc.vector.reciprocal(rd[:], psd_l[:])
                            av_s = tpool.tile([128, 512], F32, tag="avs")
                            nc.vector.tensor_copy(av_s[:], psav_l[:])

                            def make_epilogue(qb=qb, av_s=av_s, rd=rd, l=l,
                                              apack=apack):
                                def emit():
                                    # s_a/denom broadcast across partitions on
                                    # GpSimd: no PE matmul, no PSUM bank
                                    bc = tpool.tile([128, 512], F32, tag="bc")
                                    nc.gpsimd.partition_broadcast(bc[:], rd[:])
                                    t_ = tpool.tile([128, 512], F32, tag="t")
                                    nc.vector.tensor_tensor(t_[:], av_s[:],
                                                            bc[:], op=ALU.mult)
                                    q0 = (qb - 1) * 256
                                    hi = apack[:, 1, l, q0:q0 + 512]
                                    nc.gpsimd.tensor_copy(hi, t_[:])
                                    nc.gpsimd.tensor_tensor(
                                        apack[:, 0, l, q0:q0 + 512], t_[:], hi,
                                        op=ALU.subtract)
                                return emit
                            pending[0] = make_epilogue()

                        # all scores (and their masks) are emitted before any
                        # PV block: every engine queue sees the masks first
                        emit_scores(0)
                        emit_scores(1)
                        emit_scores(2)
                        emit_scores(3)
                        emit_pv(0)
                        emit_pv(1)
                        emit_pv(2)
                        emit_pv(3)
                    # W_o projection for batch b (fp8 DoubleRow main+corr);
                    # two 256-col chains per PSUM bank, one wide copy each.
                    # The last pair epilogue (l=3, qb 2-3) flushes after the
                    # first m-block, which only reads early tokens.
                    for m in range(8):
                        osb = opool.tile([128, 4, 1024], BF16, tag="osb")
                        msl = slice(m * 128, (m + 1) * 128)
                        for quad in range(4):
                            pso = ps_o.tile([128, 1024], F32, tag="o")
                            for part in range(4):
                                csl = slice((4 * quad + part) * 256,
                                            (4 * quad + part + 1) * 256)
                                po = pso[:, part * 256:(part + 1) * 256]
                                for c in range(2):
                                    nc.tensor.matmul(
                                        po, apack[:, 1, 2 * c:2 * c + 2, msl],
                                        wo_a[:, 2 * c:2 * c + 2, 0, csl],
                                        start=(c == 0), stop=False, perf_mode=DR)
                                for h in range(HC):
                                    nc.tensor.matmul(
                                        po, apack[:, :, h, msl],
                                        wo_a[:, h, :, csl],
                                        start=False, stop=(h == HC - 1),
                                        perf_mode=DR)
                            if quad % 2 == 0:
                                nc.vector.tensor_copy(osb[:, quad, :], pso[:])
                            else:
                                nc.scalar.copy(osb[:, quad, :], pso[:])
                        nc.sync.dma_start(
                            out_p[bs + m * 128:bs + (m + 1) * 128, :],
                            osb[:].rearrange("p nc c -> p (nc c)"))
                        if m == 0 and pending[0] is not None:
                            pending[0]()
                            pending[0] = None
    nc.compile()
    return nc


def _q8hl(a, scale):
    """Quantize to fp8 e4m3 hi/lo pair at a shared scale."""
    import ml_dtypes
    hi = (a * scale).astype(ml_dtypes.float8_e4m3)
    lo = ((a * scale) - hi.astype(np.float32)).astype(ml_dtypes.float8_e4m3)
    return hi, lo


def _host_prep(hidden_states, W_pack, W_o, attention_mask, position_ids):
    import ml_dtypes
    x = np.asarray(hidden_states, dtype=np.float32).reshape(T, H)
    W_pack = np.asarray(W_pack, dtype=np.float32)
    W_o = np.asarray(W_o, dtype=np.float32)
    mask = np.asarray(attention_mask, dtype=np.float32)
    pos = np.asarray(position_ids)

    # causal structure is hardcoded in the kernel; verify it holds
    m0 = mask[0, 0]
    iu = np.triu_indices(S, 1)
    assert (m0[iu] < -1e8).all() and (np.tril(m0) == 0).all(), \
        "kernel requires the standard causal mask"

    # x pack: [128p, tb, kk, j(hi,lo), t] -> flat [128, NTB*KC*2*TB]
    xh, xl = _q8hl(x, S_X)
    xv_h = xh.reshape(NTB, TB, KC, 128).transpose(3, 0, 2, 1)
    xv_l = xl.reshape(NTB, TB, KC, 128).transpose(3, 0, 2, 1)
    xq_np = np.empty((128, NTB, KC, 2, TB), ml_dtypes.float8_e4m3)
    xq_np[:, :, :, 0, :] = xv_h
    xq_np[:, :, :, 1, :] = xv_l
    xq_np = np.ascontiguousarray(xq_np.reshape(128, -1))

    # rope tables with the fp8 descale folded in; rotate-half sign in sinS
    inv = 1.0 / (ROPE_BASE ** (np.arange(0, D, 2, dtype=np.float64) / D))
    inv = np.concatenate([inv, inv])
    ang = pos.astype(np.float64).reshape(T)[None, :] * inv[:, None]   # [D, T]
    cosT_np = np.ascontiguousarray((np.cos(ang) * DESCALE).astype(np.float32))
    sinT = (np.sin(ang) * DESCALE).astype(np.float32)
    sinS_np = sinT.copy()
    sinS_np[:64] = -sinT[:64]
    sinS_np = np.ascontiguousarray(sinS_np)

    # diagonal exp-mask triangle [128p(key), 128(query)], duplicated so one
    # strided DVE op covers both diagonal tiles' live quarters
    em = np.exp(m0)
    tri = em[0:128, 0:128].T.astype(ml_dtypes.bfloat16)   # [p(key), t(query)]
    maskD_np = np.ascontiguousarray(
        np.concatenate([tri, tri], axis=1))               # [128, 256]

    in_maps = []
    for core in range(N_CORES):
        h0 = core * HC
        # wqk cols ordered [q0,k0,q1,k1,q2,k2,q3,k3] per head slice
        cols = []
        for l in range(HC):
            cols.append(W_pack[:, (h0 + l) * D:(h0 + l + 1) * D])
            cols.append(W_pack[:, H + (h0 + l) * D:H + (h0 + l + 1) * D])
        wqk_f = np.concatenate(cols, axis=1)              # [H, 1024]
        wh, wl = _q8hl(wqk_f, S_W)
        wv_h = wh.reshape(KC, 128, 8, 128).transpose(1, 2, 0, 3)
        wv_l = wl.reshape(KC, 128, 8, 128).transpose(1, 2, 0, 3)
        wqk_np = np.empty((128, 8, KC, 2, 128), ml_dtypes.float8_e4m3)
        wqk_np[:, :, :, 0, :] = wv_l
        wqk_np[:, :, :, 1, :] = wv_h
        wqk_np = np.ascontiguousarray(wqk_np.reshape(128, -1))

        wv_f = np.concatenate(
            [W_pack[:, 2 * H + (h0 + l) * D:2 * H + (h0 + l + 1) * D]
             for l in range(HC)], axis=1)                 # [H, 512]
        wh, wl = _q8hl(wv_f, S_W)
        wvv_h = wh.reshape(KC, 128, 2, 256).transpose(1, 2, 0, 3)
        wvv_l = wl.reshape(KC, 128, 2, 256).transpose(1, 2, 0, 3)
        wv_np = np.empty((128, 2, KC, 2, 256), ml_dtypes.float8_e4m3)
        wv_np[:, :, :, 0, :] = wvv_l
        wv_np[:, :, :, 1, :] = wvv_h
        wv_np = np.ascontiguousarray(wv_np.reshape(128, -1))

        wo_f = W_o[h0 * D:(h0 + HC) * D, :]               # [512, H]
        wh, wl = _q8hl(wo_f, S_W)
        wov_h = wh.reshape(HC, 128, H).transpose(1, 0, 2)
        wov_l = wl.reshape(HC, 128, H).transpose(1, 0, 2)
        wo_np = np.empty((128, HC, 2, H), ml_dtypes.float8_e4m3)
        wo_np[:, :, 0, :] = wov_h
        wo_np[:, :, 1, :] = wov_l
        wo_np = np.ascontiguousarray(wo_np.reshape(128, -1))

        in_maps.append({
            "xq": xq_np, "wqk": wqk_np, "wv": wv_np, "wo": wo_np,
            "cosT": cosT_np, "sinS": sinS_np, "maskD": maskD_np,
        })
    return in_maps


def kernel(hidden_states, W_pack, W_o, attention_mask, position_ids):
    if "nc" not in _CACHE:
        _CACHE["nc"] = _build_module()
    nc = _CACHE["nc"]
    in_maps = _host_prep(hidden_states, W_pack, W_o, attention_mask, position_ids)
    res = bass_utils.run_bass_kernel_spmd(nc, in_maps, core_ids=list(range(N_CORES)))
    out = res.results[0]["out_p"].astype(np.float32)
    for c in range(1, N_CORES):
        out += res.results[c]["out_p"]
    out *= 1.0 / (S_A * S_W)
    return out.reshape(B, S, H).astype(np.float32)


# revision 75
# speedup vs baseline: 1.0148x; 1.0024x over previous
"""Trainium2 Bass kernel for nn_Attention_60567628808865.

Dense transformer attention block (B=4, S=1024, H=4096, NH=32, D=128):
  qkv = x @ W_pack; RoPE(q, k); causal-masked softmax attention; out @ W_o.

Sharding: tensor-parallel over heads across 8 NeuronCores. Each core computes
4 heads end-to-end; the host sums the 8 partial W_o outputs (row-sharded W_o).

Precision/performance scheme (validated on host to rel_err ~2.7e-3 vs the
2e-2 gate):
  - QKV and W_o projections run in fp8(e4m3) with the DoubleRow perf mode
    (K=256 per instruction, 0.5 cycles/row) using an exact-style two-term
    decomposition: x@W ~= x_hi@W_hi + [x_hi@W_lo + x_lo@W_hi], where
    t_hi = fp8(t*s), t_lo = fp8(t*s - t_hi). Both terms accumulate into ONE
    PSUM chain (identical scale), so the epilogue is unchanged. 48 DoubleRow
    instructions replace 32 f32r instructions per [128col x 256tok] unit:
    0.75x PE cycles.
  - hi/lo operands are slot-interleaved in a single packed tensor
    ([part, chunk, 2, free]) so the correction chain reads (hi,lo) slot pairs
    and the main chain reads (hi,hi) chunk pairs from the same SBUF bytes.
  - Attention is causal-aware: score/PV/denominator work is emitted only for
    the 20/32 key-tile x query-block units on or below the diagonal; the
    second diagonal tile of each query block runs at half moving-width (its
    lower query half is fully masked), and both diagonal triangles are masked
    in-place with one strided DVE multiply against a single host-built
    exp(mask) triangle (mask asserted causal). q/k/v round-trip DRAM in bf16;
    scores/PV matmuls run in bf16 (same PE rate as f32r, half the DMA).
  - Softmax is unnormalized; denominators come from a (1/s_a)-vector matmul
    accumulated in PSUM; the reciprocal is broadcast across partitions on
    GpSimd (partition_broadcast - no PE matmul, no PSUM bank), and the
    normalized attention is quantized to fp8 hi/lo pairs on the fly (hi/lo
    writes on GpSimd) for the W_o DoubleRow chain.
  - Output partials are stored bf16; the host sum applies the global descale.
  - Engine budget per head in attention: PE ~5.8us, Act (exp) ~5.3us,
    DVE (rope-free here: recip/attn-copy/t-mult/masks) ~4.7us, Pool
    (broadcast/hi/lo) ~4.8us.
"""
import numpy as np

import concourse.bass as bass  # noqa: F401
import concourse.tile as tile
from contextlib import ExitStack
from concourse import bacc, mybir
from concourse import bass_utils

F32 = mybir.dt.float32
F32R = mybir.dt.float32r
BF16 = mybir.dt.bfloat16
F8 = mybir.dt.float8e4
AF = mybir.ActivationFunctionType
ALU = mybir.AluOpType
DR = mybir.MatmulPerfMode.DoubleRow

B, S, H, NH = 4, 1024, 4096, 32
D = H // NH          # 128
T = B * S            # 4096 tokens
N_CORES = 8
HC = NH // N_CORES   # 4 heads per core
SCALE = float(1.0 / np.sqrt(D))
ROPE_BASE = 10000.0

TB = 256             # phase-1 token block
NTB = T // TB        # 16
KC = H // 128        # 32 fp8 k-chunks of 128 features
S_X = 32.0           # x quant scale
S_W = 2048.0         # W_pack / W_o quant scale
S_A = 32.0           # attention-output quant scale
DESCALE = 1.0 / (S_X * S_W)

_CACHE = {}


def _build_module(phases=("p1", "p2")):
    nc = bacc.Bacc("TRN2", target_bir_lowering=False, debug=False,
                   num_devices=N_CORES)

    # packed fp8 inputs (see _host_prep for layouts)
    xq = nc.dram_tensor("xq", [128, NTB * KC * 2 * TB], F8, kind="ExternalInput").ap()
    wqk = nc.dram_tensor("wqk", [128, 8 * KC * 2 * 128], F8, kind="ExternalInput").ap()
    wv = nc.dram_tensor("wv", [128, 2 * KC * 2 * 256], F8, kind="ExternalInput").ap()
    wo = nc.dram_tensor("wo", [128, HC * 2 * H], F8, kind="ExternalInput").ap()
    cosT = nc.dram_tensor("cosT", [128, T], F32, kind="ExternalInput").ap()
    sinS = nc.dram_tensor("sinS", [128, T], F32, kind="ExternalInput").ap()
    maskD = nc.dram_tensor("maskD", [128, 256], BF16, kind="ExternalInput").ap()
    out_p = nc.dram_tensor("out_p", [T, H], BF16, kind="ExternalOutput").ap()

    import ml_dtypes
    # denominator ones-vector carries 1/S_A so the reciprocal yields
    # S_A/denom directly (the fp8 attn quant scale)
    ones128 = nc.inline_tensor(
        np.full((128, 1), 1.0 / S_A, ml_dtypes.bfloat16), "ones128").ap()

    with tile.TileContext(nc) as tc, \
         nc.allow_low_precision(reason="fp8/bf16 matmuls; verified vs reference"):
        with ExitStack() as octx:
            dram = octx.enter_context(tc.tile_pool(name="dram", bufs=1, space="DRAM"))
            cpool = octx.enter_context(tc.tile_pool(name="consts", bufs=1))
            # DRAM scratch: qkT rows ordered [q0,k0,q1,k1,q2,k2,q3,k3] x d
            qkT_d = dram.tile([8 * 128, T], BF16)
            v_d = dram.tile([T, HC * 128], BF16)

            o128 = cpool.tile([128, 1], BF16)
            nc.sync.dma_start(o128[:], ones128[:])
            # [tri | tri]: the same lower-triangle pattern serves both
            # diagonal tiles (B's triangle is A's shifted by 128 both ways)
            mask_t = cpool.tile([128, 256], BF16)
            nc.sync.dma_start(mask_t[:], maskD[:])

            # phase-2 tiles prefetched during phase 1 (wo_a has no deps; the
            # first head's kq/vt depend on the tb0-3 scratch stores)
            wopool = octx.enter_context(tc.tile_pool(name="p2wo", bufs=1))
            kqpool = octx.enter_context(tc.tile_pool(name="p2kq", bufs=2))
            vtpool = octx.enter_context(tc.tile_pool(name="p2vt", bufs=2))
            _wo_a = [None]
            _first_kv = [None]

            def load_kv(b, l):
                bs = b * S
                kq = kqpool.tile([128, 2, S], BF16, tag="kq")
                nc.sync.dma_start(
                    kq[:],
                    qkT_d[l * 256:(l + 1) * 256, bs:bs + S]
                        .rearrange("(j p) t -> p j t", p=128))
                vt = vtpool.tile([128, 8, 128], BF16, tag="vt")
                nc.sync.dma_start(
                    vt[:],
                    v_d[bs:bs + S, l * 128:(l + 1) * 128]
                        .rearrange("(kt p) d -> p kt d", p=128))
                return kq, vt

            def prefetch_wo():
                # W_o resident: [128, h(4), j(2), c(4096)]; j=0 -> hi, 1 -> lo
                wo_a = wopool.tile([128, HC, 2, H], F8, tag="wo")
                for h in range(HC):
                    nc.sync.dma_start(
                        wo_a[:, h],
                        wo[:, h * 2 * H:(h + 1) * 2 * H]
                            .rearrange("p (j c) -> p j c", j=2))
                _wo_a[0] = wo_a

            def prefetch_kv():
                _first_kv[0] = load_kv(0, 0)

            # ---------------- Phase 1: QKV projection (fp8 DoubleRow) -------
            if "p1" in phases:
              with ExitStack() as ctx:
                wpool = ctx.enter_context(tc.tile_pool(name="p1w", bufs=1))
                xpool = ctx.enter_context(tc.tile_pool(name="p1x", bufs=2))
                opool = ctx.enter_context(tc.tile_pool(name="p1o", bufs=2))
                cspool = ctx.enter_context(tc.tile_pool(name="p1cs", bufs=2))
                rpool = ctx.enter_context(tc.tile_pool(name="p1rope", bufs=3))
                pqk = ctx.enter_context(tc.tile_pool(name="p1pqk", bufs=4, space="PSUM"))
                pv = ctx.enter_context(tc.tile_pool(name="p1pv", bufs=2, space="PSUM"))

                def load_tb(tb):
                    t0 = tb * TB
                    # x pack [128, kk(32), j(2), t(256)]; j=0 -> x_hi, j=1 -> x_lo
                    xall = xpool.tile([128, KC, 2, TB], F8, tag="x")
                    nc.sync.dma_start(
                        xall[:],
                        xq[:, tb * 16384:(tb + 1) * 16384]
                            .rearrange("p (kk j t) -> p kk j t", kk=KC, j=2))
                    cos_tb = cspool.tile([128, TB], F32, tag="cos")
                    nc.sync.dma_start(cos_tb[:], cosT[:, t0:t0 + TB])
                    sin_tb = cspool.tile([128, TB], F32, tag="sin")
                    nc.sync.dma_start(sin_tb[:], sinS[:, t0:t0 + TB])
                    return xall, cos_tb, sin_tb

                # tb0 inputs first (first chain needs x + wqk ct0 only), then
                # resident weights: wqk [128, ct(8), kk(32), j(2), c(128)],
                # wv [128, ct(2), kk(32), j(2), c(256)]; j=0 -> W_lo, j=1 -> W_hi
                tb0_inputs = load_tb(0)
                wqk_a = wpool.tile([128, 8, KC, 2, 128], F8, tag="wqk")
                wv_a = wpool.tile([128, 2, KC, 2, 256], F8, tag="wv")
                for ct in range(8):
                    nc.sync.dma_start(
                        wqk_a[:, ct],
                        wqk[:, ct * 8192:(ct + 1) * 8192]
                            .rearrange("p (kk j c) -> p kk j c", kk=KC, j=2))
                for cv in range(2):
                    nc.sync.dma_start(
                        wv_a[:, cv],
                        wv[:, cv * 16384:(cv + 1) * 16384]
                            .rearrange("p (kk j c) -> p kk j c", kk=KC, j=2))

                def emit_qk(xall, cos_tb, sin_tb, t0):
                    qs_all = opool.tile([128, 8, TB], BF16, tag="qs")
                    for i in range(8):
                        ps = pqk.tile([128, TB], F32, tag="qk")
                        for c in range(16):
                            nc.tensor.matmul(
                                ps[:], wqk_a[:, i, 2 * c:2 * c + 2, 1, :],
                                xall[:, 2 * c:2 * c + 2, 0, :],
                                start=(c == 0), stop=False, perf_mode=DR)
                        for kk in range(KC):
                            nc.tensor.matmul(
                                ps[:], wqk_a[:, i, kk, :, :],
                                xall[:, kk, :, :],
                                start=False, stop=(kk == KC - 1), perf_mode=DR)
                        # RoPE epilogue (psum scale folded into cos/sin tables)
                        rot = rpool.tile([128, TB], F32, tag="rot")
                        nc.scalar.copy(rot[0:64, :], ps[64:128, :])
                        nc.vector.tensor_copy(rot[64:128, :], ps[0:64, :])
                        m1_ = rpool.tile([128, TB], F32, tag="m1")
                        nc.vector.tensor_tensor(m1_[:], ps[:], cos_tb[:], op=ALU.mult)
                        m2_ = rpool.tile([128, TB], F32, tag="m2")
                        nc.vector.tensor_tensor(m2_[:], rot[:], sin_tb[:], op=ALU.mult)
                        nc.vector.tensor_tensor(qs_all[:, i, :], m1_[:], m2_[:],
                                                op=ALU.add)
                    nc.sync.dma_start(
                        qkT_d[:, t0:t0 + TB].rearrange("(i p) t -> p i t", p=128),
                        qs_all[:])

                def emit_v(xall, t0):
                    vs_all = opool.tile([128, 2, 2, 256], BF16, tag="vs")
                    for th in range(2):
                        for ch in range(2):
                            ps = pv.tile([128, 256], F32, tag="v")
                            for c in range(16):
                                nc.tensor.matmul(
                                    ps[:],
                                    xall[:, 2 * c:2 * c + 2, 0,
                                         th * 128:(th + 1) * 128],
                                    wv_a[:, ch, 2 * c:2 * c + 2, 1, :],
                                    start=(c == 0), stop=False, perf_mode=DR)
                            for kk in range(KC):
                                nc.tensor.matmul(
                                    ps[:],
                                    xall[:, kk, :, th * 128:(th + 1) * 128],
                                    wv_a[:, ch, kk, :, :],
                                    start=False, stop=(kk == KC - 1), perf_mode=DR)
                            nc.scalar.activation(vs_all[:, th, ch, :], ps[:],
                                                 AF.Copy, scale=DESCALE)
                    nc.sync.dma_start(
                        v_d[t0:t0 + TB, :]
                            .rearrange("(th p) (ch c) -> p th ch c", p=128, ch=2),
                        vs_all[:])

                # v(0) is deferred until after qk(1): tb0's PE work then
                # needs only x+wqk, hiding the wv weight-load latency
                deferred_v0 = [None]
                for tb in range(NTB):
                    t0 = tb * TB
                    if tb == 0:
                        xall, cos_tb, sin_tb = tb0_inputs
                    else:
                        xall, cos_tb, sin_tb = load_tb(tb)
                    if tb == 1:
                        prefetch_wo()
                    elif tb == 4:
                        prefetch_kv()
                    emit_qk(xall, cos_tb, sin_tb, t0)
                    if tb == 0:
                        deferred_v0[0] = (xall, t0)
                    else:
                        if deferred_v0[0] is not None:
                            emit_v(*deferred_v0[0])
                            deferred_v0[0] = None
                        emit_v(xall, t0)

            # ---------------- Phase 2: attention + W_o ----------------------
            if "p2" in phases:
              with ExitStack() as ctx:
                apool = ctx.enter_context(tc.tile_pool(name="p2a", bufs=2))
                efpool = ctx.enter_context(tc.tile_pool(name="p2ef", bufs=14))
                tpool = ctx.enter_context(tc.tile_pool(name="p2t", bufs=3))
                rpool2 = ctx.enter_context(tc.tile_pool(name="p2rd", bufs=3))
                opool = ctx.enter_context(tc.tile_pool(name="p2o", bufs=2))
                ps_s = ctx.enter_context(tc.tile_pool(name="p2ps", bufs=2, space="PSUM"))
                ps_av = ctx.enter_context(tc.tile_pool(name="p2pav", bufs=1, space="PSUM"))
                ps_d = ctx.enter_context(tc.tile_pool(name="p2pd", bufs=1, space="PSUM"))
                ps_o = ctx.enter_context(tc.tile_pool(name="p2po", bufs=2, space="PSUM"))

                if _wo_a[0] is None:     # p2-only debug build
                    prefetch_wo()
                    prefetch_kv()
                wo_a = _wo_a[0]
                for b in range(B):
                    bs = b * S
                    # attn pack [128, lh(2), l(4), t(1024)]; lh=0 -> lo, 1 -> hi
                    apack = apool.tile([128, 2, HC, S], F8, tag="apack")
                    pending = [None]
                    for l in range(HC):
                        if b == 0 and l == 0:
                            kq, vt = _first_kv[0]
                        else:
                            kq, vt = load_kv(b, l)

                        psd_l = ps_d.tile([1, 512], F32, tag="dbc")
                        psav_l = ps_av.tile([128, 512], F32, tag="av")
                        all_efs = {}

                        def emit_scores(qb):
                            u = 2 * qb + 2
                            q_sl = kq[:, 0, qb * 256:(qb + 1) * 256]
                            efs = [None] * u
                            # diagonal pair first: its exp+mask latency hides
                            # behind the remaining pairs' matmuls
                            for g in [qb] + list(range(qb)):
                                pss = ps_s.tile([128, 512], F32, tag="s")
                                if g == qb:
                                    # diagonal pair: tile A is full; tile B
                                    # only sees the top query half, packed
                                    # right after A so the exp is [128,384]
                                    nc.tensor.matmul(
                                        pss[:, 0:256],
                                        kq[:, 1, 2 * g * 128:(2 * g + 1) * 128],
                                        q_sl, start=True, stop=True)
                                    nc.tensor.matmul(
                                        pss[:, 256:384],
                                        kq[:, 1, (2 * g + 1) * 128:(2 * g + 2) * 128],
                                        q_sl[:, 128:256], start=True, stop=True)
                                    ef = efpool.tile([128, 384], BF16, tag="ef")
                                    nc.scalar.activation(ef[:], pss[:, 0:384],
                                                         AF.Exp, scale=SCALE)
                                    # in-place triangular mask on A's left
                                    # quarter and B's live quarter (same
                                    # pattern), one strided DVE op
                                    quarters = ef[:].rearrange(
                                        "p (g c) -> p g c", g=3)[:, ::2, :]
                                    nc.vector.tensor_tensor(
                                        quarters, quarters,
                                        mask_t[:].rearrange(
                                            "p (j c) -> p j c", j=2),
                                        op=ALU.mult)
                                    efs[2 * g] = ef[:, 0:256]
                                    efs[2 * g + 1] = ef[:, 256:384]
                                else:
                                    for sHalf in range(2):
                                        mt = 2 * g + sHalf
                                        nc.tensor.matmul(
                                            pss[:, sHalf * 256:(sHalf + 1) * 256],
                                            kq[:, 1, mt * 128:(mt + 1) * 128],
                                            q_sl, start=True, stop=True)
                                    ef = efpool.tile([128, 512], BF16, tag="ef")
                                    nc.scalar.activation(ef[:], pss[:], AF.Exp,
                                                         scale=SCALE)
                                    efs[2 * g] = ef[:, 0:256]
                                    efs[2 * g + 1] = ef[:, 256:512]
                            all_efs[qb] = efs

                        def emit_pv(qb):
                            u = 2 * qb + 2
                            efs = all_efs.pop(qb)
                            # masked diagonal units last in the chains
                            order = list(range(2 * qb)) + [2 * qb, 2 * qb + 1]
                            if pending[0] is not None:
                                pending[0]()
                                pending[0] = None
                            # the final (diagonal-B) unit only covers the top
                            # query half: half-width accumulation step
                            psav = psav_l[:, (qb % 2) * 256:(qb % 2 + 1) * 256]
                            for n, mt in enumerate(order):
                                half = mt == 2 * qb + 1
                                nc.tensor.matmul(
                                    psav[:, 128:256] if half else psav,
                                    vt[:, mt, :], efs[mt],
                                    start=(n == 0), stop=(n == u - 1),
                                    skip_group_check=True)
                            psd = psd_l[:, (qb % 2) * 256:(qb % 2 + 1) * 256]
                            for n, mt in enumerate(order):
                                half = mt == 2 * qb + 1
                                nc.tensor.matmul(
                                    psd[:, 128:256] if half else psd,
                                    o128[:], efs[mt],
                                    start=(n == 0), stop=(n == u - 1),
                                    skip_group_check=True)
                            if qb % 2 == 0:
                                return
                            # pair epilogue (qb-1, qb): unnormalized attn to
                            # SBUF (frees the psum bank), denominators to
                            # reciprocal; the normalization + fp8 hi/lo split
                            # is deferred into the next PV block
                            rd = rpool2.tile([1, 512], F32, tag="rd")
                            nc.vector.reciprocal(rd[:], psd_l[:])
                            av_s = tpool.tile([128, 512], F32, tag="avs")
                            nc.vector.tensor_copy(av_s[:], psav_l[:])

                            def make_epilogue(qb=qb, av_s=av_s, rd=rd, l=l,
                                              apack=apack):
                                def emit():
                                    # s_a/denom broadcast across partitions on
                                    # GpSimd: no PE matmul, no PSUM bank
                                    bc = tpool.tile([128, 512], F32, tag="bc")
                                    nc.gpsimd.partition_broadcast(bc[:], rd[:])
                                    t_ = tpool.tile([128, 512], F32, tag="t")
                                    nc.vector.tensor_tensor(t_[:], av_s[:],
                                                            bc[:], op=ALU.mult)
                                    q0 = (qb - 1) * 256
                                    hi = apack[:, 1, l, q0:q0 + 512]
                                    nc.gpsimd.tensor_copy(hi, t_[:])
                                    nc.gpsimd.tensor_tensor(
                                        apack[:, 0, l, q0:q0 + 512], t_[:], hi,
                                        op=ALU.subtract)
                                return emit
                            pending[0] = make_epilogue()

                        # all scores (and their masks) are emitted before any
                        # PV block: every engine queue sees the masks first
                        emit_scores(0)
                        emit_scores(1)
                        emit_scores(2)
                        emit_scores(3)
                        emit_pv(0)
                        emit_pv(1)
                        emit_pv(2)
                        emit_pv(3)
                    # W_o projection for batch b (fp8 DoubleRow main+corr);
                    # two 256-col chains per PSUM bank, one wide copy each.
                    # The last pair epilogue (l=3, qb 2-3) flushes after the
                    # first m-block, which only reads early tokens.
                    for m in range(8):
                        osb = opool.tile([128, 4, 1024], BF16, tag="osb")
                        msl = slice(m * 128, (m + 1) * 128)
                        for quad in range(4):
                            pso = ps_o.tile([128, 1024], F32, tag="o")
                            for part in range(4):
                                csl = slice((4 * quad + part) * 256,
                                            (4 * quad + part + 1) * 256)
                                po = pso[:, part * 256:(part + 1) * 256]
                                for c in range(2):
                                    nc.tensor.matmul(
                                        po, apack[:, 1, 2 * c:2 * c + 2, msl],
                                        wo_a[:, 2 * c:2 * c + 2, 0, csl],
                                        start=(c == 0), stop=False, perf_mode=DR)
                                for h in range(HC):
                                    nc.tensor.matmul(
                                        po, apack[:, :, h, msl],
                                        wo_a[:, h, :, csl],
                                        start=False, stop=(h == HC - 1),
                                        perf_mode=DR)
                            if quad % 2 == 0:
                                nc.vector.tensor_copy(osb[:, quad, :], pso[:])
                            else:
                                nc.scalar.copy(osb[:, quad, :], pso[:])
                            if b == B - 1 and m == 7:
                                # stream the final stores per quad so the
                                # end-of-kernel drain tail is minimal
                                nc.sync.dma_start(
                                    out_p[bs + m * 128:bs + (m + 1) * 128,
                                          quad * 1024:(quad + 1) * 1024],
                                    osb[:, quad, :])
                        if b != B - 1 or m != 7:
                            nc.sync.dma_start(
                                out_p[bs + m * 128:bs + (m + 1) * 128, :],
                                osb[:].rearrange("p nc c -> p (nc c)"))
                        if m == 0 and pending[0] is not None:
                            pending[0]()
                            pending[0] = None
    nc.compile()
    return nc


def _q8hl(a, scale):
    """Quantize to fp8 e4m3 hi/lo pair at a shared scale."""
    import ml_dtypes
    hi = (a * scale).astype(ml_dtypes.float8_e4m3)
    lo = ((a * scale) - hi.astype(np.float32)).astype(ml_dtypes.float8_e4m3)
    return hi, lo


def _host_prep(hidden_states, W_pack, W_o, attention_mask, position_ids):
    import ml_dtypes
    x = np.asarray(hidden_states, dtype=np.float32).reshape(T, H)
    W_pack = np.asarray(W_pack, dtype=np.float32)
    W_o = np.asarray(W_o, dtype=np.float32)
    mask = np.asarray(attention_mask, dtype=np.float32)
    pos = np.asarray(position_ids)

    # causal structure is hardcoded in the kernel; verify it holds
    m0 = mask[0, 0]
    iu = np.triu_indices(S, 1)
    assert (m0[iu] < -1e8).all() and (np.tril(m0) == 0).all(), \
        "kernel requires the standard causal mask"

    # x pack: [128p, tb, kk, j(hi,lo), t] -> flat [128, NTB*KC*2*TB]
    xh, xl = _q8hl(x, S_X)
    xv_h = xh.reshape(NTB, TB, KC, 128).transpose(3, 0, 2, 1)
    xv_l = xl.reshape(NTB, TB, KC, 128).transpose(3, 0, 2, 1)
    xq_np = np.empty((128, NTB, KC, 2, TB), ml_dtypes.float8_e4m3)
    xq_np[:, :, :, 0, :] = xv_h
    xq_np[:, :, :, 1, :] = xv_l
    xq_np = np.ascontiguousarray(xq_np.reshape(128, -1))

    # rope tables with the fp8 descale folded in; rotate-half sign in sinS
    inv = 1.0 / (ROPE_BASE ** (np.arange(0, D, 2, dtype=np.float64) / D))
    inv = np.concatenate([inv, inv])
    ang = pos.astype(np.float64).reshape(T)[None, :] * inv[:, None]   # [D, T]
    cosT_np = np.ascontiguousarray((np.cos(ang) * DESCALE).astype(np.float32))
    sinT = (np.sin(ang) * DESCALE).astype(np.float32)
    sinS_np = sinT.copy()
    sinS_np[:64] = -sinT[:64]
    sinS_np = np.ascontiguousarray(sinS_np)

    # diagonal exp-mask triangle [128p(key), 128(query)], duplicated so one
    # strided DVE op covers both diagonal tiles' live quarters
    em = np.exp(m0)
    tri = em[0:128, 0:128].T.astype(ml_dtypes.bfloat16)   # [p(key), t(query)]
    maskD_np = np.ascontiguousarray(
        np.concatenate([tri, tri], axis=1))               # [128, 256]

    in_maps = []
    for core in range(N_CORES):
        h0 = core * HC
        # wqk cols ordered [q0,k0,q1,k1,q2,k2,q3,k3] per head slice
        cols = []
        for l in range(HC):
            cols.append(W_pack[:, (h0 + l) * D:(h0 + l + 1) * D])
            cols.append(W_pack[:, H + (h0 + l) * D:H + (h0 + l + 1) * D])
        wqk_f = np.concatenate(cols, axis=1)              # [H, 1024]
        wh, wl = _q8hl(wqk_f, S_W)
        wv_h = wh.reshape(KC, 128, 8, 128).transpose(1, 2, 0, 3)
        wv_l = wl.reshape(KC, 128, 8, 128).transpose(1, 2, 0, 3)
        wqk_np = np.empty((128, 8, KC, 2, 128), ml_dtypes.float8_e4m3)
        wqk_np[:, :, :, 0, :] = wv_l
        wqk_np[:, :, :, 1, :] = wv_h
        wqk_np = np.ascontiguousarray(wqk_np.reshape(128, -1))

        wv_f = np.concatenate(
            [W_pack[:, 2 * H + (h0 + l) * D:2 * H + (h0 + l + 1) * D]
             for l in range(HC)], axis=1)                 # [H, 512]
        wh, wl = _q8hl(wv_f, S_W)
        wvv_h = wh.reshape(KC, 128, 2, 256).transpose(1, 2, 0, 3)
        wvv_l = wl.reshape(KC, 128, 2, 256).transpose(1, 2, 0, 3)
        wv_np = np.empty((128, 2, KC, 2, 256), ml_dtypes.float8_e4m3)
        wv_np[:, :, :, 0, :] = wvv_l
        wv_np[:, :, :, 1, :] = wvv_h
        wv_np = np.ascontiguousarray(wv_np.reshape(128, -1))

        wo_f = W_o[h0 * D:(h0 + HC) * D, :]               # [512, H]
        wh, wl = _q8hl(wo_f, S_W)
        wov_h = wh.reshape(HC, 128, H).transpose(1, 0, 2)
        wov_l = wl.reshape(HC, 128, H).transpose(1, 0, 2)
        wo_np = np.empty((128, HC, 2, H), ml_dtypes.float8_e4m3)
        wo_np[:, :, 0, :] = wov_h
        wo_np[:, :, 1, :] = wov_l
        wo_np = np.ascontiguousarray(wo_np.reshape(128, -1))

        in_maps.append({
            "xq": xq_np, "wqk": wqk_np, "wv": wv_np, "wo": wo_np,
            "cosT": cosT_np, "sinS": sinS_np, "maskD": maskD_np,
        })
    return in_maps


def kernel(hidden_states, W_pack, W_o, attention_mask, position_ids):
    if "nc" not in _CACHE:
        _CACHE["nc"] = _build_module()
    nc = _CACHE["nc"]
    in_maps = _host_prep(hidden_states, W_pack, W_o, attention_mask, position_ids)
    res = bass_utils.run_bass_kernel_spmd(nc, in_maps, core_ids=list(range(N_CORES)))
    out = res.results[0]["out_p"].astype(np.float32)
    for c in range(1, N_CORES):
        out += res.results[c]["out_p"]
    out *= 1.0 / (S_A * S_W)
    return out.reshape(B, S, H).astype(np.float32)


# revision 81
# speedup vs baseline: 1.0233x; 1.0084x over previous
"""Trainium2 Bass kernel for nn_Attention_60567628808865.

Dense transformer attention block (B=4, S=1024, H=4096, NH=32, D=128):
  qkv = x @ W_pack; RoPE(q, k); causal-masked softmax attention; out @ W_o.

Sharding: tensor-parallel over heads across 8 NeuronCores. Each core computes
4 heads end-to-end; the host sums the 8 partial W_o outputs (row-sharded W_o).

Precision/performance scheme (validated on host to rel_err ~2.7e-3 vs the
2e-2 gate):
  - QKV and W_o projections run in fp8(e4m3) with the DoubleRow perf mode
    (K=256 per instruction, 0.5 cycles/row) using an exact-style two-term
    decomposition: x@W ~= x_hi@W_hi + [x_hi@W_lo + x_lo@W_hi], where
    t_hi = fp8(t*s), t_lo = fp8(t*s - t_hi). Both terms accumulate into ONE
    PSUM chain (identical scale), so the epilogue is unchanged. 48 DoubleRow
    instructions replace 32 f32r instructions per [128col x 256tok] unit:
    0.75x PE cycles.
  - hi/lo operands are slot-interleaved in a single packed tensor
    ([part, chunk, 2, free]) so the correction chain reads (hi,lo) slot pairs
    and the main chain reads (hi,hi) chunk pairs from the same SBUF bytes.
  - Attention is causal-aware: score/PV/denominator work is emitted only for
    the 20/32 key-tile x query-block units on or below the diagonal; the
    second diagonal tile of each query block runs at half moving-width (its
    lower query half is fully masked), and both diagonal triangles are masked
    in-place with one strided DVE multiply against a single host-built
    exp(mask) triangle (mask asserted causal). q/k/v round-trip DRAM in bf16;
    scores/PV matmuls run in bf16 (same PE rate as f32r, half the DMA).
  - Softmax is unnormalized; denominators come from a (1/s_a)-vector matmul
    accumulated in PSUM; the reciprocal is broadcast across partitions on
    GpSimd (partition_broadcast - no PE matmul, no PSUM bank), and the
    normalized attention is quantized to fp8 hi/lo pairs on the fly (hi/lo
    writes on GpSimd) for the W_o DoubleRow chain.
  - Output partials are stored bf16; the host sum applies the global descale.
  - Engine budget per head in attention: PE ~5.8us, Act (exp) ~5.3us,
    DVE (rope-free here: recip/attn-copy/t-mult/masks) ~4.7us, Pool
    (broadcast/hi/lo) ~4.8us.
"""
import numpy as np

import concourse.bass as bass  # noqa: F401
import concourse.tile as tile
from contextlib import ExitStack
from concourse import bacc, mybir
from concourse import bass_utils

F32 = mybir.dt.float32
F32R = mybir.dt.float32r
BF16 = mybir.dt.bfloat16
F8 = mybir.dt.float8e4
AF = mybir.ActivationFunctionType
ALU = mybir.AluOpType
DR = mybir.MatmulPerfMode.DoubleRow

B, S, H, NH = 4, 1024, 4096, 32
D = H // NH          # 128
T = B * S            # 4096 tokens
N_CORES = 8
HC = NH // N_CORES   # 4 heads per core
SCALE = float(1.0 / np.sqrt(D))
ROPE_BASE = 10000.0

TB = 256             # phase-1 token block
NTB = T // TB        # 16
KC = H // 128        # 32 fp8 k-chunks of 128 features
S_X = 32.0           # x quant scale
S_W = 2048.0         # W_pack / W_o quant scale
S_A = 32.0           # attention-output quant scale
DESCALE = 1.0 / (S_X * S_W)

_CACHE = {}


def _build_module(phases=("p1", "p2")):
    nc = bacc.Bacc("TRN2", target_bir_lowering=False, debug=False,
                   num_devices=N_CORES)

    # packed fp8 inputs (see _host_prep for layouts)
    xq = nc.dram_tensor("xq", [128, NTB * KC * 2 * TB], F8, kind="ExternalInput").ap()
    wqk = nc.dram_tensor("wqk", [128, 8 * KC * 2 * 128], F8, kind="ExternalInput").ap()
    wv = nc.dram_tensor("wv", [128, 2 * KC * 2 * 256], F8, kind="ExternalInput").ap()
    wo = nc.dram_tensor("wo", [128, HC * 2 * H], F8, kind="ExternalInput").ap()
    cosT = nc.dram_tensor("cosT", [128, T], F32, kind="ExternalInput").ap()
    sinS = nc.dram_tensor("sinS", [128, T], F32, kind="ExternalInput").ap()
    maskD = nc.dram_tensor("maskD", [128, 256], BF16, kind="ExternalInput").ap()
    out_p = nc.dram_tensor("out_p", [T, H], BF16, kind="ExternalOutput").ap()

    import ml_dtypes
    # denominator ones-vector carries 1/S_A so the reciprocal yields
    # S_A/denom directly (the fp8 attn quant scale)
    ones128 = nc.inline_tensor(
        np.full((128, 1), 1.0 / S_A, ml_dtypes.bfloat16), "ones128").ap()

    with tile.TileContext(nc) as tc, \
         nc.allow_low_precision(reason="fp8/bf16 matmuls; verified vs reference"):
        with ExitStack() as octx:
            dram = octx.enter_context(tc.tile_pool(name="dram", bufs=1, space="DRAM"))
            cpool = octx.enter_context(tc.tile_pool(name="consts", bufs=1))
            # DRAM scratch: qkT rows ordered [q0,k0,q1,k1,q2,k2,q3,k3] x d
            qkT_d = dram.tile([8 * 128, T], BF16)
            v_d = dram.tile([T, HC * 128], BF16)

            o128 = cpool.tile([128, 1], BF16)
            nc.sync.dma_start(o128[:], ones128[:])
            # [tri | tri]: the same lower-triangle pattern serves both
            # diagonal tiles (B's triangle is A's shifted by 128 both ways)
            mask_t = cpool.tile([128, 256], BF16)
            nc.sync.dma_start(mask_t[:], maskD[:])

            # phase-2 tiles prefetched during phase 1 (wo_a has no deps; the
            # first head's kq/vt depend on the tb0-3 scratch stores)
            wopool = octx.enter_context(tc.tile_pool(name="p2wo", bufs=1))
            kqpool = octx.enter_context(tc.tile_pool(name="p2kq", bufs=2))
            vtpool = octx.enter_context(tc.tile_pool(name="p2vt", bufs=2))
            _wo_a = [None]
            _first_kv = [None]

            def load_kv(b, l):
                bs = b * S
                kq = kqpool.tile([128, 2, S], BF16, tag="kq")
                nc.sync.dma_start(
                    kq[:],
                    qkT_d[l * 256:(l + 1) * 256, bs:bs + S]
                        .rearrange("(j p) t -> p j t", p=128))
                vt = vtpool.tile([128, 8, 128], BF16, tag="vt")
                nc.sync.dma_start(
                    vt[:],
                    v_d[bs:bs + S, l * 128:(l + 1) * 128]
                        .rearrange("(kt p) d -> p kt d", p=128))
                return kq, vt

            def prefetch_wo():
                # W_o resident: [128, h(4), j(2), c(4096)]; j=0 -> hi, 1 -> lo
                wo_a = wopool.tile([128, HC, 2, H], F8, tag="wo")
                for h in range(HC):
                    nc.sync.dma_start(
                        wo_a[:, h],
                        wo[:, h * 2 * H:(h + 1) * 2 * H]
                            .rearrange("p (j c) -> p j c", j=2))
                _wo_a[0] = wo_a

            def prefetch_kv():
                _first_kv[0] = load_kv(0, 0)

            # ---------------- Phase 1: QKV projection (fp8 DoubleRow) -------
            if "p1" in phases:
              with ExitStack() as ctx:
                wpool = ctx.enter_context(tc.tile_pool(name="p1w", bufs=1))
                xpool = ctx.enter_context(tc.tile_pool(name="p1x", bufs=2))
                opool = ctx.enter_context(tc.tile_pool(name="p1o", bufs=2))
                cspool = ctx.enter_context(tc.tile_pool(name="p1cs", bufs=2))
                rpool = ctx.enter_context(tc.tile_pool(name="p1rope", bufs=3))
                pqk = ctx.enter_context(tc.tile_pool(name="p1pqk", bufs=4, space="PSUM"))
                pv = ctx.enter_context(tc.tile_pool(name="p1pv", bufs=2, space="PSUM"))

                def load_tb(tb):
                    t0 = tb * TB
                    # x pack [128, kk(32), j(2), t(256)]; j=0 -> x_hi, j=1 -> x_lo
                    xall = xpool.tile([128, KC, 2, TB], F8, tag="x")
                    nc.sync.dma_start(
                        xall[:],
                        xq[:, tb * 16384:(tb + 1) * 16384]
                            .rearrange("p (kk j t) -> p kk j t", kk=KC, j=2))
                    cos_tb = cspool.tile([128, TB], F32, tag="cos")
                    nc.sync.dma_start(cos_tb[:], cosT[:, t0:t0 + TB])
                    sin_tb = cspool.tile([128, TB], F32, tag="sin")
                    nc.sync.dma_start(sin_tb[:], sinS[:, t0:t0 + TB])
                    return xall, cos_tb, sin_tb

                # tb0 inputs first (first chain needs x + wqk ct0 only), then
                # resident weights: wqk [128, ct(8), kk(32), j(2), c(128)],
                # wv [128, ct(2), kk(32), j(2), c(256)]; j=0 -> W_lo, j=1 -> W_hi
                tb0_inputs = load_tb(0)
                wqk_a = wpool.tile([128, 8, KC, 2, 128], F8, tag="wqk")
                wv_a = wpool.tile([128, 2, KC, 2, 256], F8, tag="wv")
                for ct in range(8):
                    nc.sync.dma_start(
                        wqk_a[:, ct],
                        wqk[:, ct * 8192:(ct + 1) * 8192]
                            .rearrange("p (kk j c) -> p kk j c", kk=KC, j=2))
                for cv in range(2):
                    nc.sync.dma_start(
                        wv_a[:, cv],
                        wv[:, cv * 16384:(cv + 1) * 16384]
                            .rearrange("p (kk j c) -> p kk j c", kk=KC, j=2))

                def emit_qk(xall, cos_tb, sin_tb, t0):
                    qs_all = opool.tile([128, 8, TB], BF16, tag="qs")
                    for i in range(8):
                        ps = pqk.tile([128, TB], F32, tag="qk")
                        for c in range(16):
                            nc.tensor.matmul(
                                ps[:], wqk_a[:, i, 2 * c:2 * c + 2, 1, :],
                                xall[:, 2 * c:2 * c + 2, 0, :],
                                start=(c == 0), stop=False, perf_mode=DR)
                        for kk in range(KC):
                            nc.tensor.matmul(
                                ps[:], wqk_a[:, i, kk, :, :],
                                xall[:, kk, :, :],
                                start=False, stop=(kk == KC - 1), perf_mode=DR)
                        # RoPE epilogue (psum scale folded into cos/sin tables)
                        rot = rpool.tile([128, TB], F32, tag="rot")
                        nc.scalar.copy(rot[0:64, :], ps[64:128, :])
                        nc.vector.tensor_copy(rot[64:128, :], ps[0:64, :])
                        m1_ = rpool.tile([128, TB], F32, tag="m1")
                        nc.vector.tensor_tensor(m1_[:], ps[:], cos_tb[:], op=ALU.mult)
                        m2_ = rpool.tile([128, TB], F32, tag="m2")
                        nc.vector.tensor_tensor(m2_[:], rot[:], sin_tb[:], op=ALU.mult)
                        nc.vector.tensor_tensor(qs_all[:, i, :], m1_[:], m2_[:],
                                                op=ALU.add)
                    nc.sync.dma_start(
                        qkT_d[:, t0:t0 + TB].rearrange("(i p) t -> p i t", p=128),
                        qs_all[:])

                def emit_v(xall, t0):
                    vs_all = opool.tile([128, 2, 2, 256], BF16, tag="vs")
                    for th in range(2):
                        for ch in range(2):
                            ps = pv.tile([128, 256], F32, tag="v")
                            for c in range(16):
                                nc.tensor.matmul(
                                    ps[:],
                                    xall[:, 2 * c:2 * c + 2, 0,
                                         th * 128:(th + 1) * 128],
                                    wv_a[:, ch, 2 * c:2 * c + 2, 1, :],
                                    start=(c == 0), stop=False, perf_mode=DR)
                            for kk in range(KC):
                                nc.tensor.matmul(
                                    ps[:],
                                    xall[:, kk, :, th * 128:(th + 1) * 128],
                                    wv_a[:, ch, kk, :, :],
                                    start=False, stop=(kk == KC - 1), perf_mode=DR)
                            nc.scalar.activation(vs_all[:, th, ch, :], ps[:],
                                                 AF.Copy, scale=DESCALE)
                    nc.sync.dma_start(
                        v_d[t0:t0 + TB, :]
                            .rearrange("(th p) (ch c) -> p th ch c", p=128, ch=2),
                        vs_all[:])

                # v(0) is deferred until after qk(1): tb0's PE work then
                # needs only x+wqk, hiding the wv weight-load latency
                deferred_v0 = [None]
                for tb in range(NTB):
                    t0 = tb * TB
                    if tb == 0:
                        xall, cos_tb, sin_tb = tb0_inputs
                    else:
                        xall, cos_tb, sin_tb = load_tb(tb)
                    if tb == 2:
                        prefetch_wo()
                    elif tb == 4:
                        prefetch_kv()
                    emit_qk(xall, cos_tb, sin_tb, t0)
                    if tb == 0:
                        deferred_v0[0] = (xall, t0)
                    else:
                        if deferred_v0[0] is not None:
                            emit_v(*deferred_v0[0])
                            deferred_v0[0] = None
                        emit_v(xall, t0)

            # ---------------- Phase 2: attention + W_o ----------------------
            if "p2" in phases:
              with ExitStack() as ctx:
                apool = ctx.enter_context(tc.tile_pool(name="p2a", bufs=2))
                efpool = ctx.enter_context(tc.tile_pool(name="p2ef", bufs=14))
                tpool = ctx.enter_context(tc.tile_pool(name="p2t", bufs=3))
                rpool2 = ctx.enter_context(tc.tile_pool(name="p2rd", bufs=3))
                opool = ctx.enter_context(tc.tile_pool(name="p2o", bufs=2))
                ps_s = ctx.enter_context(tc.tile_pool(name="p2ps", bufs=3, space="PSUM"))
                ps_av = ctx.enter_context(tc.tile_pool(name="p2pav", bufs=1, space="PSUM"))
                ps_d = ctx.enter_context(tc.tile_pool(name="p2pd", bufs=1, space="PSUM"))
                ps_o = ctx.enter_context(tc.tile_pool(name="p2po", bufs=3, space="PSUM"))

                if _wo_a[0] is None:     # p2-only debug build
                    prefetch_wo()
                    prefetch_kv()
                wo_a = _wo_a[0]
                kv_stash = {}
                for b in range(B):
                    bs = b * S
                    # attn pack [128, lh(2), l(4), t(1024)]; lh=0 -> lo, 1 -> hi
                    apack = apool.tile([128, 2, HC, S], F8, tag="apack")
                    pending = [None]
                    for l in range(HC):
                        if b == 0 and l == 0:
                            kq, vt = _first_kv[0]
                        elif (b, l) in kv_stash:
                            kq, vt = kv_stash.pop((b, l))
                        else:
                            kq, vt = load_kv(b, l)

                        psd_l = ps_d.tile([1, 512], F32, tag="dbc")
                        psav_l = ps_av.tile([128, 512], F32, tag="av")
                        all_efs = {}

                        def emit_scores(qb):
                            u = 2 * qb + 2
                            q_sl = kq[:, 0, qb * 256:(qb + 1) * 256]
                            efs = [None] * u
                            # diagonal pair first: its exp+mask latency hides
                            # behind the remaining pairs' matmuls
                            for g in [qb] + list(range(qb)):
                                pss = ps_s.tile([128, 512], F32, tag="s")
                                if g == qb:
                                    # diagonal pair: tile A is full; tile B
                                    # only sees the top query half, packed
                                    # right after A so the exp is [128,384]
                                    nc.tensor.matmul(
                                        pss[:, 0:256],
                                        kq[:, 1, 2 * g * 128:(2 * g + 1) * 128],
                                        q_sl, start=True, stop=True)
                                    nc.tensor.matmul(
                                        pss[:, 256:384],
                                        kq[:, 1, (2 * g + 1) * 128:(2 * g + 2) * 128],
                                        q_sl[:, 128:256], start=True, stop=True)
                                    ef = efpool.tile([128, 384], BF16, tag="ef")
                                    nc.scalar.activation(ef[:], pss[:, 0:384],
                                                         AF.Exp, scale=SCALE)
                                    # in-place triangular mask on A's left
                                    # quarter and B's live quarter (same
                                    # pattern), one strided DVE op
                                    quarters = ef[:].rearrange(
                                        "p (g c) -> p g c", g=3)[:, ::2, :]
                                    nc.vector.tensor_tensor(
                                        quarters, quarters,
                                        mask_t[:].rearrange(
                                            "p (j c) -> p j c", j=2),
                                        op=ALU.mult)
                                    efs[2 * g] = ef[:, 0:256]
                                    efs[2 * g + 1] = ef[:, 256:384]
                                else:
                                    for sHalf in range(2):
                                        mt = 2 * g + sHalf
                                        nc.tensor.matmul(
                                            pss[:, sHalf * 256:(sHalf + 1) * 256],
                                            kq[:, 1, mt * 128:(mt + 1) * 128],
                                            q_sl, start=True, stop=True)
                                    ef = efpool.tile([128, 512], BF16, tag="ef")
                                    nc.scalar.activation(ef[:], pss[:], AF.Exp,
                                                         scale=SCALE)
                                    efs[2 * g] = ef[:, 0:256]
                                    efs[2 * g + 1] = ef[:, 256:512]
                            all_efs[qb] = efs

                        def emit_pv(qb):
                            u = 2 * qb + 2
                            efs = all_efs.pop(qb)
                            # masked diagonal units last in the chains
                            order = list(range(2 * qb)) + [2 * qb, 2 * qb + 1]
                            if pending[0] is not None:
                                pending[0]()
                                pending[0] = None
                            # the final (diagonal-B) unit only covers the top
                            # query half: half-width accumulation step
                            psav = psav_l[:, (qb % 2) * 256:(qb % 2 + 1) * 256]
                            for n, mt in enumerate(order):
                                half = mt == 2 * qb + 1
                                nc.tensor.matmul(
                                    psav[:, 128:256] if half else psav,
                                    vt[:, mt, :], efs[mt],
                                    start=(n == 0), stop=(n == u - 1),
                                    skip_group_check=True)
                            psd = psd_l[:, (qb % 2) * 256:(qb % 2 + 1) * 256]
                            for n, mt in enumerate(order):
                                half = mt == 2 * qb + 1
                                nc.tensor.matmul(
                                    psd[:, 128:256] if half else psd,
                                    o128[:], efs[mt],
                                    start=(n == 0), stop=(n == u - 1),
                                    skip_group_check=True)
                            if qb % 2 == 0:
                                return
                            # pair epilogue (qb-1, qb): unnormalized attn to
                            # SBUF (frees the psum bank), denominators to
                            # reciprocal; the normalization + fp8 hi/lo split
                            # is deferred into the next PV block
                            rd = rpool2.tile([1, 512], F32, tag="rd")
                            nc.vector.reciprocal(rd[:], psd_l[:])
                            av_s = tpool.tile([128, 512], F32, tag="avs")
                            nc.vector.tensor_copy(av_s[:], psav_l[:])

                            def make_epilogue(qb=qb, av_s=av_s, rd=rd, l=l,
                                              apack=apack):
                                def emit():
                                    # s_a/denom broadcast across partitions on
                                    # GpSimd: no PE matmul, no PSUM bank
                                    bc = tpool.tile([128, 512], F32, tag="bc")
                                    nc.gpsimd.partition_broadcast(bc[:], rd[:])
                                    t_ = tpool.tile([128, 512], F32, tag="t")
                                    nc.vector.tensor_tensor(t_[:], av_s[:],
                                                            bc[:], op=ALU.mult)
                                    q0 = (qb - 1) * 256
                                    hi = apack[:, 1, l, q0:q0 + 512]
                                    # last head: W_o waits on these writes and
                                    # the Pool queue is ~3us deep, so use DVE
                                    eng = nc.vector if l == HC - 1 else nc.gpsimd
                                    eng.tensor_copy(hi, t_[:])
                                    eng.tensor_tensor(
                                        apack[:, 0, l, q0:q0 + 512], t_[:], hi,
                                        op=ALU.subtract)
                                return emit
                            pending[0] = make_epilogue()

                        # all scores (and their masks) are emitted before any
                        # PV block: every engine queue sees the masks first
                        emit_scores(0)
                        emit_scores(1)
                        emit_scores(2)
                        emit_scores(3)
                        emit_pv(0)
                        emit_pv(1)
                        emit_pv(2)
                        emit_pv(3)
                    # W_o projection for batch b (fp8 DoubleRow main+corr);
                    # two 256-col chains per PSUM bank (bufs=3 keeps copies
                    # off the critical path). The next batch's first-head
                    # kq/vt loads are issued BEFORE the big output stores so
                    # they don't queue behind 8 MB in the DMA FIFO. The last
                    # pair epilogue (l=3, qb 2-3) flushes after the first
                    # m-block, which only reads early tokens.
                    if b + 1 < B:
                        kv_stash[(b + 1, 0)] = load_kv(b + 1, 0)
                    for m in range(8):
                        osb = opool.tile([128, 8, 512], BF16, tag="osb")
                        msl = slice(m * 128, (m + 1) * 128)
                        for pair in range(8):
                            pso = ps_o.tile([128, 512], F32, tag="o")
                            for part in range(2):
                                csl = slice((2 * pair + part) * 256,
                                            (2 * pair + part + 1) * 256)
                                po = pso[:, part * 256:(part + 1) * 256]
                                for c in range(2):
                                    nc.tensor.matmul(
                                        po, apack[:, 1, 2 * c:2 * c + 2, msl],
                                        wo_a[:, 2 * c:2 * c + 2, 0, csl],
                                        start=(c == 0), stop=False, perf_mode=DR)
                                for h in range(HC):
                                    nc.tensor.matmul(
                                        po, apack[:, :, h, msl],
                                        wo_a[:, h, :, csl],
                                        start=False, stop=(h == HC - 1),
                                        perf_mode=DR)
                            if pair % 2 == 0 or m >= 6:
                                # tail m-blocks go entirely to DVE (GpSimd has
                                # no PSUM access): keeps Act free for the next
                                # batch's first exps
                                nc.vector.tensor_copy(osb[:, pair, :], pso[:])
                            else:
                                nc.scalar.copy(osb[:, pair, :], pso[:])
                            if b == B - 1 and m == 7 and pair % 2 == 1:
                                # stream the final stores so the end-of-kernel
                                # drain tail is minimal
                                nc.sync.dma_start(
                                    out_p[bs + m * 128:bs + (m + 1) * 128,
                                          (pair - 1) * 512:(pair + 1) * 512],
                                    osb[:, pair - 1:pair + 1, :]
                                        .rearrange("p a c -> p (a c)"))
                        if b != B - 1 or m != 7:
                            nc.sync.dma_start(
                                out_p[bs + m * 128:bs + (m + 1) * 128, :],
                                osb[:].rearrange("p nc c -> p (nc c)"))
                        if m == 0 and pending[0] is not None:
                            pending[0]()
                            pending[0] = None
    nc.compile()
    return nc


def _q8hl(a, scale):
    """Quantize to fp8 e4m3 hi/lo pair at a shared scale."""
    import ml_dtypes
    hi = (a * scale).astype(ml_dtypes.float8_e4m3)
    lo = ((a * scale) - hi.astype(np.float32)).astype(ml_dtypes.float8_e4m3)
    return hi, lo


def _host_prep(hidden_states, W_pack, W_o, attention_mask, position_ids):
    import ml_dtypes
    x = np.asarray(hidden_states, dtype=np.float32).reshape(T, H)
    W_pack = np.asarray(W_pack, dtype=np.float32)
    W_o = np.asarray(W_o, dtype=np.float32)
    mask = np.asarray(attention_mask, dtype=np.float32)
    pos = np.asarray(position_ids)

    # causal structure is hardcoded in the kernel; verify it holds
    m0 = mask[0, 0]
    iu = np.triu_indices(S, 1)
    assert (m0[iu] < -1e8).all() and (np.tril(m0) == 0).all(), \
        "kernel requires the standard causal mask"

    # x pack: [128p, tb, kk, j(hi,lo), t] -> flat [128, NTB*KC*2*TB]
    xh, xl = _q8hl(x, S_X)
    xv_h = xh.reshape(NTB, TB, KC, 128).transpose(3, 0, 2, 1)
    xv_l = xl.reshape(NTB, TB, KC, 128).transpose(3, 0, 2, 1)
    xq_np = np.empty((128, NTB, KC, 2, TB), ml_dtypes.float8_e4m3)
    xq_np[:, :, :, 0, :] = xv_h
    xq_np[:, :, :, 1, :] = xv_l
    xq_np = np.ascontiguousarray(xq_np.reshape(128, -1))

    # rope tables with the fp8 descale folded in; rotate-half sign in sinS
    inv = 1.0 / (ROPE_BASE ** (np.arange(0, D, 2, dtype=np.float64) / D))
    inv = np.concatenate([inv, inv])
    ang = pos.astype(np.float64).reshape(T)[None, :] * inv[:, None]   # [D, T]
    cosT_np = np.ascontiguousarray((np.cos(ang) * DESCALE).astype(np.float32))
    sinT = (np.sin(ang) * DESCALE).astype(np.float32)
    sinS_np = sinT.copy()
    sinS_np[:64] = -sinT[:64]
    sinS_np = np.ascontiguousarray(sinS_np)

    # diagonal exp-mask triangle [128p(key), 128(query)], duplicated so one
    # strided DVE op covers both diagonal tiles' live quarters
    em = np.exp(m0)
    tri = em[0:128, 0:128].T.astype(ml_dtypes.bfloat16)   # [p(key), t(query)]
    maskD_np = np.ascontiguousarray(
        np.concatenate([tri, tri], axis=1))               # [128, 256]

    in_maps = []
    for core in range(N_CORES):
        h0 = core * HC
        # wqk cols ordered [q0,k0,q1,k1,q2,k2,q3,k3] per head slice
        cols = []
        for l in range(HC):
            cols.append(W_pack[:, (h0 + l) * D:(h0 + l + 1) * D])
            cols.append(W_pack[:, H + (h0 + l) * D:H + (h0 + l + 1) * D])
        wqk_f = np.concatenate(cols, axis=1)              # [H, 1024]
        wh, wl = _q8hl(wqk_f, S_W)
        wv_h = wh.reshape(KC, 128, 8, 128).transpose(1, 2, 0, 3)
        wv_l = wl.reshape(KC, 128, 8, 128).transpose(1, 2, 0, 3)
        wqk_np = np.empty((128, 8, KC, 2, 128), ml_dtypes.float8_e4m3)
        wqk_np[:, :, :, 0, :] = wv_l
        wqk_np[:, :, :, 1, :] = wv_h
        wqk_np = np.ascontiguousarray(wqk_np.reshape(128, -1))

        wv_f = np.concatenate(
            [W_pack[:, 2 * H + (h0 + l) * D:2 * H + (h0 + l + 1) * D]
             for l in range(HC)], axis=1)                 # [H, 512]
        wh, wl = _q8hl(wv_f, S_W)
        wvv_h = wh.reshape(KC, 128, 2, 256).transpose(1, 2, 0, 3)
        wvv_l = wl.reshape(KC, 128, 2, 256).transpose(1, 2, 0, 3)
        wv_np = np.empty((128, 2, KC, 2, 256), ml_dtypes.float8_e4m3)
        wv_np[:, :, :, 0, :] = wvv_l
        wv_np[:, :, :, 1, :] = wvv_h
        wv_np = np.ascontiguousarray(wv_np.reshape(128, -1))

        wo_f = W_o[h0 * D:(h0 + HC) * D, :]               # [512, H]
        wh, wl = _q8hl(wo_f, S_W)
        wov_h = wh.reshape(HC, 128, H).transpose(1, 0, 2)
        wov_l = wl.reshape(HC, 128, H).transpose(1, 0, 2)
        wo_np = np.empty((128, HC, 2, H), ml_dtypes.float8_e4m3)
        wo_np[:, :, 0, :] = wov_h
        wo_np[:, :, 1, :] = wov_l
        wo_np = np.ascontiguousarray(wo_np.reshape(128, -1))

        in_maps.append({
            "xq": xq_np, "wqk": wqk_np, "wv": wv_np, "wo": wo_np,
            "cosT": cosT_np, "sinS": sinS_np, "maskD": maskD_np,
        })
    return in_maps


def kernel(hidden_states, W_pack, W_o, attention_mask, position_ids):
    if "nc" not in _CACHE:
        _CACHE["nc"] = _build_module()
    nc = _CACHE["nc"]
    in_maps = _host_prep(hidden_states, W_pack, W_o, attention_mask, position_ids)
    res = bass_utils.run_bass_kernel_spmd(nc, in_maps, core_ids=list(range(N_CORES)))
    out = res.results[0]["out_p"].astype(np.float32)
    for c in range(1, N_CORES):
        out += res.results[c]["out_p"]
    out *= 1.0 / (S_A * S_W)
    return out.reshape(B, S, H).astype(np.float32)


# revision 82
# speedup vs baseline: 1.0275x; 1.0041x over previous
"""Trainium2 Bass kernel for nn_Attention_60567628808865.

Dense transformer attention block (B=4, S=1024, H=4096, NH=32, D=128):
  qkv = x @ W_pack; RoPE(q, k); causal-masked softmax attention; out @ W_o.

Sharding: tensor-parallel over heads across 8 NeuronCores. Each core computes
4 heads end-to-end; the host sums the 8 partial W_o outputs (row-sharded W_o).

Precision/performance scheme (validated on host to rel_err ~2.7e-3 vs the
2e-2 gate):
  - QKV and W_o projections run in fp8(e4m3) with the DoubleRow perf mode
    (K=256 per instruction, 0.5 cycles/row) using an exact-style two-term
    decomposition: x@W ~= x_hi@W_hi + [x_hi@W_lo + x_lo@W_hi], where
    t_hi = fp8(t*s), t_lo = fp8(t*s - t_hi). Both terms accumulate into ONE
    PSUM chain (identical scale), so the epilogue is unchanged. 48 DoubleRow
    instructions replace 32 f32r instructions per [128col x 256tok] unit:
    0.75x PE cycles.
  - hi/lo operands are slot-interleaved in a single packed tensor
    ([part, chunk, 2, free]) so the correction chain reads (hi,lo) slot pairs
    and the main chain reads (hi,hi) chunk pairs from the same SBUF bytes.
  - Attention is causal-aware: score/PV/denominator work is emitted only for
    the 20/32 key-tile x query-block units on or below the diagonal; the
    second diagonal tile of each query block runs at half moving-width (its
    lower query half is fully masked), and both diagonal triangles are masked
    in-place with one strided DVE multiply against a single host-built
    exp(mask) triangle (mask asserted causal). q/k/v round-trip DRAM in bf16;
    scores/PV matmuls run in bf16 (same PE rate as f32r, half the DMA).
  - Softmax is unnormalized; denominators come from a (1/s_a)-vector matmul
    accumulated in PSUM; the reciprocal is broadcast across partitions on
    GpSimd (partition_broadcast - no PE matmul, no PSUM bank), and the
    normalized attention is quantized to fp8 hi/lo pairs on the fly (hi/lo
    writes on GpSimd) for the W_o DoubleRow chain.
  - Output partials are stored bf16; the host sum applies the global descale.
  - Engine budget per head in attention: PE ~5.8us, Act (exp) ~5.3us,
    DVE (rope-free here: recip/attn-copy/t-mult/masks) ~4.7us, Pool
    (broadcast/hi/lo) ~4.8us.
"""
import numpy as np

import concourse.bass as bass  # noqa: F401
import concourse.tile as tile
from contextlib import ExitStack
from concourse import bacc, mybir
from concourse import bass_utils

F32 = mybir.dt.float32
F32R = mybir.dt.float32r
BF16 = mybir.dt.bfloat16
F8 = mybir.dt.float8e4
AF = mybir.ActivationFunctionType
ALU = mybir.AluOpType
DR = mybir.MatmulPerfMode.DoubleRow

B, S, H, NH = 4, 1024, 4096, 32
D = H // NH          # 128
T = B * S            # 4096 tokens
N_CORES = 8
HC = NH // N_CORES   # 4 heads per core
SCALE = float(1.0 / np.sqrt(D))
ROPE_BASE = 10000.0

TB = 256             # phase-1 token block
NTB = T // TB        # 16
KC = H // 128        # 32 fp8 k-chunks of 128 features
S_X = 32.0           # x quant scale
S_W = 2048.0         # W_pack / W_o quant scale
S_A = 32.0           # attention-output quant scale
DESCALE = 1.0 / (S_X * S_W)

_CACHE = {}


def _build_module(phases=("p1", "p2")):
    nc = bacc.Bacc("TRN2", target_bir_lowering=False, debug=False,
                   num_devices=N_CORES)

    # packed fp8 inputs (see _host_prep for layouts)
    xq = nc.dram_tensor("xq", [128, NTB * KC * 2 * TB], F8, kind="ExternalInput").ap()
    wqk = nc.dram_tensor("wqk", [128, 8 * KC * 2 * 128], F8, kind="ExternalInput").ap()
    wv = nc.dram_tensor("wv", [128, 2 * KC * 2 * 256], F8, kind="ExternalInput").ap()
    wo = nc.dram_tensor("wo", [128, HC * 2 * H], F8, kind="ExternalInput").ap()
    cosT = nc.dram_tensor("cosT", [128, T], F32, kind="ExternalInput").ap()
    sinS = nc.dram_tensor("sinS", [128, T], F32, kind="ExternalInput").ap()
    maskD = nc.dram_tensor("maskD", [128, 256], BF16, kind="ExternalInput").ap()
    out_p = nc.dram_tensor("out_p", [T, H], BF16, kind="ExternalOutput").ap()

    import ml_dtypes
    # denominator ones-vector carries 1/S_A so the reciprocal yields
    # S_A/denom directly (the fp8 attn quant scale)
    ones128 = nc.inline_tensor(
        np.full((128, 1), 1.0 / S_A, ml_dtypes.bfloat16), "ones128").ap()

    with tile.TileContext(nc) as tc, \
         nc.allow_low_precision(reason="fp8/bf16 matmuls; verified vs reference"):
        with ExitStack() as octx:
            dram = octx.enter_context(tc.tile_pool(name="dram", bufs=1, space="DRAM"))
            cpool = octx.enter_context(tc.tile_pool(name="consts", bufs=1))
            # DRAM scratch: qkT rows ordered [q0,k0,q1,k1,q2,k2,q3,k3] x d
            qkT_d = dram.tile([8 * 128, T], BF16)
            v_d = dram.tile([T, HC * 128], BF16)

            o128 = cpool.tile([128, 1], BF16)
            nc.sync.dma_start(o128[:], ones128[:])
            # [tri | tri]: the same lower-triangle pattern serves both
            # diagonal tiles (B's triangle is A's shifted by 128 both ways)
            mask_t = cpool.tile([128, 256], BF16)
            nc.sync.dma_start(mask_t[:], maskD[:])

            # phase-2 tiles prefetched during phase 1 (wo_a has no deps; the
            # first head's kq/vt depend on the tb0-3 scratch stores)
            wopool = octx.enter_context(tc.tile_pool(name="p2wo", bufs=1))
            kqpool = octx.enter_context(tc.tile_pool(name="p2kq", bufs=2))
            vtpool = octx.enter_context(tc.tile_pool(name="p2vt", bufs=2))
            _wo_a = [None]
            _first_kv = [None]

            def load_kv(b, l):
                bs = b * S
                kq = kqpool.tile([128, 2, S], BF16, tag="kq")
                nc.sync.dma_start(
                    kq[:],
                    qkT_d[l * 256:(l + 1) * 256, bs:bs + S]
                        .rearrange("(j p) t -> p j t", p=128))
                vt = vtpool.tile([128, 8, 128], BF16, tag="vt")
                nc.sync.dma_start(
                    vt[:],
                    v_d[bs:bs + S, l * 128:(l + 1) * 128]
                        .rearrange("(kt p) d -> p kt d", p=128))
                return kq, vt

            def prefetch_wo():
                # W_o resident: [128, h(4), j(2), c(4096)]; j=0 -> hi, 1 -> lo
                wo_a = wopool.tile([128, HC, 2, H], F8, tag="wo")
                for h in range(HC):
                    nc.sync.dma_start(
                        wo_a[:, h],
                        wo[:, h * 2 * H:(h + 1) * 2 * H]
                            .rearrange("p (j c) -> p j c", j=2))
                _wo_a[0] = wo_a

            def prefetch_kv():
                _first_kv[0] = load_kv(0, 0)

            # ---------------- Phase 1: QKV projection (fp8 DoubleRow) -------
            if "p1" in phases:
              with ExitStack() as ctx:
                wpool = ctx.enter_context(tc.tile_pool(name="p1w", bufs=1))
                xpool = ctx.enter_context(tc.tile_pool(name="p1x", bufs=2))
                opool = ctx.enter_context(tc.tile_pool(name="p1o", bufs=2))
                cspool = ctx.enter_context(tc.tile_pool(name="p1cs", bufs=2))
                rpool = ctx.enter_context(tc.tile_pool(name="p1rope", bufs=3))
                pqk = ctx.enter_context(tc.tile_pool(name="p1pqk", bufs=4, space="PSUM"))
                pv = ctx.enter_context(tc.tile_pool(name="p1pv", bufs=2, space="PSUM"))

                def load_tb(tb):
                    t0 = tb * TB
                    # x pack [128, kk(32), j(2), t(256)]; j=0 -> x_hi, j=1 -> x_lo
                    xall = xpool.tile([128, KC, 2, TB], F8, tag="x")
                    nc.sync.dma_start(
                        xall[:],
                        xq[:, tb * 16384:(tb + 1) * 16384]
                            .rearrange("p (kk j t) -> p kk j t", kk=KC, j=2))
                    cos_tb = cspool.tile([128, TB], F32, tag="cos")
                    nc.sync.dma_start(cos_tb[:], cosT[:, t0:t0 + TB])
                    sin_tb = cspool.tile([128, TB], F32, tag="sin")
                    nc.sync.dma_start(sin_tb[:], sinS[:, t0:t0 + TB])
                    return xall, cos_tb, sin_tb

                # tb0 inputs first (first chain needs x + wqk ct0 only), then
                # resident weights: wqk [128, ct(8), kk(32), j(2), c(128)],
                # wv [128, ct(2), kk(32), j(2), c(256)]; j=0 -> W_lo, j=1 -> W_hi
                tb0_inputs = load_tb(0)
                wqk_a = wpool.tile([128, 8, KC, 2, 128], F8, tag="wqk")
                wv_a = wpool.tile([128, 2, KC, 2, 256], F8, tag="wv")
                for ct in range(8):
                    nc.sync.dma_start(
                        wqk_a[:, ct],
                        wqk[:, ct * 8192:(ct + 1) * 8192]
                            .rearrange("p (kk j c) -> p kk j c", kk=KC, j=2))
                # tb1's x before wv: qk(1) needs it sooner than v(0) needs wv
                tb1_inputs = load_tb(1)
                for cv in range(2):
                    nc.sync.dma_start(
                        wv_a[:, cv],
                        wv[:, cv * 16384:(cv + 1) * 16384]
                            .rearrange("p (kk j c) -> p kk j c", kk=KC, j=2))

                def emit_qk(xall, cos_tb, sin_tb, t0):
                    qs_all = opool.tile([128, 8, TB], BF16, tag="qs")
                    for i in range(8):
                        ps = pqk.tile([128, TB], F32, tag="qk")
                        for c in range(16):
                            nc.tensor.matmul(
                                ps[:], wqk_a[:, i, 2 * c:2 * c + 2, 1, :],
                                xall[:, 2 * c:2 * c + 2, 0, :],
                                start=(c == 0), stop=False, perf_mode=DR)
                        for kk in range(KC):
                            nc.tensor.matmul(
                                ps[:], wqk_a[:, i, kk, :, :],
                                xall[:, kk, :, :],
                                start=False, stop=(kk == KC - 1), perf_mode=DR)
                        # RoPE epilogue (psum scale folded into cos/sin tables)
                        rot = rpool.tile([128, TB], F32, tag="rot")
                        nc.scalar.copy(rot[0:64, :], ps[64:128, :])
                        nc.vector.tensor_copy(rot[64:128, :], ps[0:64, :])
                        m1_ = rpool.tile([128, TB], F32, tag="m1")
                        nc.vector.tensor_tensor(m1_[:], ps[:], cos_tb[:], op=ALU.mult)
                        m2_ = rpool.tile([128, TB], F32, tag="m2")
                        nc.vector.tensor_tensor(m2_[:], rot[:], sin_tb[:], op=ALU.mult)
                        nc.vector.tensor_tensor(qs_all[:, i, :], m1_[:], m2_[:],
                                                op=ALU.add)
                    nc.sync.dma_start(
                        qkT_d[:, t0:t0 + TB].rearrange("(i p) t -> p i t", p=128),
                        qs_all[:])

                def emit_v(xall, t0):
                    vs_all = opool.tile([128, 2, 2, 256], BF16, tag="vs")
                    for th in range(2):
                        for ch in range(2):
                            ps = pv.tile([128, 256], F32, tag="v")
                            for c in range(16):
                                nc.tensor.matmul(
                                    ps[:],
                                    xall[:, 2 * c:2 * c + 2, 0,
                                         th * 128:(th + 1) * 128],
                                    wv_a[:, ch, 2 * c:2 * c + 2, 1, :],
                                    start=(c == 0), stop=False, perf_mode=DR)
                            for kk in range(KC):
                                nc.tensor.matmul(
                                    ps[:],
                                    xall[:, kk, :, th * 128:(th + 1) * 128],
                                    wv_a[:, ch, kk, :, :],
                                    start=False, stop=(kk == KC - 1), perf_mode=DR)
                            nc.scalar.activation(vs_all[:, th, ch, :], ps[:],
                                                 AF.Copy, scale=DESCALE)
                    nc.sync.dma_start(
                        v_d[t0:t0 + TB, :]
                            .rearrange("(th p) (ch c) -> p th ch c", p=128, ch=2),
                        vs_all[:])

                # v(0) is deferred until after qk(1): tb0's PE work then
                # needs only x+wqk, hiding the wv weight-load latency
                deferred_v0 = [None]
                for tb in range(NTB):
                    t0 = tb * TB
                    if tb == 0:
                        xall, cos_tb, sin_tb = tb0_inputs
                    elif tb == 1:
                        xall, cos_tb, sin_tb = tb1_inputs
                    else:
                        xall, cos_tb, sin_tb = load_tb(tb)
                    if tb == 2:
                        prefetch_wo()
                    elif tb == 4:
                        prefetch_kv()
                    emit_qk(xall, cos_tb, sin_tb, t0)
                    if tb == 0:
                        deferred_v0[0] = (xall, t0)
                    else:
                        if deferred_v0[0] is not None:
                            emit_v(*deferred_v0[0])
                            deferred_v0[0] = None
                        emit_v(xall, t0)

            # ---------------- Phase 2: attention + W_o ----------------------
            if "p2" in phases:
              with ExitStack() as ctx:
                apool = ctx.enter_context(tc.tile_pool(name="p2a", bufs=2))
                efpool = ctx.enter_context(tc.tile_pool(name="p2ef", bufs=14))
                tpool = ctx.enter_context(tc.tile_pool(name="p2t", bufs=3))
                rpool2 = ctx.enter_context(tc.tile_pool(name="p2rd", bufs=3))
                opool = ctx.enter_context(tc.tile_pool(name="p2o", bufs=2))
                ps_s = ctx.enter_context(tc.tile_pool(name="p2ps", bufs=3, space="PSUM"))
                ps_av = ctx.enter_context(tc.tile_pool(name="p2pav", bufs=1, space="PSUM"))
                ps_d = ctx.enter_context(tc.tile_pool(name="p2pd", bufs=1, space="PSUM"))
                ps_o = ctx.enter_context(tc.tile_pool(name="p2po", bufs=3, space="PSUM"))

                if _wo_a[0] is None:     # p2-only debug build
                    prefetch_wo()
                    prefetch_kv()
                wo_a = _wo_a[0]
                kv_stash = {}
                for b in range(B):
                    bs = b * S
                    # attn pack [128, lh(2), l(4), t(1024)]; lh=0 -> lo, 1 -> hi
                    apack = apool.tile([128, 2, HC, S], F8, tag="apack")
                    pending = [None]
                    for l in range(HC):
                        if b == 0 and l == 0:
                            kq, vt = _first_kv[0]
                        elif (b, l) in kv_stash:
                            kq, vt = kv_stash.pop((b, l))
                        else:
                            kq, vt = load_kv(b, l)

                        psd_l = ps_d.tile([1, 512], F32, tag="dbc")
                        psav_l = ps_av.tile([128, 512], F32, tag="av")
                        all_efs = {}

                        def emit_scores(qb):
                            u = 2 * qb + 2
                            q_sl = kq[:, 0, qb * 256:(qb + 1) * 256]
                            efs = [None] * u
                            # diagonal pair first: its exp+mask latency hides
                            # behind the remaining pairs' matmuls
                            for g in [qb] + list(range(qb)):
                                pss = ps_s.tile([128, 512], F32, tag="s")
                                if g == qb:
                                    # diagonal pair: tile A is full; tile B
                                    # only sees the top query half, packed
                                    # right after A so the exp is [128,384]
                                    nc.tensor.matmul(
                                        pss[:, 0:256],
                                        kq[:, 1, 2 * g * 128:(2 * g + 1) * 128],
                                        q_sl, start=True, stop=True)
                                    nc.tensor.matmul(
                                        pss[:, 256:384],
                                        kq[:, 1, (2 * g + 1) * 128:(2 * g + 2) * 128],
                                        q_sl[:, 128:256], start=True, stop=True)
                                    ef = efpool.tile([128, 384], BF16, tag="ef")
                                    nc.scalar.activation(ef[:], pss[:, 0:384],
                                                         AF.Exp, scale=SCALE)
                                    # in-place triangular mask on A's left
                                    # quarter and B's live quarter (same
                                    # pattern), one strided DVE op
                                    quarters = ef[:].rearrange(
                                        "p (g c) -> p g c", g=3)[:, ::2, :]
                                    nc.vector.tensor_tensor(
                                        quarters, quarters,
                                        mask_t[:].rearrange(
                                            "p (j c) -> p j c", j=2),
                                        op=ALU.mult)
                                    efs[2 * g] = ef[:, 0:256]
                                    efs[2 * g + 1] = ef[:, 256:384]
                                else:
                                    for sHalf in range(2):
                                        mt = 2 * g + sHalf
                                        nc.tensor.matmul(
                                            pss[:, sHalf * 256:(sHalf + 1) * 256],
                                            kq[:, 1, mt * 128:(mt + 1) * 128],
                                            q_sl, start=True, stop=True)
                                    ef = efpool.tile([128, 512], BF16, tag="ef")
                                    nc.scalar.activation(ef[:], pss[:], AF.Exp,
                                                         scale=SCALE)
                                    efs[2 * g] = ef[:, 0:256]
                                    efs[2 * g + 1] = ef[:, 256:512]
                            all_efs[qb] = efs

                        def emit_pv(qb):
                            u = 2 * qb + 2
                            efs = all_efs.pop(qb)
                            # masked diagonal units last in the chains
                            order = list(range(2 * qb)) + [2 * qb, 2 * qb + 1]
                            if pending[0] is not None:
                                pending[0]()
                                pending[0] = None
                            # the final (diagonal-B) unit only covers the top
                            # query half: half-width accumulation step
                            psav = psav_l[:, (qb % 2) * 256:(qb % 2 + 1) * 256]
                            for n, mt in enumerate(order):
                                half = mt == 2 * qb + 1
                                nc.tensor.matmul(
                                    psav[:, 128:256] if half else psav,
                                    vt[:, mt, :], efs[mt],
                                    start=(n == 0), stop=(n == u - 1),
                                    skip_group_check=True)
                            psd = psd_l[:, (qb % 2) * 256:(qb % 2 + 1) * 256]
                            for n, mt in enumerate(order):
                                half = mt == 2 * qb + 1
                                nc.tensor.matmul(
                                    psd[:, 128:256] if half else psd,
                                    o128[:], efs[mt],
                                    start=(n == 0), stop=(n == u - 1),
                                    skip_group_check=True)
                            if qb % 2 == 0:
                                return
                            # pair epilogue (qb-1, qb): unnormalized attn to
                            # SBUF (frees the psum bank), denominators to
                            # reciprocal; the normalization + fp8 hi/lo split
                            # is deferred into the next PV block
                            rd = rpool2.tile([1, 512], F32, tag="rd")
                            nc.vector.reciprocal(rd[:], psd_l[:])
                            av_s = tpool.tile([128, 512], F32, tag="avs")
                            nc.vector.tensor_copy(av_s[:], psav_l[:])

                            def make_epilogue(qb=qb, av_s=av_s, rd=rd, l=l,
                                              apack=apack):
                                def emit():
                                    # s_a/denom broadcast across partitions on
                                    # GpSimd: no PE matmul, no PSUM bank
                                    bc = tpool.tile([128, 512], F32, tag="bc")
                                    nc.gpsimd.partition_broadcast(bc[:], rd[:])
                                    t_ = tpool.tile([128, 512], F32, tag="t")
                                    nc.vector.tensor_tensor(t_[:], av_s[:],
                                                            bc[:], op=ALU.mult)
                                    q0 = (qb - 1) * 256
                                    hi = apack[:, 1, l, q0:q0 + 512]
                                    # last head: W_o waits on these writes and
                                    # the Pool queue is ~3us deep, so use DVE
                                    eng = nc.vector if l == HC - 1 else nc.gpsimd
                                    eng.tensor_copy(hi, t_[:])
                                    eng.tensor_tensor(
                                        apack[:, 0, l, q0:q0 + 512], t_[:], hi,
                                        op=ALU.subtract)
                                return emit
                            pending[0] = make_epilogue()

                        # all scores (and their masks) are emitted before any
                        # PV block: every engine queue sees the masks first
                        emit_scores(0)
                        emit_scores(1)
                        emit_scores(2)
                        emit_scores(3)
                        emit_pv(0)
                        emit_pv(1)
                        emit_pv(2)
                        emit_pv(3)
                    # W_o projection for batch b (fp8 DoubleRow main+corr);
                    # two 256-col chains per PSUM bank (bufs=3 keeps copies
                    # off the critical path). The next batch's first-head
                    # kq/vt loads are issued BEFORE the big output stores so
                    # they don't queue behind 8 MB in the DMA FIFO. The last
                    # pair epilogue (l=3, qb 2-3) flushes after the first
                    # m-block, which only reads early tokens.
                    if b + 1 < B:
                        kv_stash[(b + 1, 0)] = load_kv(b + 1, 0)
                    for m in range(8):
                        osb = opool.tile([128, 8, 512], BF16, tag="osb")
                        msl = slice(m * 128, (m + 1) * 128)
                        for pair in range(8):
                            pso = ps_o.tile([128, 512], F32, tag="o")
                            for part in range(2):
                                csl = slice((2 * pair + part) * 256,
                                            (2 * pair + part + 1) * 256)
                                po = pso[:, part * 256:(part + 1) * 256]
                                for c in range(2):
                                    nc.tensor.matmul(
                                        po, apack[:, 1, 2 * c:2 * c + 2, msl],
                                        wo_a[:, 2 * c:2 * c + 2, 0, csl],
                                        start=(c == 0), stop=False, perf_mode=DR)
                                for h in range(HC):
                                    nc.tensor.matmul(
                                        po, apack[:, :, h, msl],
                                        wo_a[:, h, :, csl],
                                        start=False, stop=(h == HC - 1),
                                        perf_mode=DR)
                            if pair % 2 == 0 or m >= 6:
                                # tail m-blocks go entirely to DVE (GpSimd has
                                # no PSUM access): keeps Act free for the next
                                # batch's first exps
                                nc.vector.tensor_copy(osb[:, pair, :], pso[:])
                            else:
                                nc.scalar.copy(osb[:, pair, :], pso[:])
                            if b == B - 1 and m == 7 and pair % 2 == 1:
                                # stream the final stores so the end-of-kernel
                                # drain tail is minimal
                                nc.sync.dma_start(
                                    out_p[bs + m * 128:bs + (m + 1) * 128,
                                          (pair - 1) * 512:(pair + 1) * 512],
                                    osb[:, pair - 1:pair + 1, :]
                                        .rearrange("p a c -> p (a c)"))
                        if b != B - 1 or m != 7:
                            nc.sync.dma_start(
                                out_p[bs + m * 128:bs + (m + 1) * 128, :],
                                osb[:].rearrange("p nc c -> p (nc c)"))
                        if m == 0 and pending[0] is not None:
                            pending[0]()
                            pending[0] = None
    nc.compile()
    return nc


def _q8hl(a, scale):
    """Quantize to fp8 e4m3 hi/lo pair at a shared scale."""
    import ml_dtypes
    hi = (a * scale).astype(ml_dtypes.float8_e4m3)
    lo = ((a * scale) - hi.astype(np.float32)).astype(ml_dtypes.float8_e4m3)
    return hi, lo


def _host_prep(hidden_states, W_pack, W_o, attention_mask, position_ids):
    import ml_dtypes
    x = np.asarray(hidden_states, dtype=np.float32).reshape(T, H)
    W_pack = np.asarray(W_pack, dtype=np.float32)
    W_o = np.asarray(W_o, dtype=np.float32)
    mask = np.asarray(attention_mask, dtype=np.float32)
    pos = np.asarray(position_ids)

    # causal structure is hardcoded in the kernel; verify it holds
    m0 = mask[0, 0]
    iu = np.triu_indices(S, 1)
    assert (m0[iu] < -1e8).all() and (np.tril(m0) == 0).all(), \
        "kernel requires the standard causal mask"

    # x pack: [128p, tb, kk, j(hi,lo), t] -> flat [128, NTB*KC*2*TB]
    xh, xl = _q8hl(x, S_X)
    xv_h = xh.reshape(NTB, TB, KC, 128).transpose(3, 0, 2, 1)
    xv_l = xl.reshape(NTB, TB, KC, 128).transpose(3, 0, 2, 1)
    xq_np = np.empty((128, NTB, KC, 2, TB), ml_dtypes.float8_e4m3)
    xq_np[:, :, :, 0, :] = xv_h
    xq_np[:, :, :, 1, :] = xv_l
    xq_np = np.ascontiguousarray(xq_np.reshape(128, -1))

    # rope tables with the fp8 descale folded in; rotate-half sign in sinS
    inv = 1.0 / (ROPE_BASE ** (np.arange(0, D, 2, dtype=np.float64) / D))
    inv = np.concatenate([inv, inv])
    ang = pos.astype(np.float64).reshape(T)[None, :] * inv[:, None]   # [D, T]
    cosT_np = np.ascontiguousarray((np.cos(ang) * DESCALE).astype(np.float32))
    sinT = (np.sin(ang) * DESCALE).astype(np.float32)
    sinS_np = sinT.copy()
    sinS_np[:64] = -sinT[:64]
    sinS_np = np.ascontiguousarray(sinS_np)

    # diagonal exp-mask triangle [128p(key), 128(query)], duplicated so one
    # strided DVE op covers both diagonal tiles' live quarters
    em = np.exp(m0)
    tri = em[0:128, 0:128].T.astype(ml_dtypes.bfloat16)   # [p(key), t(query)]
    maskD_np = np.ascontiguousarray(
        np.concatenate([tri, tri], axis=1))               # [128, 256]

    in_maps = []
    for core in range(N_CORES):
        h0 = core * HC
        # wqk cols ordered [q0,k0,q1,k1,q2,k2,q3,k3] per head slice
        cols = []
        for l in range(HC):
            cols.append(W_pack[:, (h0 + l) * D:(h0 + l + 1) * D])
            cols.append(W_pack[:, H + (h0 + l) * D:H + (h0 + l + 1) * D])
        wqk_f = np.concatenate(cols, axis=1)              # [H, 1024]
        wh, wl = _q8hl(wqk_f, S_W)
        wv_h = wh.reshape(KC, 128, 8, 128).transpose(1, 2, 0, 3)
        wv_l = wl.reshape(KC, 128, 8, 128).transpose(1, 2, 0, 3)
        wqk_np = np.empty((128, 8, KC, 2, 128), ml_dtypes.float8_e4m3)
        wqk_np[:, :, :, 0, :] = wv_l
        wqk_np[:, :, :, 1, :] = wv_h
        wqk_np = np.ascontiguousarray(wqk_np.reshape(128, -1))

        wv_f = np.concatenate(
            [W_pack[:, 2 * H + (h0 + l) * D:2 * H + (h0 + l + 1) * D]
             for l in range(HC)], axis=1)                 # [H, 512]
        wh, wl = _q8hl(wv_f, S_W)
        wvv_h = wh.reshape(KC, 128, 2, 256).transpose(1, 2, 0, 3)
        wvv_l = wl.reshape(KC, 128, 2, 256).transpose(1, 2, 0, 3)
        wv_np = np.empty((128, 2, KC, 2, 256), ml_dtypes.float8_e4m3)
        wv_np[:, :, :, 0, :] = wvv_l
        wv_np[:, :, :, 1, :] = wvv_h
        wv_np = np.ascontiguousarray(wv_np.reshape(128, -1))

        wo_f = W_o[h0 * D:(h0 + HC) * D, :]               # [512, H]
        wh, wl = _q8hl(wo_f, S_W)
        wov_h = wh.reshape(HC, 128, H).transpose(1, 0, 2)
        wov_l = wl.reshape(HC, 128, H).transpose(1, 0, 2)
        wo_np = np.empty((128, HC, 2, H), ml_dtypes.float8_e4m3)
        wo_np[:, :, 0, :] = wov_h
        wo_np[:, :, 1, :] = wov_l
        wo_np = np.ascontiguousarray(wo_np.reshape(128, -1))

        in_maps.append({
            "xq": xq_np, "wqk": wqk_np, "wv": wv_np, "wo": wo_np,
            "cosT": cosT_np, "sinS": sinS_np, "maskD": maskD_np,
        })
    return in_maps


def kernel(hidden_states, W_pack, W_o, attention_mask, position_ids):
    if "nc" not in _CACHE:
        _CACHE["nc"] = _build_module()
    nc = _CACHE["nc"]
    in_maps = _host_prep(hidden_states, W_pack, W_o, attention_mask, position_ids)
    res = bass_utils.run_bass_kernel_spmd(nc, in_maps, core_ids=list(range(N_CORES)))
    out = res.results[0]["out_p"].astype(np.float32)
    for c in range(1, N_CORES):
        out += res.results[c]["out_p"]
    out *= 1.0 / (S_A * S_W)
    return out.reshape(B, S, H).astype(np.float32)


# revision 84
# speedup vs baseline: 1.0295x; 1.0019x over previous
"""Trainium2 Bass kernel for nn_Attention_60567628808865.

Dense transformer attention block (B=4, S=1024, H=4096, NH=32, D=128):
  qkv = x @ W_pack; RoPE(q, k); causal-masked softmax attention; out @ W_o.

Sharding: tensor-parallel over heads across 8 NeuronCores. Each core computes
4 heads end-to-end; the host sums the 8 partial W_o outputs (row-sharded W_o).

Precision/performance scheme (validated on host to rel_err ~2.7e-3 vs the
2e-2 gate):
  - QKV and W_o projections run in fp8(e4m3) with the DoubleRow perf mode
    (K=256 per instruction, 0.5 cycles/row) using an exact-style two-term
    decomposition: x@W ~= x_hi@W_hi + [x_hi@W_lo + x_lo@W_hi], where
    t_hi = fp8(t*s), t_lo = fp8(t*s - t_hi). Both terms accumulate into ONE
    PSUM chain (identical scale), so the epilogue is unchanged. 48 DoubleRow
    instructions replace 32 f32r instructions per [128col x 256tok] unit:
    0.75x PE cycles.
  - hi/lo operands are slot-interleaved in a single packed tensor
    ([part, chunk, 2, free]) so the correction chain reads (hi,lo) slot pairs
    and the main chain reads (hi,hi) chunk pairs from the same SBUF bytes.
  - Attention is causal-aware: score/PV/denominator work is emitted only for
    the 20/32 key-tile x query-block units on or below the diagonal; the
    second diagonal tile of each query block runs at half moving-width (its
    lower query half is fully masked), and both diagonal triangles are masked
    in-place with one strided DVE multiply against a single host-built
    exp(mask) triangle (mask asserted causal). q/k/v round-trip DRAM in bf16;
    scores/PV matmuls run in bf16 (same PE rate as f32r, half the DMA).
  - Softmax is unnormalized; denominators come from a (1/s_a)-vector matmul
    accumulated in PSUM; the reciprocal is broadcast across partitions on
    GpSimd (partition_broadcast - no PE matmul, no PSUM bank), and the
    normalized attention is quantized to fp8 hi/lo pairs on the fly (hi/lo
    writes on GpSimd) for the W_o DoubleRow chain.
  - Output partials are stored bf16; the host sum applies the global descale.
  - Engine budget per head in attention: PE ~5.8us, Act (exp) ~5.3us,
    DVE (rope-free here: recip/attn-copy/t-mult/masks) ~4.7us, Pool
    (broadcast/hi/lo) ~4.8us.
"""
import numpy as np

import concourse.bass as bass  # noqa: F401
import concourse.tile as tile
from contextlib import ExitStack
from concourse import bacc, mybir
from concourse import bass_utils

F32 = mybir.dt.float32
F32R = mybir.dt.float32r
BF16 = mybir.dt.bfloat16
F8 = mybir.dt.float8e4
AF = mybir.ActivationFunctionType
ALU = mybir.AluOpType
DR = mybir.MatmulPerfMode.DoubleRow

B, S, H, NH = 4, 1024, 4096, 32
D = H // NH          # 128
T = B * S            # 4096 tokens
N_CORES = 8
HC = NH // N_CORES   # 4 heads per core
SCALE = float(1.0 / np.sqrt(D))
ROPE_BASE = 10000.0

TB = 256             # phase-1 token block
NTB = T // TB        # 16
KC = H // 128        # 32 fp8 k-chunks of 128 features
S_X = 32.0           # x quant scale
S_W = 2048.0         # W_pack / W_o quant scale
S_A = 32.0           # attention-output quant scale
DESCALE = 1.0 / (S_X * S_W)

_CACHE = {}


def _build_module(phases=("p1", "p2")):
    nc = bacc.Bacc("TRN2", target_bir_lowering=False, debug=False,
                   num_devices=N_CORES)

    # packed fp8 inputs (see _host_prep for layouts)
    xq = nc.dram_tensor("xq", [128, NTB * KC * 2 * TB], F8, kind="ExternalInput").ap()
    wqk = nc.dram_tensor("wqk", [128, 8 * KC * 2 * 128], F8, kind="ExternalInput").ap()
    wv = nc.dram_tensor("wv", [128, 2 * KC * 2 * 256], F8, kind="ExternalInput").ap()
    wo = nc.dram_tensor("wo", [128, HC * 2 * H], F8, kind="ExternalInput").ap()
    cosT = nc.dram_tensor("cosT", [128, T], F32, kind="ExternalInput").ap()
    sinS = nc.dram_tensor("sinS", [128, T], F32, kind="ExternalInput").ap()
    maskD = nc.dram_tensor("maskD", [128, 256], BF16, kind="ExternalInput").ap()
    out_p = nc.dram_tensor("out_p", [T, H], BF16, kind="ExternalOutput").ap()

    import ml_dtypes
    # denominator ones-vector carries 1/S_A so the reciprocal yields
    # S_A/denom directly (the fp8 attn quant scale)
    ones128 = nc.inline_tensor(
        np.full((128, 1), 1.0 / S_A, ml_dtypes.bfloat16), "ones128").ap()

    with tile.TileContext(nc) as tc, \
         nc.allow_low_precision(reason="fp8/bf16 matmuls; verified vs reference"):
        with ExitStack() as octx:
            dram = octx.enter_context(tc.tile_pool(name="dram", bufs=1, space="DRAM"))
            cpool = octx.enter_context(tc.tile_pool(name="consts", bufs=1))
            # DRAM scratch: qkT rows ordered [q0,k0,q1,k1,q2,k2,q3,k3] x d
            qkT_d = dram.tile([8 * 128, T], BF16)
            v_d = dram.tile([T, HC * 128], BF16)

            # consts are tiny but each DMA costs ~625ns of FIFO-head issue
            # time: defer them behind the critical startup loads
            o128 = cpool.tile([128, 1], BF16)
            mask_t = cpool.tile([128, 256], BF16)
            _consts = [False]

            def load_consts():
                if not _consts[0]:
                    nc.sync.dma_start(o128[:], ones128[:])
                    nc.sync.dma_start(mask_t[:], maskD[:])
                    _consts[0] = True

            # phase-2 tiles prefetched during phase 1 (wo_a has no deps; the
            # first head's kq/vt depend on the tb0-3 scratch stores)
            wopool = octx.enter_context(tc.tile_pool(name="p2wo", bufs=1))
            kqpool = octx.enter_context(tc.tile_pool(name="p2kq", bufs=2))
            vtpool = octx.enter_context(tc.tile_pool(name="p2vt", bufs=2))
            _wo_a = [None]
            _first_kv = [None]

            def load_kv(b, l):
                bs = b * S
                kq = kqpool.tile([128, 2, S], BF16, tag="kq")
                nc.sync.dma_start(
                    kq[:],
                    qkT_d[l * 256:(l + 1) * 256, bs:bs + S]
                        .rearrange("(j p) t -> p j t", p=128))
                vt = vtpool.tile([128, 8, 128], BF16, tag="vt")
                nc.sync.dma_start(
                    vt[:],
                    v_d[bs:bs + S, l * 128:(l + 1) * 128]
                        .rearrange("(kt p) d -> p kt d", p=128))
                return kq, vt

            def prefetch_wo():
                # W_o resident: [128, h(4), j(2), c(4096)]; j=0 -> hi, 1 -> lo
                wo_a = wopool.tile([128, HC, 2, H], F8, tag="wo")
                for h in range(HC):
                    nc.sync.dma_start(
                        wo_a[:, h],
                        wo[:, h * 2 * H:(h + 1) * 2 * H]
                            .rearrange("p (j c) -> p j c", j=2))
                _wo_a[0] = wo_a

            def prefetch_kv():
                _first_kv[0] = load_kv(0, 0)

            # ---------------- Phase 1: QKV projection (fp8 DoubleRow) -------
            if "p1" in phases:
              with ExitStack() as ctx:
                wpool = ctx.enter_context(tc.tile_pool(name="p1w", bufs=1))
                xpool = ctx.enter_context(tc.tile_pool(name="p1x", bufs=2))
                opool = ctx.enter_context(tc.tile_pool(name="p1o", bufs=2))
                cspool = ctx.enter_context(tc.tile_pool(name="p1cs", bufs=2))
                rpool = ctx.enter_context(tc.tile_pool(name="p1rope", bufs=3))
                pqk = ctx.enter_context(tc.tile_pool(name="p1pqk", bufs=4, space="PSUM"))
                pv = ctx.enter_context(tc.tile_pool(name="p1pv", bufs=2, space="PSUM"))

                def load_tb(tb):
                    t0 = tb * TB
                    # x pack [128, kk(32), j(2), t(256)]; j=0 -> x_hi, j=1 -> x_lo
                    xall = xpool.tile([128, KC, 2, TB], F8, tag="x")
                    nc.sync.dma_start(
                        xall[:],
                        xq[:, tb * 16384:(tb + 1) * 16384]
                            .rearrange("p (kk j t) -> p kk j t", kk=KC, j=2))
                    cos_tb = cspool.tile([128, TB], F32, tag="cos")
                    nc.sync.dma_start(cos_tb[:], cosT[:, t0:t0 + TB])
                    sin_tb = cspool.tile([128, TB], F32, tag="sin")
                    nc.sync.dma_start(sin_tb[:], sinS[:, t0:t0 + TB])
                    return xall, cos_tb, sin_tb

                # tb0 inputs first (first chain needs x + wqk ct0 only), then
                # resident weights: wqk [128, ct(8), kk(32), j(2), c(128)],
                # wv [128, ct(2), kk(32), j(2), c(256)]; j=0 -> W_lo, j=1 -> W_hi
                tb0_inputs = load_tb(0)
                wqk_a = wpool.tile([128, 8, KC, 2, 128], F8, tag="wqk")
                wv_a = wpool.tile([128, 2, KC, 2, 256], F8, tag="wv")
                for ct in range(8):
                    nc.sync.dma_start(
                        wqk_a[:, ct],
                        wqk[:, ct * 8192:(ct + 1) * 8192]
                            .rearrange("p (kk j c) -> p kk j c", kk=KC, j=2))
                # tb1's x before wv: qk(1) needs it sooner than v(0) needs wv
                tb1_inputs = load_tb(1)
                for cv in range(2):
                    nc.sync.dma_start(
                        wv_a[:, cv],
                        wv[:, cv * 16384:(cv + 1) * 16384]
                            .rearrange("p (kk j c) -> p kk j c", kk=KC, j=2))
                load_consts()

                def emit_qk(xall, cos_tb, sin_tb, t0):
                    qs_all = opool.tile([128, 8, TB], BF16, tag="qs")
                    for i in range(8):
                        ps = pqk.tile([128, TB], F32, tag="qk")
                        for c in range(16):
                            nc.tensor.matmul(
                                ps[:], wqk_a[:, i, 2 * c:2 * c + 2, 1, :],
                                xall[:, 2 * c:2 * c + 2, 0, :],
                                start=(c == 0), stop=False, perf_mode=DR)
                        for kk in range(KC):
                            nc.tensor.matmul(
                                ps[:], wqk_a[:, i, kk, :, :],
                                xall[:, kk, :, :],
                                start=False, stop=(kk == KC - 1), perf_mode=DR)
                        # RoPE epilogue (psum scale folded into cos/sin tables)
                        rot = rpool.tile([128, TB], F32, tag="rot")
                        nc.scalar.copy(rot[0:64, :], ps[64:128, :])
                        nc.vector.tensor_copy(rot[64:128, :], ps[0:64, :])
                        m1_ = rpool.tile([128, TB], F32, tag="m1")
                        nc.vector.tensor_tensor(m1_[:], ps[:], cos_tb[:], op=ALU.mult)
                        m2_ = rpool.tile([128, TB], F32, tag="m2")
                        nc.vector.tensor_tensor(m2_[:], rot[:], sin_tb[:], op=ALU.mult)
                        nc.vector.tensor_tensor(qs_all[:, i, :], m1_[:], m2_[:],
                                                op=ALU.add)
                    nc.sync.dma_start(
                        qkT_d[:, t0:t0 + TB].rearrange("(i p) t -> p i t", p=128),
                        qs_all[:])

                def emit_v(xall, t0):
                    vs_all = opool.tile([128, 2, 2, 256], BF16, tag="vs")
                    for th in range(2):
                        for ch in range(2):
                            ps = pv.tile([128, 256], F32, tag="v")
                            for c in range(16):
                                nc.tensor.matmul(
                                    ps[:],
                                    xall[:, 2 * c:2 * c + 2, 0,
                                         th * 128:(th + 1) * 128],
                                    wv_a[:, ch, 2 * c:2 * c + 2, 1, :],
                                    start=(c == 0), stop=False, perf_mode=DR)
                            for kk in range(KC):
                                nc.tensor.matmul(
                                    ps[:],
                                    xall[:, kk, :, th * 128:(th + 1) * 128],
                                    wv_a[:, ch, kk, :, :],
                                    start=False, stop=(kk == KC - 1), perf_mode=DR)
                            nc.scalar.activation(vs_all[:, th, ch, :], ps[:],
                                                 AF.Copy, scale=DESCALE)
                    nc.sync.dma_start(
                        v_d[t0:t0 + TB, :]
                            .rearrange("(th p) (ch c) -> p th ch c", p=128, ch=2),
                        vs_all[:])

                # v(0) is deferred until after qk(1): tb0's PE work then
                # needs only x+wqk, hiding the wv weight-load latency
                deferred_v0 = [None]
                for tb in range(NTB):
                    t0 = tb * TB
                    if tb == 0:
                        xall, cos_tb, sin_tb = tb0_inputs
                    elif tb == 1:
                        xall, cos_tb, sin_tb = tb1_inputs
                    else:
                        xall, cos_tb, sin_tb = load_tb(tb)
                    if tb == 2:
                        prefetch_wo()
                    elif tb == 4:
                        prefetch_kv()
                    emit_qk(xall, cos_tb, sin_tb, t0)
                    if tb == 0:
                        deferred_v0[0] = (xall, t0)
                    else:
                        if deferred_v0[0] is not None:
                            emit_v(*deferred_v0[0])
                            deferred_v0[0] = None
                        emit_v(xall, t0)

            # ---------------- Phase 2: attention + W_o ----------------------
            if "p2" in phases:
              with ExitStack() as ctx:
                apool = ctx.enter_context(tc.tile_pool(name="p2a", bufs=2))
                efpool = ctx.enter_context(tc.tile_pool(name="p2ef", bufs=14))
                tpool = ctx.enter_context(tc.tile_pool(name="p2t", bufs=3))
                rpool2 = ctx.enter_context(tc.tile_pool(name="p2rd", bufs=3))
                opool = ctx.enter_context(tc.tile_pool(name="p2o", bufs=2))
                ps_s = ctx.enter_context(tc.tile_pool(name="p2ps", bufs=3, space="PSUM"))
                ps_av = ctx.enter_context(tc.tile_pool(name="p2pav", bufs=1, space="PSUM"))
                ps_d = ctx.enter_context(tc.tile_pool(name="p2pd", bufs=1, space="PSUM"))
                ps_o = ctx.enter_context(tc.tile_pool(name="p2po", bufs=3, space="PSUM"))

                if _wo_a[0] is None:     # p2-only debug build
                    prefetch_wo()
                    prefetch_kv()
                load_consts()
                wo_a = _wo_a[0]
                kv_stash = {}
                for b in range(B):
                    bs = b * S
                    # attn pack [128, lh(2), l(4), t(1024)]; lh=0 -> lo, 1 -> hi
                    apack = apool.tile([128, 2, HC, S], F8, tag="apack")
                    pending = [None]
                    for l in range(HC):
                        if b == 0 and l == 0:
                            kq, vt = _first_kv[0]
                        elif (b, l) in kv_stash:
                            kq, vt = kv_stash.pop((b, l))
                        else:
                            kq, vt = load_kv(b, l)

                        psd_l = ps_d.tile([1, 512], F32, tag="dbc")
                        psav_l = ps_av.tile([128, 512], F32, tag="av")
                        all_efs = {}

                        def emit_scores(qb):
                            u = 2 * qb + 2
                            q_sl = kq[:, 0, qb * 256:(qb + 1) * 256]
                            efs = [None] * u
                            # diagonal pair first: its exp+mask latency hides
                            # behind the remaining pairs' matmuls
                            for g in [qb] + list(range(qb)):
                                pss = ps_s.tile([128, 512], F32, tag="s")
                                if g == qb:
                                    # diagonal pair: tile A is full; tile B
                                    # only sees the top query half, packed
                                    # right after A so the exp is [128,384]
                                    nc.tensor.matmul(
                                        pss[:, 0:256],
                                        kq[:, 1, 2 * g * 128:(2 * g + 1) * 128],
                                        q_sl, start=True, stop=True)
                                    nc.tensor.matmul(
                                        pss[:, 256:384],
                                        kq[:, 1, (2 * g + 1) * 128:(2 * g + 2) * 128],
                                        q_sl[:, 128:256], start=True, stop=True)
                                    ef = efpool.tile([128, 384], BF16, tag="ef")
                                    nc.scalar.activation(ef[:], pss[:, 0:384],
                                                         AF.Exp, scale=SCALE)
                                    # in-place triangular mask on A's left
                                    # quarter and B's live quarter (same
                                    # pattern), one strided DVE op
                                    quarters = ef[:].rearrange(
                                        "p (g c) -> p g c", g=3)[:, ::2, :]
                                    nc.vector.tensor_tensor(
                                        quarters, quarters,
                                        mask_t[:].rearrange(
                                            "p (j c) -> p j c", j=2),
                                        op=ALU.mult)
                                    efs[2 * g] = ef[:, 0:256]
                                    efs[2 * g + 1] = ef[:, 256:384]
                                else:
                                    for sHalf in range(2):
                                        mt = 2 * g + sHalf
                                        nc.tensor.matmul(
                                            pss[:, sHalf * 256:(sHalf + 1) * 256],
                                            kq[:, 1, mt * 128:(mt + 1) * 128],
                                            q_sl, start=True, stop=True)
                                    ef = efpool.tile([128, 512], BF16, tag="ef")
                                    nc.scalar.activation(ef[:], pss[:], AF.Exp,
                                                         scale=SCALE)
                                    efs[2 * g] = ef[:, 0:256]
                                    efs[2 * g + 1] = ef[:, 256:512]
                            all_efs[qb] = efs

                        def emit_pv(qb):
                            u = 2 * qb + 2
                            efs = all_efs.pop(qb)
                            # masked diagonal units last in the chains
                            order = list(range(2 * qb)) + [2 * qb, 2 * qb + 1]
                            if pending[0] is not None:
                                pending[0]()
                                pending[0] = None
                            # the final (diagonal-B) unit only covers the top
                            # query half: half-width accumulation step
                            psav = psav_l[:, (qb % 2) * 256:(qb % 2 + 1) * 256]
                            for n, mt in enumerate(order):
                                half = mt == 2 * qb + 1
                                nc.tensor.matmul(
                                    psav[:, 128:256] if half else psav,
                                    vt[:, mt, :], efs[mt],
                                    start=(n == 0), stop=(n == u - 1),
                                    skip_group_check=True)
                            psd = psd_l[:, (qb % 2) * 256:(qb % 2 + 1) * 256]
                            for n, mt in enumerate(order):
                                half = mt == 2 * qb + 1
                                nc.tensor.matmul(
                                    psd[:, 128:256] if half else psd,
                                    o128[:], efs[mt],
                                    start=(n == 0), stop=(n == u - 1),
                                    skip_group_check=True)
                            if qb % 2 == 0:
                                return
                            # pair epilogue (qb-1, qb): unnormalized attn to
                            # SBUF (frees the psum bank), denominators to
                            # reciprocal; the normalization + fp8 hi/lo split
                            # is deferred into the next PV block
                            rd = rpool2.tile([1, 512], F32, tag="rd")
                            nc.vector.reciprocal(rd[:], psd_l[:])
                            av_s = tpool.tile([128, 512], F32, tag="avs")
                            nc.vector.tensor_copy(av_s[:], psav_l[:])

                            def make_epilogue(qb=qb, av_s=av_s, rd=rd, l=l,
                                              apack=apack):
                                def emit():
                                    # s_a/denom broadcast across partitions on
                                    # GpSimd: no PE matmul, no PSUM bank
                                    bc = tpool.tile([128, 512], F32, tag="bc")
                                    nc.gpsimd.partition_broadcast(bc[:], rd[:])
                                    t_ = tpool.tile([128, 512], F32, tag="t")
                                    nc.vector.tensor_tensor(t_[:], av_s[:],
                                                            bc[:], op=ALU.mult)
                                    q0 = (qb - 1) * 256
                                    hi = apack[:, 1, l, q0:q0 + 512]
                                    # last head: W_o waits on these writes and
                                    # the Pool queue is ~3us deep, so use DVE
                                    eng = nc.vector if l == HC - 1 else nc.gpsimd
                                    eng.tensor_copy(hi, t_[:])
                                    eng.tensor_tensor(
                                        apack[:, 0, l, q0:q0 + 512], t_[:], hi,
                                        op=ALU.subtract)
                                return emit
                            pending[0] = make_epilogue()

                        # all scores (and their masks) are emitted before any
                        # PV block: every engine queue sees the masks first
                        emit_scores(0)
                        emit_scores(1)
                        emit_scores(2)
                        emit_scores(3)
                        emit_pv(0)
                        emit_pv(1)
                        emit_pv(2)
                        emit_pv(3)
                    # W_o projection for batch b (fp8 DoubleRow main+corr);
                    # two 256-col chains per PSUM bank (bufs=3 keeps copies
                    # off the critical path). The next batch's first-head
                    # kq/vt loads are issued BEFORE the big output stores so
                    # they don't queue behind 8 MB in the DMA FIFO. The last
                    # pair epilogue (l=3, qb 2-3) flushes after the first
                    # m-block, which only reads early tokens.
                    if b + 1 < B:
                        kv_stash[(b + 1, 0)] = load_kv(b + 1, 0)
                    for m in range(8):
                        osb = opool.tile([128, 8, 512], BF16, tag="osb")
                        msl = slice(m * 128, (m + 1) * 128)
                        for pair in range(8):
                            pso = ps_o.tile([128, 512], F32, tag="o")
                            for part in range(2):
                                csl = slice((2 * pair + part) * 256,
                                            (2 * pair + part + 1) * 256)
                                po = pso[:, part * 256:(part + 1) * 256]
                                for c in range(2):
                                    nc.tensor.matmul(
                                        po, apack[:, 1, 2 * c:2 * c + 2, msl],
                                        wo_a[:, 2 * c:2 * c + 2, 0, csl],
                                        start=(c == 0), stop=False, perf_mode=DR)
                                for h in range(HC):
                                    nc.tensor.matmul(
                                        po, apack[:, :, h, msl],
                                        wo_a[:, h, :, csl],
                                        start=False, stop=(h == HC - 1),
                                        perf_mode=DR)
                            if pair % 2 == 0 or m >= 6:
                                # tail m-blocks go entirely to DVE (GpSimd has
                                # no PSUM access): keeps Act free for the next
                                # batch's first exps
                                nc.vector.tensor_copy(osb[:, pair, :], pso[:])
                            else:
                                nc.scalar.copy(osb[:, pair, :], pso[:])
                            if b == B - 1 and m == 7:
                                # stream the final stores per pair so the
                                # end-of-kernel drain tail is minimal
                                nc.sync.dma_start(
                                    out_p[bs + m * 128:bs + (m + 1) * 128,
                                          pair * 512:(pair + 1) * 512],
                                    osb[:, pair, :])
                        if b != B - 1 or m != 7:
                            nc.sync.dma_start(
                                out_p[bs + m * 128:bs + (m + 1) * 128, :],
                                osb[:].rearrange("p nc c -> p (nc c)"))
                        if m == 0 and pending[0] is not None:
                            pending[0]()
                            pending[0] = None
    nc.compile()
    return nc


def _q8hl(a, scale):
    """Quantize to fp8 e4m3 hi/lo pair at a shared scale."""
    import ml_dtypes
    hi = (a * scale).astype(ml_dtypes.float8_e4m3)
    lo = ((a * scale) - hi.astype(np.float32)).astype(ml_dtypes.float8_e4m3)
    return hi, lo


def _host_prep(hidden_states, W_pack, W_o, attention_mask, position_ids):
    import ml_dtypes
    x = np.asarray(hidden_states, dtype=np.float32).reshape(T, H)
    W_pack = np.asarray(W_pack, dtype=np.float32)
    W_o = np.asarray(W_o, dtype=np.float32)
    mask = np.asarray(attention_mask, dtype=np.float32)
    pos = np.asarray(position_ids)

    # causal structure is hardcoded in the kernel; verify it holds
    m0 = mask[0, 0]
    iu = np.triu_indices(S, 1)
    assert (m0[iu] < -1e8).all() and (np.tril(m0) == 0).all(), \
        "kernel requires the standard causal mask"

    # x pack: [128p, tb, kk, j(hi,lo), t] -> flat [128, NTB*KC*2*TB]
    xh, xl = _q8hl(x, S_X)
    xv_h = xh.reshape(NTB, TB, KC, 128).transpose(3, 0, 2, 1)
    xv_l = xl.reshape(NTB, TB, KC, 128).transpose(3, 0, 2, 1)
    xq_np = np.empty((128, NTB, KC, 2, TB), ml_dtypes.float8_e4m3)
    xq_np[:, :, :, 0, :] = xv_h
    xq_np[:, :, :, 1, :] = xv_l
    xq_np = np.ascontiguousarray(xq_np.reshape(128, -1))

    # rope tables with the fp8 descale folded in; rotate-half sign in sinS
    inv = 1.0 / (ROPE_BASE ** (np.arange(0, D, 2, dtype=np.float64) / D))
    inv = np.concatenate([inv, inv])
    ang = pos.astype(np.float64).reshape(T)[None, :] * inv[:, None]   # [D, T]
    cosT_np = np.ascontiguousarray((np.cos(ang) * DESCALE).astype(np.float32))
    sinT = (np.sin(ang) * DESCALE).astype(np.float32)
    sinS_np = sinT.copy()
    sinS_np[:64] = -sinT[:64]
    sinS_np = np.ascontiguousarray(sinS_np)

    # diagonal exp-mask triangle [128p(key), 128(query)], duplicated so one
    # strided DVE op covers both diagonal tiles' live quarters
    em = np.exp(m0)
    tri = em[0:128, 0:128].T.astype(ml_dtypes.bfloat16)   # [p(key), t(query)]
    maskD_np = np.ascontiguousarray(
        np.concatenate([tri, tri], axis=1))               # [128, 256]

    in_maps = []
    for core in range(N_CORES):
        h0 = core * HC
        # wqk cols ordered [q0,k0,q1,k1,q2,k2,q3,k3] per head slice
        cols = []
        for l in range(HC):
            cols.append(W_pack[:, (h0 + l) * D:(h0 + l + 1) * D])
            cols.append(W_pack[:, H + (h0 + l) * D:H + (h0 + l + 1) * D])
        wqk_f = np.concatenate(cols, axis=1)              # [H, 1024]
        wh, wl = _q8hl(wqk_f, S_W)
        wv_h = wh.reshape(KC, 128, 8, 128).transpose(1, 2, 0, 3)
        wv_l = wl.reshape(KC, 128, 8, 128).transpose(1, 2, 0, 3)
        wqk_np = np.empty((128, 8, KC, 2, 128), ml_dtypes.float8_e4m3)
        wqk_np[:, :, :, 0, :] = wv_l
        wqk_np[:, :, :, 1, :] = wv_h
        wqk_np = np.ascontiguousarray(wqk_np.reshape(128, -1))

        wv_f = np.concatenate(
            [W_pack[:, 2 * H + (h0 + l) * D:2 * H + (h0 + l + 1) * D]
             for l in range(HC)], axis=1)                 # [H, 512]
        wh, wl = _q8hl(wv_f, S_W)
        wvv_h = wh.reshape(KC, 128, 2, 256).transpose(1, 2, 0, 3)
        wvv_l = wl.reshape(KC, 128, 2, 256).transpose(1, 2, 0, 3)
        wv_np = np.empty((128, 2, KC, 2, 256), ml_dtypes.float8_e4m3)
        wv_np[:, :, :, 0, :] = wvv_l
        wv_np[:, :, :, 1, :] = wvv_h
        wv_np = np.ascontiguousarray(wv_np.reshape(128, -1))

        wo_f = W_o[h0 * D:(h0 + HC) * D, :]               # [512, H]
        wh, wl = _q8hl(wo_f, S_W)
        wov_h = wh.reshape(HC, 128, H).transpose(1, 0, 2)
        wov_l = wl.reshape(HC, 128, H).transpose(1, 0, 2)
        wo_np = np.empty((128, HC, 2, H), ml_dtypes.float8_e4m3)
        wo_np[:, :, 0, :] = wov_h
        wo_np[:, :, 1, :] = wov_l
        wo_np = np.ascontiguousarray(wo_np.reshape(128, -1))

        in_maps.append({
            "xq": xq_np, "wqk": wqk_np, "wv": wv_np, "wo": wo_np,
            "cosT": cosT_np, "sinS": sinS_np, "maskD": maskD_np,
        })
    return in_maps


def kernel(hidden_states, W_pack, W_o, attention_mask, position_ids):
    if "nc" not in _CACHE:
        _CACHE["nc"] = _build_module()
    nc = _CACHE["nc"]
    in_maps = _host_prep(hidden_states, W_pack, W_o, attention_mask, position_ids)
    res = bass_utils.run_bass_kernel_spmd(nc, in_maps, core_ids=list(range(N_CORES)))
    out = res.results[0]["out_p"].astype(np.float32)
    for c in range(1, N_CORES):
        out += res.results[c]["out_p"]
    out *= 1.0 / (S_A * S_W)
    return out.reshape(B, S, H).astype(np.float32)


# revision 85
# speedup vs baseline: 1.0307x; 1.0012x over previous
"""Trainium2 Bass kernel for nn_Attention_60567628808865.

Dense transformer attention block (B=4, S=1024, H=4096, NH=32, D=128):
  qkv = x @ W_pack; RoPE(q, k); causal-masked softmax attention; out @ W_o.

Sharding: tensor-parallel over heads across 8 NeuronCores. Each core computes
4 heads end-to-end; the host sums the 8 partial W_o outputs (row-sharded W_o).

Precision/performance scheme (validated on host to rel_err ~2.7e-3 vs the
2e-2 gate):
  - QKV and W_o projections run in fp8(e4m3) with the DoubleRow perf mode
    (K=256 per instruction, 0.5 cycles/row) using an exact-style two-term
    decomposition: x@W ~= x_hi@W_hi + [x_hi@W_lo + x_lo@W_hi], where
    t_hi = fp8(t*s), t_lo = fp8(t*s - t_hi). Both terms accumulate into ONE
    PSUM chain (identical scale), so the epilogue is unchanged. 48 DoubleRow
    instructions replace 32 f32r instructions per [128col x 256tok] unit:
    0.75x PE cycles.
  - hi/lo operands are slot-interleaved in a single packed tensor
    ([part, chunk, 2, free]) so the correction chain reads (hi,lo) slot pairs
    and the main chain reads (hi,hi) chunk pairs from the same SBUF bytes.
  - Attention is causal-aware: score/PV/denominator work is emitted only for
    the 20/32 key-tile x query-block units on or below the diagonal; the
    second diagonal tile of each query block runs at half moving-width (its
    lower query half is fully masked), and both diagonal triangles are masked
    in-place with one strided DVE multiply against a single host-built
    exp(mask) triangle (mask asserted causal). q/k/v round-trip DRAM in bf16;
    scores/PV matmuls run in bf16 (same PE rate as f32r, half the DMA).
  - Softmax is unnormalized; denominators come from a (1/s_a)-vector matmul
    accumulated in PSUM; the reciprocal is broadcast across partitions on
    GpSimd (partition_broadcast - no PE matmul, no PSUM bank), and the
    normalized attention is quantized to fp8 hi/lo pairs on the fly (hi/lo
    writes on GpSimd) for the W_o DoubleRow chain.
  - Output partials are stored bf16; the host sum applies the global descale.
  - Engine budget per head in attention: PE ~5.8us, Act (exp) ~5.3us,
    DVE (rope-free here: recip/attn-copy/t-mult/masks) ~4.7us, Pool
    (broadcast/hi/lo) ~4.8us.
"""
import numpy as np

import concourse.bass as bass  # noqa: F401
import concourse.tile as tile
from contextlib import ExitStack
from concourse import bacc, mybir
from concourse import bass_utils

F32 = mybir.dt.float32
F32R = mybir.dt.float32r
BF16 = mybir.dt.bfloat16
F8 = mybir.dt.float8e4
AF = mybir.ActivationFunctionType
ALU = mybir.AluOpType
DR = mybir.MatmulPerfMode.DoubleRow

B, S, H, NH = 4, 1024, 4096, 32
D = H // NH          # 128
T = B * S            # 4096 tokens
N_CORES = 8
HC = NH // N_CORES   # 4 heads per core
SCALE = float(1.0 / np.sqrt(D))
ROPE_BASE = 10000.0

TB = 256             # phase-1 token block
NTB = T // TB        # 16
KC = H // 128        # 32 fp8 k-chunks of 128 features
S_X = 32.0           # x quant scale
S_W = 2048.0         # W_pack / W_o quant scale
S_A = 32.0           # attention-output quant scale
DESCALE = 1.0 / (S_X * S_W)

_CACHE = {}


def _build_module(phases=("p1", "p2")):
    nc = bacc.Bacc("TRN2", target_bir_lowering=False, debug=False,
                   num_devices=N_CORES)

    # packed fp8 inputs (see _host_prep for layouts)
    xq = nc.dram_tensor("xq", [128, NTB * KC * 2 * TB], F8, kind="ExternalInput").ap()
    wqk = nc.dram_tensor("wqk", [128, 8 * KC * 2 * 128], F8, kind="ExternalInput").ap()
    wv = nc.dram_tensor("wv", [128, 2 * KC * 2 * 256], F8, kind="ExternalInput").ap()
    wo = nc.dram_tensor("wo", [128, HC * 2 * H], F8, kind="ExternalInput").ap()
    cosT = nc.dram_tensor("cosT", [128, T], F32, kind="ExternalInput").ap()
    sinS = nc.dram_tensor("sinS", [128, T], F32, kind="ExternalInput").ap()
    maskD = nc.dram_tensor("maskD", [128, 256], BF16, kind="ExternalInput").ap()
    out_p = nc.dram_tensor("out_p", [T, H], BF16, kind="ExternalOutput").ap()

    import ml_dtypes
    # denominator ones-vector carries 1/S_A so the reciprocal yields
    # S_A/denom directly (the fp8 attn quant scale)
    ones128 = nc.inline_tensor(
        np.full((128, 1), 1.0 / S_A, ml_dtypes.bfloat16), "ones128").ap()

    with tile.TileContext(nc) as tc, \
         nc.allow_low_precision(reason="fp8/bf16 matmuls; verified vs reference"):
        with ExitStack() as octx:
            dram = octx.enter_context(tc.tile_pool(name="dram", bufs=1, space="DRAM"))
            cpool = octx.enter_context(tc.tile_pool(name="consts", bufs=1))
            # DRAM scratch: qkT rows ordered [q0,k0,q1,k1,q2,k2,q3,k3] x d
            qkT_d = dram.tile([8 * 128, T], BF16)
            v_d = dram.tile([T, HC * 128], BF16)

            # consts are tiny but each DMA costs ~625ns of FIFO-head issue
            # time: defer them behind the critical startup loads
            o128 = cpool.tile([128, 1], BF16)
            mask_t = cpool.tile([128, 256], BF16)
            _consts = [False]

            def load_consts():
                if not _consts[0]:
                    nc.sync.dma_start(o128[:], ones128[:])
                    nc.sync.dma_start(mask_t[:], maskD[:])
                    _consts[0] = True

            # phase-2 tiles prefetched during phase 1 (wo_a has no deps; the
            # first head's kq/vt depend on the tb0-3 scratch stores)
            wopool = octx.enter_context(tc.tile_pool(name="p2wo", bufs=1))
            kqpool = octx.enter_context(tc.tile_pool(name="p2kq", bufs=2))
            vtpool = octx.enter_context(tc.tile_pool(name="p2vt", bufs=2))
            _wo_a = [None]
            _first_kv = [None]

            def load_kv(b, l):
                bs = b * S
                kq = kqpool.tile([128, 2, S], BF16, tag="kq")
                nc.sync.dma_start(
                    kq[:],
                    qkT_d[l * 256:(l + 1) * 256, bs:bs + S]
                        .rearrange("(j p) t -> p j t", p=128))
                vt = vtpool.tile([128, 8, 128], BF16, tag="vt")
                nc.sync.dma_start(
                    vt[:],
                    v_d[bs:bs + S, l * 128:(l + 1) * 128]
                        .rearrange("(kt p) d -> p kt d", p=128))
                return kq, vt

            def prefetch_wo():
                # W_o resident: [128, h(4), j(2), c(4096)]; j=0 -> hi, 1 -> lo
                wo_a = wopool.tile([128, HC, 2, H], F8, tag="wo")
                for h in range(HC):
                    nc.sync.dma_start(
                        wo_a[:, h],
                        wo[:, h * 2 * H:(h + 1) * 2 * H]
                            .rearrange("p (j c) -> p j c", j=2))
                _wo_a[0] = wo_a

            def prefetch_kv():
                _first_kv[0] = load_kv(0, 0)

            # ---------------- Phase 1: QKV projection (fp8 DoubleRow) -------
            if "p1" in phases:
              with ExitStack() as ctx:
                wpool = ctx.enter_context(tc.tile_pool(name="p1w", bufs=1))
                xpool = ctx.enter_context(tc.tile_pool(name="p1x", bufs=2))
                opool = ctx.enter_context(tc.tile_pool(name="p1o", bufs=2))
                cspool = ctx.enter_context(tc.tile_pool(name="p1cs", bufs=2))
                rpool = ctx.enter_context(tc.tile_pool(name="p1rope", bufs=3))
                pqk = ctx.enter_context(tc.tile_pool(name="p1pqk", bufs=4, space="PSUM"))
                pv = ctx.enter_context(tc.tile_pool(name="p1pv", bufs=2, space="PSUM"))

                def load_tb(tb):
                    t0 = tb * TB
                    # x pack [128, kk(32), j(2), t(256)]; j=0 -> x_hi, j=1 -> x_lo
                    xall = xpool.tile([128, KC, 2, TB], F8, tag="x")
                    nc.sync.dma_start(
                        xall[:],
                        xq[:, tb * 16384:(tb + 1) * 16384]
                            .rearrange("p (kk j t) -> p kk j t", kk=KC, j=2))
                    cos_tb = cspool.tile([128, TB], F32, tag="cos")
                    nc.sync.dma_start(cos_tb[:], cosT[:, t0:t0 + TB])
                    sin_tb = cspool.tile([128, TB], F32, tag="sin")
                    nc.sync.dma_start(sin_tb[:], sinS[:, t0:t0 + TB])
                    return xall, cos_tb, sin_tb

                # tb0 inputs first (first chain needs x + wqk ct0 only), then
                # resident weights: wqk [128, ct(8), kk(32), j(2), c(128)],
                # wv [128, ct(2), kk(32), j(2), c(256)]; j=0 -> W_lo, j=1 -> W_hi
                tb0_inputs = load_tb(0)
                wqk_a = wpool.tile([128, 8, KC, 2, 128], F8, tag="wqk")
                wv_a = wpool.tile([128, 2, KC, 2, 256], F8, tag="wv")
                for ct in range(8):
                    nc.sync.dma_start(
                        wqk_a[:, ct],
                        wqk[:, ct * 8192:(ct + 1) * 8192]
                            .rearrange("p (kk j c) -> p kk j c", kk=KC, j=2))
                # tb1's x before wv: qk(1) needs it sooner than v(0) needs wv
                tb1_inputs = load_tb(1)
                for cv in range(2):
                    nc.sync.dma_start(
                        wv_a[:, cv],
                        wv[:, cv * 16384:(cv + 1) * 16384]
                            .rearrange("p (kk j c) -> p kk j c", kk=KC, j=2))
                load_consts()

                def emit_qk(xall, cos_tb, sin_tb, t0):
                    qs_all = opool.tile([128, 8, TB], BF16, tag="qs")
                    for i in range(8):
                        ps = pqk.tile([128, TB], F32, tag="qk")
                        for c in range(16):
                            nc.tensor.matmul(
                                ps[:], wqk_a[:, i, 2 * c:2 * c + 2, 1, :],
                                xall[:, 2 * c:2 * c + 2, 0, :],
                                start=(c == 0), stop=False, perf_mode=DR)
                        for kk in range(KC):
                            nc.tensor.matmul(
                                ps[:], wqk_a[:, i, kk, :, :],
                                xall[:, kk, :, :],
                                start=False, stop=(kk == KC - 1), perf_mode=DR)
                        # RoPE epilogue (psum scale folded into cos/sin tables)
                        rot = rpool.tile([128, TB], F32, tag="rot")
                        nc.scalar.copy(rot[0:64, :], ps[64:128, :])
                        nc.vector.tensor_copy(rot[64:128, :], ps[0:64, :])
                        m1_ = rpool.tile([128, TB], F32, tag="m1")
                        nc.vector.tensor_tensor(m1_[:], ps[:], cos_tb[:], op=ALU.mult)
                        m2_ = rpool.tile([128, TB], F32, tag="m2")
                        nc.vector.tensor_tensor(m2_[:], rot[:], sin_tb[:], op=ALU.mult)
                        nc.vector.tensor_tensor(qs_all[:, i, :], m1_[:], m2_[:],
                                                op=ALU.add)
                    nc.sync.dma_start(
                        qkT_d[:, t0:t0 + TB].rearrange("(i p) t -> p i t", p=128),
                        qs_all[:])

                def emit_v(xall, t0):
                    vs_all = opool.tile([128, 2, 2, 256], BF16, tag="vs")
                    for th in range(2):
                        for ch in range(2):
                            ps = pv.tile([128, 256], F32, tag="v")
                            for c in range(16):
                                nc.tensor.matmul(
                                    ps[:],
                                    xall[:, 2 * c:2 * c + 2, 0,
                                         th * 128:(th + 1) * 128],
                                    wv_a[:, ch, 2 * c:2 * c + 2, 1, :],
                                    start=(c == 0), stop=False, perf_mode=DR)
                            for kk in range(KC):
                                nc.tensor.matmul(
                                    ps[:],
                                    xall[:, kk, :, th * 128:(th + 1) * 128],
                                    wv_a[:, ch, kk, :, :],
                                    start=False, stop=(kk == KC - 1), perf_mode=DR)
                            nc.scalar.activation(vs_all[:, th, ch, :], ps[:],
                                                 AF.Copy, scale=DESCALE)
                    nc.sync.dma_start(
                        v_d[t0:t0 + TB, :]
                            .rearrange("(th p) (ch c) -> p th ch c", p=128, ch=2),
                        vs_all[:])

                # v(0) is deferred until after qk(1): tb0's PE work then
                # needs only x+wqk, hiding the wv weight-load latency
                deferred_v0 = [None]
                for tb in range(NTB):
                    t0 = tb * TB
                    if tb == 0:
                        xall, cos_tb, sin_tb = tb0_inputs
                    elif tb == 1:
                        xall, cos_tb, sin_tb = tb1_inputs
                    else:
                        xall, cos_tb, sin_tb = load_tb(tb)
                    if tb == 2:
                        prefetch_wo()
                    elif tb == 4:
                        prefetch_kv()
                    emit_qk(xall, cos_tb, sin_tb, t0)
                    if tb == 0:
                        deferred_v0[0] = (xall, t0)
                    else:
                        if deferred_v0[0] is not None:
                            emit_v(*deferred_v0[0])
                            deferred_v0[0] = None
                        emit_v(xall, t0)

            # ---------------- Phase 2: attention + W_o ----------------------
            if "p2" in phases:
              with ExitStack() as ctx:
                apool = ctx.enter_context(tc.tile_pool(name="p2a", bufs=2))
                efpool = ctx.enter_context(tc.tile_pool(name="p2ef", bufs=14))
                tpool = ctx.enter_context(tc.tile_pool(name="p2t", bufs=3))
                rpool2 = ctx.enter_context(tc.tile_pool(name="p2rd", bufs=3))
                opool = ctx.enter_context(tc.tile_pool(name="p2o", bufs=3))
                ps_s = ctx.enter_context(tc.tile_pool(name="p2ps", bufs=3, space="PSUM"))
                ps_av = ctx.enter_context(tc.tile_pool(name="p2pav", bufs=1, space="PSUM"))
                ps_d = ctx.enter_context(tc.tile_pool(name="p2pd", bufs=1, space="PSUM"))
                ps_o = ctx.enter_context(tc.tile_pool(name="p2po", bufs=3, space="PSUM"))

                if _wo_a[0] is None:     # p2-only debug build
                    prefetch_wo()
                    prefetch_kv()
                load_consts()
                wo_a = _wo_a[0]
                kv_stash = {}
                for b in range(B):
                    bs = b * S
                    # attn pack [128, lh(2), l(4), t(1024)]; lh=0 -> lo, 1 -> hi
                    apack = apool.tile([128, 2, HC, S], F8, tag="apack")
                    pending = [None]
                    for l in range(HC):
                        if b == 0 and l == 0:
                            kq, vt = _first_kv[0]
                        elif (b, l) in kv_stash:
                            kq, vt = kv_stash.pop((b, l))
                        else:
                            kq, vt = load_kv(b, l)

                        psd_l = ps_d.tile([1, 512], F32, tag="dbc")
                        psav_l = ps_av.tile([128, 512], F32, tag="av")
                        all_efs = {}

                        def emit_scores(qb):
                            u = 2 * qb + 2
                            q_sl = kq[:, 0, qb * 256:(qb + 1) * 256]
                            efs = [None] * u
                            # diagonal pair first: its exp+mask latency hides
                            # behind the remaining pairs' matmuls
                            for g in [qb] + list(range(qb)):
                                pss = ps_s.tile([128, 512], F32, tag="s")
                                if g == qb:
                                    # diagonal pair: tile A is full; tile B
                                    # only sees the top query half, packed
                                    # right after A so the exp is [128,384]
                                    nc.tensor.matmul(
                                        pss[:, 0:256],
                                        kq[:, 1, 2 * g * 128:(2 * g + 1) * 128],
                                        q_sl, start=True, stop=True)
                                    nc.tensor.matmul(
                                        pss[:, 256:384],
                                        kq[:, 1, (2 * g + 1) * 128:(2 * g + 2) * 128],
                                        q_sl[:, 128:256], start=True, stop=True)
                                    ef = efpool.tile([128, 384], BF16, tag="ef")
                                    nc.scalar.activation(ef[:], pss[:, 0:384],
                                                         AF.Exp, scale=SCALE)
                                    # in-place triangular mask on A's left
                                    # quarter and B's live quarter (same
                                    # pattern), one strided DVE op
                                    quarters = ef[:].rearrange(
                                        "p (g c) -> p g c", g=3)[:, ::2, :]
                                    nc.vector.tensor_tensor(
                                        quarters, quarters,
                                        mask_t[:].rearrange(
                                            "p (j c) -> p j c", j=2),
                                        op=ALU.mult)
                                    efs[2 * g] = ef[:, 0:256]
                                    efs[2 * g + 1] = ef[:, 256:384]
                                else:
                                    for sHalf in range(2):
                                        mt = 2 * g + sHalf
                                        nc.tensor.matmul(
                                            pss[:, sHalf * 256:(sHalf + 1) * 256],
                                            kq[:, 1, mt * 128:(mt + 1) * 128],
                                            q_sl, start=True, stop=True)
                                    ef = efpool.tile([128, 512], BF16, tag="ef")
                                    nc.scalar.activation(ef[:], pss[:], AF.Exp,
                                                         scale=SCALE)
                                    efs[2 * g] = ef[:, 0:256]
                                    efs[2 * g + 1] = ef[:, 256:512]
                            all_efs[qb] = efs

                        def emit_pv(qb):
                            u = 2 * qb + 2
                            efs = all_efs.pop(qb)
                            # masked diagonal units last in the chains
                            order = list(range(2 * qb)) + [2 * qb, 2 * qb + 1]
                            if pending[0] is not None:
                                pending[0]()
                                pending[0] = None
                            # the final (diagonal-B) unit only covers the top
                            # query half: half-width accumulation step
                            psav = psav_l[:, (qb % 2) * 256:(qb % 2 + 1) * 256]
                            for n, mt in enumerate(order):
                                half = mt == 2 * qb + 1
                                nc.tensor.matmul(
                                    psav[:, 128:256] if half else psav,
                                    vt[:, mt, :], efs[mt],
                                    start=(n == 0), stop=(n == u - 1),
                                    skip_group_check=True)
                            psd = psd_l[:, (qb % 2) * 256:(qb % 2 + 1) * 256]
                            for n, mt in enumerate(order):
                                half = mt == 2 * qb + 1
                                nc.tensor.matmul(
                                    psd[:, 128:256] if half else psd,
                                    o128[:], efs[mt],
                                    start=(n == 0), stop=(n == u - 1),
                                    skip_group_check=True)
                            if qb % 2 == 0:
                                return
                            # pair epilogue (qb-1, qb): unnormalized attn to
                            # SBUF (frees the psum bank), denominators to
                            # reciprocal; the normalization + fp8 hi/lo split
                            # is deferred into the next PV block
                            rd = rpool2.tile([1, 512], F32, tag="rd")
                            nc.vector.reciprocal(rd[:], psd_l[:])
                            av_s = tpool.tile([128, 512], F32, tag="avs")
                            nc.vector.tensor_copy(av_s[:], psav_l[:])

                            def make_epilogue(qb=qb, av_s=av_s, rd=rd, l=l,
                                              apack=apack):
                                def emit():
                                    # s_a/denom broadcast across partitions on
                                    # GpSimd: no PE matmul, no PSUM bank
                                    bc = tpool.tile([128, 512], F32, tag="bc")
                                    nc.gpsimd.partition_broadcast(bc[:], rd[:])
                                    t_ = tpool.tile([128, 512], F32, tag="t")
                                    nc.vector.tensor_tensor(t_[:], av_s[:],
                                                            bc[:], op=ALU.mult)
                                    q0 = (qb - 1) * 256
                                    hi = apack[:, 1, l, q0:q0 + 512]
                                    # last head: W_o waits on these writes and
                                    # the Pool queue is ~3us deep, so use DVE
                                    eng = nc.vector if l == HC - 1 else nc.gpsimd
                                    eng.tensor_copy(hi, t_[:])
                                    eng.tensor_tensor(
                                        apack[:, 0, l, q0:q0 + 512], t_[:], hi,
                                        op=ALU.subtract)
                                return emit
                            pending[0] = make_epilogue()

                        # all scores (and their masks) are emitted before any
                        # PV block: every engine queue sees the masks first
                        emit_scores(0)
                        emit_scores(1)
                        emit_scores(2)
                        emit_scores(3)
                        emit_pv(0)
                        emit_pv(1)
                        emit_pv(2)
                        emit_pv(3)
                    # W_o projection for batch b (fp8 DoubleRow main+corr);
                    # two 256-col chains per PSUM bank (bufs=3 keeps copies
                    # off the critical path). The next batch's first-head
                    # kq/vt loads are issued BEFORE the big output stores so
                    # they don't queue behind 8 MB in the DMA FIFO. The last
                    # pair epilogue (l=3, qb 2-3) flushes after the first
                    # m-block, which only reads early tokens.
                    if b + 1 < B:
                        kv_stash[(b + 1, 0)] = load_kv(b + 1, 0)
                    for m in range(8):
                        osb = opool.tile([128, 8, 512], BF16, tag="osb")
                        msl = slice(m * 128, (m + 1) * 128)
                        for pair in range(8):
                            pso = ps_o.tile([128, 512], F32, tag="o")
                            for part in range(2):
                                csl = slice((2 * pair + part) * 256,
                                            (2 * pair + part + 1) * 256)
                                po = pso[:, part * 256:(part + 1) * 256]
                                for c in range(2):
                                    nc.tensor.matmul(
                                        po, apack[:, 1, 2 * c:2 * c + 2, msl],
                                        wo_a[:, 2 * c:2 * c + 2, 0, csl],
                                        start=(c == 0), stop=False, perf_mode=DR)
                                for h in range(HC):
                                    nc.tensor.matmul(
                                        po, apack[:, :, h, msl],
                                        wo_a[:, h, :, csl],
                                        start=False, stop=(h == HC - 1),
                                        perf_mode=DR)
                            if pair % 2 == 0 or m >= 6:
                                # tail m-blocks go entirely to DVE (GpSimd has
                                # no PSUM access): keeps Act free for the next
                                # batch's first exps
                                nc.vector.tensor_copy(osb[:, pair, :], pso[:])
                            else:
                                nc.scalar.copy(osb[:, pair, :], pso[:])
                            if b == B - 1 and m == 7:
                                # stream the final stores per pair so the
                                # end-of-kernel drain tail is minimal
                                nc.sync.dma_start(
                                    out_p[bs + m * 128:bs + (m + 1) * 128,
                                          pair * 512:(pair + 1) * 512],
                                    osb[:, pair, :])
                        if b != B - 1 or m != 7:
                            nc.sync.dma_start(
                                out_p[bs + m * 128:bs + (m + 1) * 128, :],
                                osb[:].rearrange("p nc c -> p (nc c)"))
                        if m == 0 and pending[0] is not None:
                            pending[0]()
                            pending[0] = None
    nc.compile()
    return nc


def _q8hl(a, scale):
    """Quantize to fp8 e4m3 hi/lo pair at a shared scale."""
    import ml_dtypes
    hi = (a * scale).astype(ml_dtypes.float8_e4m3)
    lo = ((a * scale) - hi.astype(np.float32)).astype(ml_dtypes.float8_e4m3)
    return hi, lo


def _host_prep(hidden_states, W_pack, W_o, attention_mask, position_ids):
    import ml_dtypes
    x = np.asarray(hidden_states, dtype=np.float32).reshape(T, H)
    W_pack = np.asarray(W_pack, dtype=np.float32)
    W_o = np.asarray(W_o, dtype=np.float32)
    mask = np.asarray(attention_mask, dtype=np.float32)
    pos = np.asarray(position_ids)

    # causal structure is hardcoded in the kernel; verify it holds
    m0 = mask[0, 0]
    iu = np.triu_indices(S, 1)
    assert (m0[iu] < -1e8).all() and (np.tril(m0) == 0).all(), \
        "kernel requires the standard causal mask"

    # x pack: [128p, tb, kk, j(hi,lo), t] -> flat [128, NTB*KC*2*TB]
    xh, xl = _q8hl(x, S_X)
    xv_h = xh.reshape(NTB, TB, KC, 128).transpose(3, 0, 2, 1)
    xv_l = xl.reshape(NTB, TB, KC, 128).transpose(3, 0, 2, 1)
    xq_np = np.empty((128, NTB, KC, 2, TB), ml_dtypes.float8_e4m3)
    xq_np[:, :, :, 0, :] = xv_h
    xq_np[:, :, :, 1, :] = xv_l
    xq_np = np.ascontiguousarray(xq_np.reshape(128, -1))

    # rope tables with the fp8 descale folded in; rotate-half sign in sinS
    inv = 1.0 / (ROPE_BASE ** (np.arange(0, D, 2, dtype=np.float64) / D))
    inv = np.concatenate([inv, inv])
    ang = pos.astype(np.float64).reshape(T)[None, :] * inv[:, None]   # [D, T]
    cosT_np = np.ascontiguousarray((np.cos(ang) * DESCALE).astype(np.float32))
    sinT = (np.sin(ang) * DESCALE).astype(np.float32)
    sinS_np = sinT.copy()
    sinS_np[:64] = -sinT[:64]
    sinS_np = np.ascontiguousarray(sinS_np)

    # diagonal exp-mask triangle [128p(key), 128(query)], duplicated so one
    # strided DVE op covers both diagonal tiles' live quarters
    em = np.exp(m0)
    tri = em[0:128, 0:128].T.astype(ml_dtypes.bfloat16)   # [p(key), t(query)]
    maskD_np = np.ascontiguousarray(
        np.concatenate([tri, tri], axis=1))               # [128, 256]

    in_maps = []
    for core in range(N_CORES):
        h0 = core * HC
        # wqk cols ordered [q0,k0,q1,k1,q2,k2,q3,k3] per head slice
        cols = []
        for l in range(HC):
            cols.append(W_pack[:, (h0 + l) * D:(h0 + l + 1) * D])
            cols.append(W_pack[:, H + (h0 + l) * D:H + (h0 + l + 1) * D])
        wqk_f = np.concatenate(cols, axis=1)              # [H, 1024]
        wh, wl = _q8hl(wqk_f, S_W)
        wv_h = wh.reshape(KC, 128, 8, 128).transpose(1, 2, 0, 3)
        wv_l = wl.reshape(KC, 128, 8, 128).transpose(1, 2, 0, 3)
        wqk_np = np.empty((128, 8, KC, 2, 128), ml_dtypes.float8_e4m3)
        wqk_np[:, :, :, 0, :] = wv_l
        wqk_np[:, :, :, 1, :] = wv_h
        wqk_np = np.ascontiguousarray(wqk_np.reshape(128, -1))

        wv_f = np.concatenate(
            [W_pack[:, 2 * H + (h0 + l) * D:2 * H + (h0 + l + 1) * D]
             for l in range(HC)], axis=1)                 # [H, 512]
        wh, wl = _q8hl(wv_f, S_W)
        wvv_h = wh.reshape(KC, 128, 2, 256).transpose(1, 2, 0, 3)
        wvv_l = wl.reshape(KC, 128, 2, 256).transpose(1, 2, 0, 3)
        wv_np = np.empty((128, 2, KC, 2, 256), ml_dtypes.float8_e4m3)
        wv_np[:, :, :, 0, :] = wvv_l
        wv_np[:, :, :, 1, :] = wvv_h
        wv_np = np.ascontiguousarray(wv_np.reshape(128, -1))

        wo_f = W_o[h0 * D:(h0 + HC) * D, :]               # [512, H]
        wh, wl = _q8hl(wo_f, S_W)
        wov_h = wh.reshape(HC, 128, H).transpose(1, 0, 2)
        wov_l = wl.reshape(HC, 128, H).transpose(1, 0, 2)
        wo_np = np.empty((128, HC, 2, H), ml_dtypes.float8_e4m3)
        wo_np[:, :, 0, :] = wov_h
        wo_np[:, :, 1, :] = wov_l
        wo_np = np.ascontiguousarray(wo_np.reshape(128, -1))

        in_maps.append({
            "xq": xq_np, "wqk": wqk_np, "wv": wv_np, "wo": wo_np,
            "cosT": cosT_np, "sinS": sinS_np, "maskD": maskD_np,
        })
    return in_maps


def kernel(hidden_states, W_pack, W_o, attention_mask, position_ids):
    if "nc" not in _CACHE:
        _CACHE["nc"] = _build_module()
    nc = _CACHE["nc"]
    in_maps = _host_prep(hidden_states, W_pack, W_o, attention_mask, position_ids)
    res = bass_utils.run_bass_kernel_spmd(nc, in_maps, core_ids=list(range(N_CORES)))
    out = res.results[0]["out_p"].astype(np.float32)
    for c in range(1, N_CORES):
        out += res.results[c]["out_p"]
    out *= 1.0 / (S_A * S_W)
    return out.reshape(B, S, H).astype(np.float32)


# revision 86
# speedup vs baseline: 1.0326x; 1.0018x over previous
"""Trainium2 Bass kernel for nn_Attention_60567628808865.

Dense transformer attention block (B=4, S=1024, H=4096, NH=32, D=128):
  qkv = x @ W_pack; RoPE(q, k); causal-masked softmax attention; out @ W_o.

Sharding: tensor-parallel over heads across 8 NeuronCores. Each core computes
4 heads end-to-end; the host sums the 8 partial W_o outputs (row-sharded W_o).

Precision/performance scheme (validated on host to rel_err ~2.7e-3 vs the
2e-2 gate):
  - QKV and W_o projections run in fp8(e4m3) with the DoubleRow perf mode
    (K=256 per instruction, 0.5 cycles/row) using an exact-style two-term
    decomposition: x@W ~= x_hi@W_hi + [x_hi@W_lo + x_lo@W_hi], where
    t_hi = fp8(t*s), t_lo = fp8(t*s - t_hi). Both terms accumulate into ONE
    PSUM chain (identical scale), so the epilogue is unchanged. 48 DoubleRow
    instructions replace 32 f32r instructions per [128col x 256tok] unit:
    0.75x PE cycles.
  - hi/lo operands are slot-interleaved in a single packed tensor
    ([part, chunk, 2, free]) so the correction chain reads (hi,lo) slot pairs
    and the main chain reads (hi,hi) chunk pairs from the same SBUF bytes.
  - Attention is causal-aware: score/PV/denominator work is emitted only for
    the 20/32 key-tile x query-block units on or below the diagonal; the
    second diagonal tile of each query block runs at half moving-width (its
    lower query half is fully masked), and both diagonal triangles are masked
    in-place with one strided DVE multiply against a single host-built
    exp(mask) triangle (mask asserted causal). q/k/v round-trip DRAM in bf16;
    scores/PV matmuls run in bf16 (same PE rate as f32r, half the DMA).
  - Softmax is unnormalized; denominators come from a (1/s_a)-vector matmul
    accumulated in PSUM; the reciprocal is broadcast across partitions on
    GpSimd (partition_broadcast - no PE matmul, no PSUM bank), and the
    normalized attention is quantized to fp8 hi/lo pairs on the fly (hi/lo
    writes on GpSimd) for the W_o DoubleRow chain.
  - Output partials are stored bf16; the host sum applies the global descale.
  - Engine budget per head in attention: PE ~5.8us, Act (exp) ~5.3us,
    DVE (rope-free here: recip/attn-copy/t-mult/masks) ~4.7us, Pool
    (broadcast/hi/lo) ~4.8us.
"""
import numpy as np

import concourse.bass as bass  # noqa: F401
import concourse.tile as tile
from contextlib import ExitStack
from concourse import bacc, mybir
from concourse import bass_utils

F32 = mybir.dt.float32
F32R = mybir.dt.float32r
BF16 = mybir.dt.bfloat16
F8 = mybir.dt.float8e4
AF = mybir.ActivationFunctionType
ALU = mybir.AluOpType
DR = mybir.MatmulPerfMode.DoubleRow

B, S, H, NH = 4, 1024, 4096, 32
D = H // NH          # 128
T = B * S            # 4096 tokens
N_CORES = 8
HC = NH // N_CORES   # 4 heads per core
SCALE = float(1.0 / np.sqrt(D))
ROPE_BASE = 10000.0

TB = 256             # phase-1 token block
NTB = T // TB        # 16
KC = H // 128        # 32 fp8 k-chunks of 128 features
S_X = 32.0           # x quant scale
S_W = 2048.0         # W_pack / W_o quant scale
S_A = 32.0           # attention-output quant scale
DESCALE = 1.0 / (S_X * S_W)

_CACHE = {}


def _build_module(phases=("p1", "p2")):
    nc = bacc.Bacc("TRN2", target_bir_lowering=False, debug=False,
                   num_devices=N_CORES)

    # packed fp8 inputs (see _host_prep for layouts)
    xq = nc.dram_tensor("xq", [128, NTB * KC * 2 * TB], F8, kind="ExternalInput").ap()
    wqk = nc.dram_tensor("wqk", [128, 8 * KC * 2 * 128], F8, kind="ExternalInput").ap()
    wv = nc.dram_tensor("wv", [128, 2 * KC * 2 * 256], F8, kind="ExternalInput").ap()
    wo = nc.dram_tensor("wo", [128, HC * 2 * H], F8, kind="ExternalInput").ap()
    cosT = nc.dram_tensor("cosT", [128, T], F32, kind="ExternalInput").ap()
    sinS = nc.dram_tensor("sinS", [128, T], F32, kind="ExternalInput").ap()
    maskD = nc.dram_tensor("maskD", [128, 256], BF16, kind="ExternalInput").ap()
    out_p = nc.dram_tensor("out_p", [T, H], BF16, kind="ExternalOutput").ap()

    import ml_dtypes
    # denominator ones-vector carries 1/S_A so the reciprocal yields
    # S_A/denom directly (the fp8 attn quant scale)
    ones128 = nc.inline_tensor(
        np.full((128, 1), 1.0 / S_A, ml_dtypes.bfloat16), "ones128").ap()

    with tile.TileContext(nc) as tc, \
         nc.allow_low_precision(reason="fp8/bf16 matmuls; verified vs reference"):
        with ExitStack() as octx:
            dram = octx.enter_context(tc.tile_pool(name="dram", bufs=1, space="DRAM"))
            cpool = octx.enter_context(tc.tile_pool(name="consts", bufs=1))
            # DRAM scratch: qkT rows ordered [q0,k0,q1,k1,q2,k2,q3,k3] x d
            qkT_d = dram.tile([8 * 128, T], BF16)
            v_d = dram.tile([T, HC * 128], BF16)

            # consts are tiny but each DMA costs ~625ns of FIFO-head issue
            # time: defer them behind the critical startup loads
            o128 = cpool.tile([128, 1], BF16)
            mask_t = cpool.tile([128, 256], BF16)
            _consts = [False]

            def load_consts():
                if not _consts[0]:
                    nc.sync.dma_start(o128[:], ones128[:])
                    nc.sync.dma_start(mask_t[:], maskD[:])
                    _consts[0] = True

            # phase-2 tiles prefetched during phase 1 (wo_a has no deps; the
            # first head's kq/vt depend on the tb0-3 scratch stores)
            wopool = octx.enter_context(tc.tile_pool(name="p2wo", bufs=1))
            kqpool = octx.enter_context(tc.tile_pool(name="p2kq", bufs=2))
            vtpool = octx.enter_context(tc.tile_pool(name="p2vt", bufs=2))
            _wo_a = [None]
            _first_kv = [None]

            def load_kv(b, l):
                bs = b * S
                kq = kqpool.tile([128, 2, S], BF16, tag="kq")
                nc.sync.dma_start(
                    kq[:],
                    qkT_d[l * 256:(l + 1) * 256, bs:bs + S]
                        .rearrange("(j p) t -> p j t", p=128))
                vt = vtpool.tile([128, 8, 128], BF16, tag="vt")
                nc.sync.dma_start(
                    vt[:],
                    v_d[bs:bs + S, l * 128:(l + 1) * 128]
                        .rearrange("(kt p) d -> p kt d", p=128))
                return kq, vt

            def prefetch_wo():
                # W_o resident: [128, h(4), j(2), c(4096)]; j=0 -> hi, 1 -> lo
                wo_a = wopool.tile([128, HC, 2, H], F8, tag="wo")
                for h in range(HC):
                    nc.sync.dma_start(
                        wo_a[:, h],
                        wo[:, h * 2 * H:(h + 1) * 2 * H]
                            .rearrange("p (j c) -> p j c", j=2))
                _wo_a[0] = wo_a

            def prefetch_kv():
                _first_kv[0] = load_kv(0, 0)

            # ---------------- Phase 1: QKV projection (fp8 DoubleRow) -------
            if "p1" in phases:
              with ExitStack() as ctx:
                wpool = ctx.enter_context(tc.tile_pool(name="p1w", bufs=1))
                xpool = ctx.enter_context(tc.tile_pool(name="p1x", bufs=2))
                opool = ctx.enter_context(tc.tile_pool(name="p1o", bufs=2))
                cspool = ctx.enter_context(tc.tile_pool(name="p1cs", bufs=2))
                rpool = ctx.enter_context(tc.tile_pool(name="p1rope", bufs=3))
                pqk = ctx.enter_context(tc.tile_pool(name="p1pqk", bufs=4, space="PSUM"))
                pv = ctx.enter_context(tc.tile_pool(name="p1pv", bufs=2, space="PSUM"))

                def load_tb(tb):
                    t0 = tb * TB
                    # x pack [128, kk(32), j(2), t(256)]; j=0 -> x_hi, j=1 -> x_lo
                    xall = xpool.tile([128, KC, 2, TB], F8, tag="x")
                    nc.sync.dma_start(
                        xall[:],
                        xq[:, tb * 16384:(tb + 1) * 16384]
                            .rearrange("p (kk j t) -> p kk j t", kk=KC, j=2))
                    cos_tb = cspool.tile([128, TB], F32, tag="cos")
                    nc.sync.dma_start(cos_tb[:], cosT[:, t0:t0 + TB])
                    sin_tb = cspool.tile([128, TB], F32, tag="sin")
                    nc.sync.dma_start(sin_tb[:], sinS[:, t0:t0 + TB])
                    return xall, cos_tb, sin_tb

                # tb0 inputs first (first chain needs x + wqk ct0 only), then
                # resident weights: wqk [128, ct(8), kk(32), j(2), c(128)],
                # wv [128, ct(2), kk(32), j(2), c(256)]; j=0 -> W_lo, j=1 -> W_hi
                tb0_inputs = load_tb(0)
                wqk_a = wpool.tile([128, 8, KC, 2, 128], F8, tag="wqk")
                wv_a = wpool.tile([128, 2, KC, 2, 256], F8, tag="wv")
                for ct in range(8):
                    nc.sync.dma_start(
                        wqk_a[:, ct],
                        wqk[:, ct * 8192:(ct + 1) * 8192]
                            .rearrange("p (kk j c) -> p kk j c", kk=KC, j=2))
                # tb1's x before wv: qk(1) needs it sooner than v(0) needs wv
                tb1_inputs = load_tb(1)
                for cv in range(2):
                    nc.sync.dma_start(
                        wv_a[:, cv],
                        wv[:, cv * 16384:(cv + 1) * 16384]
                            .rearrange("p (kk j c) -> p kk j c", kk=KC, j=2))
                load_consts()

                def emit_qk(xall, cos_tb, sin_tb, t0):
                    qs_all = opool.tile([128, 8, TB], BF16, tag="qs")
                    for i in range(8):
                        ps = pqk.tile([128, TB], F32, tag="qk")
                        for c in range(16):
                            nc.tensor.matmul(
                                ps[:], wqk_a[:, i, 2 * c:2 * c + 2, 1, :],
                                xall[:, 2 * c:2 * c + 2, 0, :],
                                start=(c == 0), stop=False, perf_mode=DR)
                        for kk in range(KC):
                            nc.tensor.matmul(
                                ps[:], wqk_a[:, i, kk, :, :],
                                xall[:, kk, :, :],
                                start=False, stop=(kk == KC - 1), perf_mode=DR)
                        # RoPE epilogue (psum scale folded into cos/sin tables)
                        rot = rpool.tile([128, TB], F32, tag="rot")
                        nc.scalar.copy(rot[0:64, :], ps[64:128, :])
                        nc.vector.tensor_copy(rot[64:128, :], ps[0:64, :])
                        m1_ = rpool.tile([128, TB], F32, tag="m1")
                        nc.vector.tensor_tensor(m1_[:], ps[:], cos_tb[:], op=ALU.mult)
                        m2_ = rpool.tile([128, TB], F32, tag="m2")
                        nc.vector.tensor_tensor(m2_[:], rot[:], sin_tb[:], op=ALU.mult)
                        nc.vector.tensor_tensor(qs_all[:, i, :], m1_[:], m2_[:],
                                                op=ALU.add)
                    nc.sync.dma_start(
                        qkT_d[:, t0:t0 + TB].rearrange("(i p) t -> p i t", p=128),
                        qs_all[:])

                def emit_v(xall, t0):
                    vs_all = opool.tile([128, 2, 2, 256], BF16, tag="vs")
                    for th in range(2):
                        for ch in range(2):
                            ps = pv.tile([128, 256], F32, tag="v")
                            for c in range(16):
                                nc.tensor.matmul(
                                    ps[:],
                                    xall[:, 2 * c:2 * c + 2, 0,
                                         th * 128:(th + 1) * 128],
                                    wv_a[:, ch, 2 * c:2 * c + 2, 1, :],
                                    start=(c == 0), stop=False, perf_mode=DR)
                            for kk in range(KC):
                                nc.tensor.matmul(
                                    ps[:],
                                    xall[:, kk, :, th * 128:(th + 1) * 128],
                                    wv_a[:, ch, kk, :, :],
                                    start=False, stop=(kk == KC - 1), perf_mode=DR)
                            nc.scalar.activation(vs_all[:, th, ch, :], ps[:],
                                                 AF.Copy, scale=DESCALE)
                    nc.sync.dma_start(
                        v_d[t0:t0 + TB, :]
                            .rearrange("(th p) (ch c) -> p th ch c", p=128, ch=2),
                        vs_all[:])

                # v(0) is deferred until after qk(1): tb0's PE work then
                # needs only x+wqk, hiding the wv weight-load latency
                deferred_v0 = [None]
                for tb in range(NTB):
                    t0 = tb * TB
                    if tb == 0:
                        xall, cos_tb, sin_tb = tb0_inputs
                    elif tb == 1:
                        xall, cos_tb, sin_tb = tb1_inputs
                    else:
                        xall, cos_tb, sin_tb = load_tb(tb)
                    if tb == 2:
                        prefetch_wo()
                    elif tb == 4:
                        prefetch_kv()
                    emit_qk(xall, cos_tb, sin_tb, t0)
                    if tb == 0:
                        deferred_v0[0] = (xall, t0)
                    else:
                        if deferred_v0[0] is not None:
                            emit_v(*deferred_v0[0])
                            deferred_v0[0] = None
                        emit_v(xall, t0)

            # ---------------- Phase 2: attention + W_o ----------------------
            if "p2" in phases:
              with ExitStack() as ctx:
                apool = ctx.enter_context(tc.tile_pool(name="p2a", bufs=2))
                efpool = ctx.enter_context(tc.tile_pool(name="p2ef", bufs=14))
                tpool = ctx.enter_context(tc.tile_pool(name="p2t", bufs=3))
                rpool2 = ctx.enter_context(tc.tile_pool(name="p2rd", bufs=3))
                opool = ctx.enter_context(tc.tile_pool(name="p2o", bufs=3))
                ps_s = ctx.enter_context(tc.tile_pool(name="p2ps", bufs=3, space="PSUM"))
                ps_av = ctx.enter_context(tc.tile_pool(name="p2pav", bufs=1, space="PSUM"))
                ps_d = ctx.enter_context(tc.tile_pool(name="p2pd", bufs=1, space="PSUM"))
                ps_o = ctx.enter_context(tc.tile_pool(name="p2po", bufs=3, space="PSUM"))

                if _wo_a[0] is None:     # p2-only debug build
                    prefetch_wo()
                    prefetch_kv()
                load_consts()
                wo_a = _wo_a[0]
                kv_stash = {}
                for b in range(B):
                    bs = b * S
                    # attn pack [128, lh(2), l(4), t(1024)]; lh=0 -> lo, 1 -> hi
                    apack = apool.tile([128, 2, HC, S], F8, tag="apack")
                    pending = [None]
                    for l in range(HC):
                        if b == 0 and l == 0:
                            kq, vt = _first_kv[0]
                        elif (b, l) in kv_stash:
                            kq, vt = kv_stash.pop((b, l))
                        else:
                            kq, vt = load_kv(b, l)

                        psd_l = ps_d.tile([1, 512], F32, tag="dbc")
                        psav_l = ps_av.tile([128, 512], F32, tag="av")
                        all_efs = {}

                        def emit_scores(qb):
                            u = 2 * qb + 2
                            q_sl = kq[:, 0, qb * 256:(qb + 1) * 256]
                            efs = [None] * u
                            # diagonal pair first: its exp+mask latency hides
                            # behind the remaining pairs' matmuls
                            for g in [qb] + list(range(qb)):
                                pss = ps_s.tile([128, 512], F32, tag="s")
                                if g == qb:
                                    # diagonal pair: tile A is full; tile B
                                    # only sees the top query half, packed
                                    # right after A so the exp is [128,384]
                                    nc.tensor.matmul(
                                        pss[:, 0:256],
                                        kq[:, 1, 2 * g * 128:(2 * g + 1) * 128],
                                        q_sl, start=True, stop=True)
                                    nc.tensor.matmul(
                                        pss[:, 256:384],
                                        kq[:, 1, (2 * g + 1) * 128:(2 * g + 2) * 128],
                                        q_sl[:, 128:256], start=True, stop=True)
                                    ef = efpool.tile([128, 384], BF16, tag="ef")
                                    nc.scalar.activation(ef[:], pss[:, 0:384],
                                                         AF.Exp, scale=SCALE)
                                    # in-place triangular mask on A's left
                                    # quarter and B's live quarter (same
                                    # pattern), one strided DVE op
                                    quarters = ef[:].rearrange(
                                        "p (g c) -> p g c", g=3)[:, ::2, :]
                                    nc.vector.tensor_tensor(
                                        quarters, quarters,
                                        mask_t[:].rearrange(
                                            "p (j c) -> p j c", j=2),
                                        op=ALU.mult)
                                    efs[2 * g] = ef[:, 0:256]
                                    efs[2 * g + 1] = ef[:, 256:384]
                                else:
                                    for sHalf in range(2):
                                        mt = 2 * g + sHalf
                                        nc.tensor.matmul(
                                            pss[:, sHalf * 256:(sHalf + 1) * 256],
                                            kq[:, 1, mt * 128:(mt + 1) * 128],
                                            q_sl, start=True, stop=True)
                                    ef = efpool.tile([128, 512], BF16, tag="ef")
                                    nc.scalar.activation(ef[:], pss[:], AF.Exp,
                                                         scale=SCALE)
                                    efs[2 * g] = ef[:, 0:256]
                                    efs[2 * g + 1] = ef[:, 256:512]
                            all_efs[qb] = efs

                        def emit_pv(qb):
                            u = 2 * qb + 2
                            efs = all_efs.pop(qb)
                            # diagonal units first: their exp+mask completes
                            # earliest (the diagonal pair is scored first)
                            order = [2 * qb, 2 * qb + 1] + list(range(2 * qb))
                            if pending[0] is not None:
                                pending[0]()
                                pending[0] = None
                            # the final (diagonal-B) unit only covers the top
                            # query half: half-width accumulation step
                            psav = psav_l[:, (qb % 2) * 256:(qb % 2 + 1) * 256]
                            for n, mt in enumerate(order):
                                half = mt == 2 * qb + 1
                                nc.tensor.matmul(
                                    psav[:, 128:256] if half else psav,
                                    vt[:, mt, :], efs[mt],
                                    start=(n == 0), stop=(n == u - 1),
                                    skip_group_check=True)
                            psd = psd_l[:, (qb % 2) * 256:(qb % 2 + 1) * 256]
                            for n, mt in enumerate(order):
                                half = mt == 2 * qb + 1
                                nc.tensor.matmul(
                                    psd[:, 128:256] if half else psd,
                                    o128[:], efs[mt],
                                    start=(n == 0), stop=(n == u - 1),
                                    skip_group_check=True)
                            if qb % 2 == 0:
                                return
                            # pair epilogue (qb-1, qb): unnormalized attn to
                            # SBUF (frees the psum bank), denominators to
                            # reciprocal; the normalization + fp8 hi/lo split
                            # is deferred into the next PV block
                            rd = rpool2.tile([1, 512], F32, tag="rd")
                            nc.vector.reciprocal(rd[:], psd_l[:])
                            av_s = tpool.tile([128, 512], F32, tag="avs")
                            nc.vector.tensor_copy(av_s[:], psav_l[:])

                            def make_epilogue(qb=qb, av_s=av_s, rd=rd, l=l,
                                              apack=apack):
                                def emit():
                                    # s_a/denom broadcast across partitions on
                                    # GpSimd: no PE matmul, no PSUM bank
                                    bc = tpool.tile([128, 512], F32, tag="bc")
                                    nc.gpsimd.partition_broadcast(bc[:], rd[:])
                                    t_ = tpool.tile([128, 512], F32, tag="t")
                                    nc.vector.tensor_tensor(t_[:], av_s[:],
                                                            bc[:], op=ALU.mult)
                                    q0 = (qb - 1) * 256
                                    hi = apack[:, 1, l, q0:q0 + 512]
                                    # last head: W_o waits on these writes and
                                    # the Pool queue is ~3us deep, so use DVE
                                    eng = nc.vector if l == HC - 1 else nc.gpsimd
                                    eng.tensor_copy(hi, t_[:])
                                    eng.tensor_tensor(
                                        apack[:, 0, l, q0:q0 + 512], t_[:], hi,
                                        op=ALU.subtract)
                                return emit
                            pending[0] = make_epilogue()

                        # all scores (and their masks) are emitted before any
                        # PV block: every engine queue sees the masks first
                        emit_scores(0)
                        emit_scores(1)
                        emit_scores(2)
                        emit_scores(3)
                        emit_pv(0)
                        emit_pv(1)
                        emit_pv(2)
                        emit_pv(3)
                    # W_o projection for batch b (fp8 DoubleRow main+corr);
                    # two 256-col chains per PSUM bank (bufs=3 keeps copies
                    # off the critical path). The next batch's first-head
                    # kq/vt loads are issued BEFORE the big output stores so
                    # they don't queue behind 8 MB in the DMA FIFO. The last
                    # pair epilogue (l=3, qb 2-3) flushes after the first
                    # m-block, which only reads early tokens.
                    if b + 1 < B:
                        kv_stash[(b + 1, 0)] = load_kv(b + 1, 0)
                    for m in range(8):
                        osb = opool.tile([128, 8, 512], BF16, tag="osb")
                        msl = slice(m * 128, (m + 1) * 128)
                        for pair in range(8):
                            pso = ps_o.tile([128, 512], F32, tag="o")
                            for part in range(2):
                                csl = slice((2 * pair + part) * 256,
                                            (2 * pair + part + 1) * 256)
                                po = pso[:, part * 256:(part + 1) * 256]
                                for c in range(2):
                                    nc.tensor.matmul(
                                        po, apack[:, 1, 2 * c:2 * c + 2, msl],
                                        wo_a[:, 2 * c:2 * c + 2, 0, csl],
                                        start=(c == 0), stop=False, perf_mode=DR)
                                for h in range(HC):
                                    nc.tensor.matmul(
                                        po, apack[:, :, h, msl],
                                        wo_a[:, h, :, csl],
                                        start=False, stop=(h == HC - 1),
                                        perf_mode=DR)
                            if pair % 2 == 0 or m >= 6:
                                # tail m-blocks go entirely to DVE (GpSimd has
                                # no PSUM access): keeps Act free for the next
                                # batch's first exps
                                nc.vector.tensor_copy(osb[:, pair, :], pso[:])
                            else:
                                nc.scalar.copy(osb[:, pair, :], pso[:])
                            if b == B - 1 and m == 7:
                                # stream the final stores per pair so the
                                # end-of-kernel drain tail is minimal
                                nc.sync.dma_start(
                                    out_p[bs + m * 128:bs + (m + 1) * 128,
                                          pair * 512:(pair + 1) * 512],
                                    osb[:, pair, :])
                        if b != B - 1 or m != 7:
                            nc.sync.dma_start(
                                out_p[bs + m * 128:bs + (m + 1) * 128, :],
                                osb[:].rearrange("p nc c -> p (nc c)"))
                        if m == 0 and pending[0] is not None:
                            pending[0]()
                            pending[0] = None
    nc.compile()
    return nc


def _q8hl(a, scale):
    """Quantize to fp8 e4m3 hi/lo pair at a shared scale."""
    import ml_dtypes
    hi = (a * scale).astype(ml_dtypes.float8_e4m3)
    lo = ((a * scale) - hi.astype(np.float32)).astype(ml_dtypes.float8_e4m3)
    return hi, lo


def _host_prep(hidden_states, W_pack, W_o, attention_mask, position_ids):
    import ml_dtypes
    x = np.asarray(hidden_states, dtype=np.float32).reshape(T, H)
    W_pack = np.asarray(W_pack, dtype=np.float32)
    W_o = np.asarray(W_o, dtype=np.float32)
    mask = np.asarray(attention_mask, dtype=np.float32)
    pos = np.asarray(position_ids)

    # causal structure is hardcoded in the kernel; verify it holds
    m0 = mask[0, 0]
    iu = np.triu_indices(S, 1)
    assert (m0[iu] < -1e8).all() and (np.tril(m0) == 0).all(), \
        "kernel requires the standard causal mask"

    # x pack: [128p, tb, kk, j(hi,lo), t] -> flat [128, NTB*KC*2*TB]
    xh, xl = _q8hl(x, S_X)
    xv_h = xh.reshape(NTB, TB, KC, 128).transpose(3, 0, 2, 1)
    xv_l = xl.reshape(NTB, TB, KC, 128).transpose(3, 0, 2, 1)
    xq_np = np.empty((128, NTB, KC, 2, TB), ml_dtypes.float8_e4m3)
    xq_np[:, :, :, 0, :] = xv_h
    xq_np[:, :, :, 1, :] = xv_l
    xq_np = np.ascontiguousarray(xq_np.reshape(128, -1))

    # rope tables with the fp8 descale folded in; rotate-half sign in sinS
    inv = 1.0 / (ROPE_BASE ** (np.arange(0, D, 2, dtype=np.float64) / D))
    inv = np.concatenate([inv, inv])
    ang = pos.astype(np.float64).reshape(T)[None, :] * inv[:, None]   # [D, T]
    cosT_np = np.ascontiguousarray((np.cos(ang) * DESCALE).astype(np.float32))
    sinT = (np.sin(ang) * DESCALE).astype(np.float32)
    sinS_np = sinT.copy()
    sinS_np[:64] = -sinT[:64]
    sinS_np = np.ascontiguousarray(sinS_np)

    # diagonal exp-mask triangle [128p(key), 128(query)], duplicated so one
    # strided DVE op covers both diagonal tiles' live quarters
    em = np.exp(m0)
    tri = em[0:128, 0:128].T.astype(ml_dtypes.bfloat16)   # [p(key), t(query)]
    maskD_np = np.ascontiguousarray(
        np.concatenate([tri, tri], axis=1))               # [128, 256]

    in_maps = []
    for core in range(N_CORES):
        h0 = core * HC
        # wqk cols ordered [q0,k0,q1,k1,q2,k2,q3,k3] per head slice
        cols = []
        for l in range(HC):
            cols.append(W_pack[:, (h0 + l) * D:(h0 + l + 1) * D])
            cols.append(W_pack[:, H + (h0 + l) * D:H + (h0 + l + 1) * D])
        wqk_f = np.concatenate(cols, axis=1)              # [H, 1024]
        wh, wl = _q8hl(wqk_f, S_W)
        wv_h = wh.reshape(KC, 128, 8, 128).transpose(1, 2, 0, 3)
        wv_l = wl.reshape(KC, 128, 8, 128).transpose(1, 2, 0, 3)
        wqk_np = np.empty((128, 8, KC, 2, 128), ml_dtypes.float8_e4m3)
        wqk_np[:, :, :, 0, :] = wv_l
        wqk_np[:, :, :, 1, :] = wv_h
        wqk_np = np.ascontiguousarray(wqk_np.reshape(128, -1))

        wv_f = np.concatenate(
            [W_pack[:, 2 * H + (h0 + l) * D:2 * H + (h0 + l + 1) * D]
             for l in range(HC)], axis=1)                 # [H, 512]
        wh, wl = _q8hl(wv_f, S_W)
        wvv_h = wh.reshape(KC, 128, 2, 256).transpose(1, 2, 0, 3)
        wvv_l = wl.reshape(KC, 128, 2, 256).transpose(1, 2, 0, 3)
        wv_np = np.empty((128, 2, KC, 2, 256), ml_dtypes.float8_e4m3)
        wv_np[:, :, :, 0, :] = wvv_l
        wv_np[:, :, :, 1, :] = wvv_h
        wv_np = np.ascontiguousarray(wv_np.reshape(128, -1))

        wo_f = W_o[h0 * D:(h0 + HC) * D, :]               # [512, H]
        wh, wl = _q8hl(wo_f, S_W)
        wov_h = wh.reshape(HC, 128, H).transpose(1, 0, 2)
        wov_l = wl.reshape(HC, 128, H).transpose(1, 0, 2)
        wo_np = np.empty((128, HC, 2, H), ml_dtypes.float8_e4m3)
        wo_np[:, :, 0, :] = wov_h
        wo_np[:, :, 1, :] = wov_l
        wo_np = np.ascontiguousarray(wo_np.reshape(128, -1))

        in_maps.append({
            "xq": xq_np, "wqk": wqk_np, "wv": wv_np, "wo": wo_np,
            "cosT": cosT_np, "sinS": sinS_np, "maskD": maskD_np,
        })
    return in_maps


def kernel(hidden_states, W_pack, W_o, attention_mask, position_ids):
    if "nc" not in _CACHE:
        _CACHE["nc"] = _build_module()
    nc = _CACHE["nc"]
    in_maps = _host_prep(hidden_states, W_pack, W_o, attention_mask, position_ids)
    res = bass_utils.run_bass_kernel_spmd(nc, in_maps, core_ids=list(range(N_CORES)))
    out = res.results[0]["out_p"].astype(np.float32)
    for c in range(1, N_CORES):
        out += res.results[c]["out_p"]
    out *= 1.0 / (S_A * S_W)
    return out.reshape(B, S, H).astype(np.float32)
